# revision 31
# baseline (speedup 1.0000x reference)
"""Trainium2 Bass kernel for a BasicTransformerBlock (self-attn + cross-attn + GEGLU FF).

Sharding: 8 cores = (batch b in 0..3) x (sequence half s in 0..1). No collectives.
Each core receives the full x[b] [512, 2048] (rotated so its local half is always
columns 0..1023), builds self-attention K/V over all 2048 positions, and computes
LN/Q/attention/FF only for its local 1024 positions. Output [512, 1024] per core.

Numerics: bf16 matmuls with fp32 PSUM accumulation; LayerNorm gains folded into the
following weight matrices on the host; attention softmax computed without
max-subtraction (scores are bounded ~+-1.5 here); softmax denominator obtained by
augmenting V^T with a ones-column (row 64 of the AV output = sum_j exp).
"""

import os
import sys

import numpy as np

for _p in ("/opt/trn_rl_repo", "/root/.axon_site/_ro/trn_rl_repo"):
    if os.path.isdir(_p) and _p not in sys.path:
        sys.path.insert(0, _p)

import ml_dtypes

import concourse.bass as bass
import concourse.tile as tile
from concourse import mybir
from concourse.bass_utils import run_bass_kernel_spmd

BF16NP = ml_dtypes.bfloat16
AFT = mybir.ActivationFunctionType
F32 = mybir.dt.float32
BF16 = mybir.dt.bfloat16

# Problem dims (hardcoded per spec)
P = 128
B = 4
C = 512      # model dim
N = 2048     # full seq len
NL = 1024    # local seq len per core
CTXC = 768   # context channels
MCTX = 256   # context seq len
H = 8
DH = 64
INNER = 512
FFI = 2048
EPS = 1e-5

CT = C // P        # 4 channel tiles
IT = INNER // P    # 4 inner tiles
XT = CTXC // P     # 6 ctx channel tiles
FT = FFI // P      # 16 ff tiles
NCH = 512          # free-dim chunk size
ICN = NL // NCH    # 2 local i-chunks
JT1 = N // P       # 16 self-attn j tiles
JT2 = MCTX // P    # 2 cross-attn j tiles
DEBUG = False


def _emit(tc):
    nc = tc.nc
    from contextlib import ExitStack

    with ExitStack() as ctx:
        ctx.enter_context(nc.allow_low_precision(
            reason="bf16 rows/broadcasts validated end-to-end vs fp32 reference"))
        main = ctx.enter_context(tc.tile_pool(name="main", bufs=1))
        tp = ctx.enter_context(tc.tile_pool(name="tp", bufs=4))

        x_d = nc.x_d
        ctx_d = nc.ctx_d
        w_d = nc.w_d
        b_d = nc.b_d
        out_d = nc.out_d

        # ---- constants ----
        ones_col = main.tile([P, 1], F32, tag="ones_col", name="ones_col")
        nc.vector.memset(ones_col, 1.0)
        ones_col_bf = main.tile([P, 1], BF16, tag="ones_col_bf", name="ones_col_bf")
        nc.vector.memset(ones_col_bf, 1.0)
        ones_row = main.tile([1, P], BF16, tag="ones_row", name="ones_row")
        nc.vector.memset(ones_row, 1.0)
        eps_t = main.tile([P, 1], F32, tag="eps", name="eps")
        nc.vector.memset(eps_t, EPS)

        # ---- load weights (attention ones up-front; FF weights later) ----
        def load_split(pool, tag, dram, nkt, cols, dtype):
            """One wide DMA for a [nkt*128, cols] DRAM tensor into a single
            [128, nkt*cols] SBUF tile; returns per-kt [128, cols] views."""
            t = pool.tile([P, nkt * cols], dtype, tag=tag, name=tag)
            nc.sync.dma_start(
                out=t.rearrange("p (kt c) -> p kt c", kt=nkt),
                in_=dram.rearrange("(kt p) c -> p kt c", p=P))
            return [t[:, kt * cols:(kt + 1) * cols] for kt in range(nkt)]

        def load_w(pool, name, nkt, cols):
            return load_split(pool, name, w_d[name], nkt, cols, BF16)


        def load_bias(name, n):
            f = n // P
            t = main.tile([P, f], F32, tag=f"b_{name}", name=f"b_{name}")
            nc.sync.dma_start(out=t, in_=b_d[name].rearrange("(f p) -> p f", p=P))
            return t

        ca_cm = tc.tile_pool(name="ca", bufs=1)
        ca = ca_cm.__enter__()
        sa_cm = tc.tile_pool(name="sa", bufs=1)
        sa = sa_cm.__enter__()
        # ---- load activations (before weights: LN1 needs x first) ----
        xfp_cm = tc.tile_pool(name="xfull", bufs=1)
        xfp = xfp_cm.__enter__()
        # xfull: one [128, CT*N] tile, DMA'd in 4 column-chunks so LN1's
        # first chunk starts as soon as its slice lands
        xft = xfp.tile([P, CT * N], BF16, tag="xf", name="xf")
        _xf_nc = N // NCH
        for cc in range(_xf_nc):
            nc.sync.dma_start(
                out=xft.rearrange("p (kt nc c) -> p nc kt c", kt=CT,
                                  nc=_xf_nc)[:, cc],
                in_=nc.xb_d.rearrange("(kt p) (nc c) -> p nc kt c", p=P,
                                      nc=_xf_nc)[:, cc])
        xfull = [xft[:, kt * N:(kt + 1) * N] for kt in range(CT)]
        xres = load_split(main, "xres", x_d, CT, NL, F32)
        ctx_sb = load_split(main, "ctx", ctx_d, XT, MCTX, BF16)

        # biases + weights after activations so LN1's x tiles arrive first
        bo1_t = load_bias("bo1", C)
        bo2_t = load_bias("bo2", C)
        bff1_t = load_bias("bff1", 2 * FFI)
        bff2_t = load_bias("bff2", C)
        wq1 = load_w(main, "wq1t", CT, INNER)
        wk1 = load_w(main, "wk1t", CT, INNER)
        wv1 = load_w(main, "wv1t", CT, INNER)
        wo1 = load_w(main, "wo1t", IT, C)
        wq2 = load_w(main, "wq2t", CT, INNER)
        wk2 = load_w(main, "wk2t", XT, INNER)
        wv2 = load_w(main, "wv2t", XT, INNER)
        wo2 = load_w(main, "wo2t", IT, C)

        attnO = [main.tile([P, NL], BF16, tag=f"attnO{t}", name=f"attnO{t}")
                 for t in range(IT)]
        # bf16 shadow of xres, refreshed during Wo phases so LN2/LN3 stats
        # read it without serial casts at the head of their chains
        xresb = [main.tile([P, NL], BF16, tag=f"xresb{t}", name=f"xresb{t}")
                 for t in range(CT)]

        # ---------- LayerNorm ----------
        def layernorm(hpool, src_tiles, ncols, lnid, xb_src=None):
            h_out = []
            for kt in range(CT):
                h_out.append(hpool.tile([P, ncols], BF16, tag=f"h{kt}",
                                        name=f"h{lnid}_{kt}"))
            with tc.tile_pool(name=f"psLN{lnid}", bufs=2, space="PSUM") as psLN, \
                 tc.tile_pool(name=f"psB{lnid}", bufs=2, space="PSUM") as psB, \
                 tc.tile_pool(name=f"st{lnid}", bufs=1) as st:
                rows = make_ln_rows(st, ncols)
                for cc in range(ncols // NCH):
                    ln_chunk(src_tiles, rows, cc * NCH, lnid, psLN, psB, h_out,
                             cc * NCH, xb_src=xb_src)
            return h_out

        def make_ln_rows(st, ncols):
            mean_row = st.tile([1, ncols], BF16, tag="mrow", name="mrow")
            msq_row = st.tile([1, ncols], F32, tag="qrow", name="qrow")
            var_row = st.tile([1, ncols], F32, tag="vrow", name="vrow")
            a_row = st.tile([1, ncols], BF16, tag="arow", name="arow")
            return (mean_row, msq_row, var_row, a_row)

        def ln_chunk(src_tiles, rows, col0, lnid, psLN, psB, h_out, hcol0,
                     xb_src=None):
            """LN stats+normalize for one 512-column chunk.

            src cols [col0, col0+NCH) -> h_out cols [hcol0.., ..+NCH)."""
            mean_row, msq_row, var_row, a_row = rows
            src_f32 = src_tiles[0].dtype == F32
            cs = slice(col0, col0 + NCH)
            rs = slice(hcol0, hcol0 + NCH)
            with tc.tile_pool(name=f"x2{lnid}c{col0}", bufs=3) as x2p:
                if xb_src is not None:
                    xb = [s[:, cs] for s in xb_src]
                elif src_f32:
                    xb = []
                    for kt in range(CT):
                        xbt = x2p.tile([P, NCH], BF16, tag="xb", name="xb")
                        nc.vector.tensor_copy(out=xbt, in_=src_tiles[kt][:, cs])
                        xb.append(xbt)
                else:
                    xb = [s[:, cs] for s in src_tiles]
                m_ps = psLN.tile([1, NCH], F32, tag="pp", name="m_ps")
                q_ps = psLN.tile([1, NCH], F32, tag="pp", name="q_ps")
                for kt in range(CT):
                    nc.tensor.matmul(m_ps, lhsT=ones_col_bf, rhs=xb[kt],
                                     start=(kt == 0), stop=(kt == CT - 1))
                for kt in range(CT):
                    x2 = x2p.tile([P, NCH], BF16, tag="x2", name="x2")
                    # gpsimd: both operands SBUF bf16; frees DVE for the
                    # normalize chain (gpsimd is otherwise idle)
                    nc.gpsimd.tensor_mul(out=x2, in0=xb[kt], in1=xb[kt])
                    nc.tensor.matmul(q_ps, lhsT=ones_col_bf, rhs=x2,
                                     start=(kt == 0), stop=(kt == CT - 1))
                nc.vector.tensor_scalar_mul(out=mean_row[0:1, rs], in0=m_ps,
                                            scalar1=1.0 / C)
                nc.vector.tensor_scalar_mul(out=msq_row[0:1, rs], in0=q_ps,
                                            scalar1=1.0 / C)
                nc.vector.tensor_mul(out=var_row[0:1, rs], in0=mean_row[0:1, rs],
                                     in1=mean_row[0:1, rs])
                nc.vector.tensor_sub(out=var_row[0:1, rs], in0=msq_row[0:1, rs],
                                     in1=var_row[0:1, rs])
                nc.scalar.activation(out=var_row[0:1, rs], in_=var_row[0:1, rs],
                                     func=AFT.Sqrt, bias=eps_t[0:1, 0:1])
                nc.vector.reciprocal(out=a_row[0:1, rs], in_=var_row[0:1, rs])
                mb = psB.tile([P, NCH], F32, tag="pp", name="mb")
                ab = psB.tile([P, NCH], F32, tag="pp", name="ab")
                nc.tensor.matmul(mb, lhsT=ones_row, rhs=mean_row[0:1, rs],
                                 start=True, stop=True)
                nc.tensor.matmul(ab, lhsT=ones_row, rhs=a_row[0:1, rs],
                                 start=True, stop=True)
                for kt in range(CT):
                    t1 = tp.tile([P, NCH], F32, tag="t1", name="t1")
                    nc.vector.tensor_sub(out=t1, in0=src_tiles[kt][:, cs], in1=mb)
                    nc.vector.tensor_mul(out=h_out[kt][:, rs], in0=t1, in1=ab)

        # ---------- projection helper ----------
        def proj(psP, w_tiles, rhs_tiles, nkt, out_mt, ncols, cb):
            cw = min(NCH, ncols)
            for mt in range(out_mt):
                for cc in range(ncols // cw):
                    ps = psP.tile([P, cw], F32, tag="pp", name="pp")
                    for kt in range(nkt):
                        nc.tensor.matmul(
                            ps,
                            lhsT=w_tiles[kt][:, mt * P:(mt + 1) * P],
                            rhs=rhs_tiles[kt][:, cc * cw:(cc + 1) * cw],
                            start=(kt == 0), stop=(kt == nkt - 1))
                    cb(mt, cc, cw, ps)

        def make_vt(psP, pool, lhs_tiles, nkt, w_tiles, jt, name):
            ps = psP.tile([P, INNER], F32, tag="pp", name="pp")
            for kt in range(nkt):
                nc.tensor.matmul(
                    ps,
                    lhsT=lhs_tiles[kt][:, jt * P:(jt + 1) * P],
                    rhs=w_tiles[kt],
                    start=(kt == 0), stop=(kt == nkt - 1))
            vt = pool.tile([P, H, DH + 1], BF16, tag=f"vt{jt}", name=name)
            nc.vector.tensor_copy(
                out=vt[:, :, 0:DH],
                in_=ps.rearrange("p (h d) -> p h d", h=H))
            nc.vector.memset(vt[:, :, DH:DH + 1], 1.0)
            return vt

        # ---------- attention ----------
        def attn_ic(k_sb, vt_sb, q_sb, njt, dst, ic, psS, psO, ep, rp):
            for hp in range(H // 2):
                t = hp
                po = [psO.tile([P, NCH], F32, tag=f"po{i}", name=f"po{i}")
                      for i in range(2)]
                for jt in range(njt):
                    ps = psS.tile([P, 2 * NCH], F32, tag="ps", name="ps")
                    for hh in range(2):
                        nc.tensor.matmul(
                            ps[:, hh * NCH:(hh + 1) * NCH],
                            lhsT=k_sb[t][hh * DH:(hh + 1) * DH, jt * P:(jt + 1) * P],
                            rhs=q_sb[t][hh * DH:(hh + 1) * DH, ic * NCH:(ic + 1) * NCH],
                            start=True, stop=True)
                    e = ep.tile([P, 2 * NCH], BF16, tag="e", name="e")
                    nc.scalar.activation(out=e, in_=ps, func=AFT.Exp)
                    for hh in range(2):
                        h = 2 * hp + hh
                        nc.tensor.matmul(
                            po[hh][0:DH + 1, :],
                            lhsT=vt_sb[jt][:, h, :],
                            rhs=e[:, hh * NCH:(hh + 1) * NCH],
                            start=(jt == 0), stop=(jt == njt - 1))
                for hh in range(2):
                    rrow = rp.tile([1, NCH], BF16, tag="rrow", name="rrow")
                    nc.vector.reciprocal(out=rrow, in_=po[hh][DH:DH + 1, :])
                    # broadcast 1/denom into po's unused partitions 64..127
                    nc.tensor.matmul(po[hh][DH:2 * DH, :],
                                     lhsT=ones_row[0:1, 0:DH], rhs=rrow,
                                     start=True, stop=True)
                    un = rp.tile([DH, NCH], BF16, tag="un", name="un")
                    nc.vector.tensor_copy(out=un, in_=po[hh][0:DH, :])
                    nc.vector.tensor_mul(
                        out=dst[t][hh * DH:(hh + 1) * DH, ic * NCH:(ic + 1) * NCH],
                        in0=un, in1=po[hh][DH:2 * DH, :])

        # ---------- output-proj + residual (one ic chunk) ----------
        def wo_resid_ic(psP, wo_tiles, src, bias_t, nkt, ic):
            cs = slice(ic * NCH, (ic + 1) * NCH)
            for mt in range(CT):
                ps = psP.tile([P, NCH], F32, tag="pp", name="pp")
                for kt in range(nkt):
                    nc.tensor.matmul(ps, lhsT=wo_tiles[kt][:, mt * P:(mt + 1) * P],
                                     rhs=src[kt][:, cs],
                                     start=(kt == 0), stop=(kt == nkt - 1))
                t1 = tp.tile([P, NCH], F32, tag="t1", name="t1")
                nc.scalar.activation(out=t1, in_=ps, func=AFT.Identity,
                                     bias=bias_t[:, mt:mt + 1])
                nc.vector.tensor_add(out=xres[mt][:, cs], in0=t1,
                                     in1=xres[mt][:, cs])
                nc.vector.tensor_copy(out=xresb[mt][:, cs], in_=xres[mt][:, cs])

        # ================= phase 1: LN1 over the full sequence =================
        h1p_cm = tc.tile_pool(name="h1p", bufs=1)
        h1p = h1p_cm.__enter__()
        h1 = layernorm(h1p, xfull, N, "1")

        # ============= phase 2: Q/K/V projections (self) =============
        q1_sb = [sa.tile([P, NL], BF16, tag=f"q{t}", name=f"q1_{t}") for t in range(IT)]
        k1_sb = [sa.tile([P, N], BF16, tag=f"k{t}", name=f"k1_{t}") for t in range(IT)]
        with tc.tile_pool(name="psP1", bufs=4, space="PSUM") as psP:
            proj(psP, wq1, [ht[:, 0:NL] for ht in h1], CT, IT, NL,
                 lambda mt, cc, cw, ps: nc.vector.tensor_copy(
                     out=q1_sb[mt][:, cc * cw:(cc + 1) * cw], in_=ps))
            proj(psP, wk1, h1, CT, IT, N,
                 lambda mt, cc, cw, ps: nc.vector.tensor_copy(
                     out=k1_sb[mt][:, cc * cw:(cc + 1) * cw], in_=ps))
            vt1_sb = [make_vt(psP, sa, h1, CT, wv1, jt, f"vt1_{jt}")
                      for jt in range(JT1)]
            # cross-attn K2/V2T depend only on ctx: emit early so the PE work
            # fills self-attention's ACT-bound phase
            k2_sb = [ca.tile([P, MCTX], BF16, tag=f"k{t}", name=f"k2_{t}")
                     for t in range(IT)]
            proj(psP, wk2, ctx_sb, XT, IT, MCTX,
                 lambda mt, cc, cw, ps: nc.vector.tensor_copy(
                     out=k2_sb[mt][:, cc * cw:(cc + 1) * cw], in_=ps))
            vt2_sb = [make_vt(psP, ca, ctx_sb, XT, wv2, jt, f"vt2_{jt}")
                      for jt in range(JT2)]
        if DEBUG:
            for kt in range(CT):
                nc.sync.dma_start(out=nc.dbg["d_h1"][kt * P:(kt + 1) * P, :], in_=h1[kt])
                nc.sync.dma_start(out=nc.dbg["d_q1"][kt * P:(kt + 1) * P, :], in_=q1_sb[kt])
                nc.sync.dma_start(out=nc.dbg["d_k1"][kt * P:(kt + 1) * P, :], in_=k1_sb[kt])
        h1p_cm.__exit__(None, None, None)
        xfp_cm.__exit__(None, None, None)

        # ===== phase 3: self-attention =====
        with tc.tile_pool(name="psS", bufs=2, space="PSUM") as psS, \
             tc.tile_pool(name="psO", bufs=2, space="PSUM") as psO, \
             tc.tile_pool(name="ep", bufs=6) as ep, \
             tc.tile_pool(name="rp", bufs=4) as rp:
            for ic in range(ICN):
                attn_ic(k1_sb, vt1_sb, q1_sb, JT1, attnO, ic, psS, psO,
                        ep, rp)
        sa_cm.__exit__(None, None, None)
        wffp_cm = tc.tile_pool(name="wffp", bufs=1, side="right")
        wffp = wffp_cm.__enter__()
        wff1 = load_w(wffp, "wff1t", CT, 2 * FFI)
        wff2 = load_w(wffp, "wff2t", FT, C)

        # ===== phase 4: Wo1 + residual =====
        with tc.tile_pool(name="psP2", bufs=4, space="PSUM") as psP:
            for ic in range(ICN):
                wo_resid_ic(psP, wo1, attnO, bo1_t, IT, ic)

        # ===== phase 5: LN2 + Q2 =====
        h2 = layernorm(ca, xres, NL, "2", xb_src=xresb)
        q2_sb = [ca.tile([P, NL], BF16, tag=f"q{t}", name=f"q2_{t}")
                 for t in range(IT)]
        with tc.tile_pool(name="psP3", bufs=4, space="PSUM") as psP:
            proj(psP, wq2, h2, CT, IT, NL,
                 lambda mt, cc, cw, ps: nc.vector.tensor_copy(
                     out=q2_sb[mt][:, cc * cw:(cc + 1) * cw], in_=ps))

        # ===== phase 6: cross-attention =====
        with tc.tile_pool(name="psS2", bufs=2, space="PSUM") as psS, \
             tc.tile_pool(name="psO2", bufs=2, space="PSUM") as psO, \
             tc.tile_pool(name="ep2", bufs=6) as ep, \
             tc.tile_pool(name="rp2", bufs=4) as rp:
            for ic in range(ICN):
                attn_ic(k2_sb, vt2_sb, q2_sb, JT2, attnO, ic, psS, psO,
                        ep, rp)

        # ===== phase 7: Wo2 + residual, then LN3 =====
        with tc.tile_pool(name="psP4", bufs=4, space="PSUM") as psP:
            for ic in range(ICN):
                wo_resid_ic(psP, wo2, attnO, bo2_t, IT, ic)
        h3 = layernorm(ca, xres, NL, "3", xb_src=xresb)

        # ============= phase 8: GEGLU FF =============
        if DEBUG:
            for kt in range(CT):
                nc.sync.dma_start(out=nc.dbg["d_h3"][kt * P:(kt + 1) * P, :], in_=h3[kt])
        with tc.tile_pool(name="psY", bufs=1, space="PSUM") as psY, \
             tc.tile_pool(name="psF", bufs=2, space="PSUM") as psF, \
             tc.tile_pool(name="gp", bufs=3) as gp, \
             tc.tile_pool(name="op", bufs=3) as op:
            for ic in range(ICN):
                pys = [psY.tile([P, NCH], F32, tag=f"y{m}", name=f"y{m}")
                       for m in range(CT)]
                for pi in range(FT):
                    ph = psF.tile([P, NCH], F32, tag="ph", name="ph")
                    pg = psF.tile([P, NCH], F32, tag="pg", name="pg")
                    for kt in range(CT):
                        nc.tensor.matmul(
                            ph,
                            lhsT=wff1[kt][:, pi * P:(pi + 1) * P],
                            rhs=h3[kt][:, ic * NCH:(ic + 1) * NCH],
                            start=(kt == 0), stop=(kt == CT - 1))
                    for kt in range(CT):
                        nc.tensor.matmul(
                            pg,
                            lhsT=wff1[kt][:, FFI + pi * P:FFI + (pi + 1) * P],
                            rhs=h3[kt][:, ic * NCH:(ic + 1) * NCH],
                            start=(kt == 0), stop=(kt == CT - 1))
                    gel = gp.tile([P, NCH], BF16, tag="gel", name="gel")
                    nc.scalar.activation(out=gel, in_=pg, func=AFT.Gelu,
                                         bias=bff1_t[:, FT + pi:FT + pi + 1])
                    hb = tp.tile([P, NCH], F32, tag="hb", name="hb")
                    nc.scalar.activation(out=hb, in_=ph, func=AFT.Identity,
                                         bias=bff1_t[:, pi:pi + 1])
                    ffh = gp.tile([P, NCH], BF16, tag="ffh", name="ffh")
                    nc.vector.tensor_mul(out=ffh, in0=hb, in1=gel)
                    for mt in range(CT):
                        nc.tensor.matmul(
                            pys[mt],
                            lhsT=wff2[pi][:, mt * P:(mt + 1) * P],
                            rhs=ffh,
                            start=(pi == 0), stop=(pi == FT - 1))
                for mt in range(CT):
                    t1 = tp.tile([P, NCH], F32, tag="t1", name="t1")
                    nc.scalar.activation(out=t1, in_=pys[mt], func=AFT.Identity,
                                         bias=bff2_t[:, mt:mt + 1])
                    ot = op.tile([P, NCH], F32, tag="ot", name="ot")
                    nc.vector.tensor_add(out=ot, in0=t1,
                                         in1=xres[mt][:, ic * NCH:(ic + 1) * NCH])
                    nc.sync.dma_start(
                        out=out_d[mt * P:(mt + 1) * P, ic * NCH:(ic + 1) * NCH],
                        in_=ot)
        ca_cm.__exit__(None, None, None)
        wffp_cm.__exit__(None, None, None)


def _split_multi_waits(nc):
    """This walrus build accepts at most one sem-wait per instruction; Tile
    emits several. Split extras into standalone InstEventSemaphore pre-waits
    on the same engine (engines execute their stream in order, so semantics
    are preserved)."""
    n = 0
    for fn in nc.m.functions:
        for blk in fn.blocks:
            out = []
            for inst in blk.instructions:
                si = inst.sync_info
                if si is not None and si.on_wait and len(si.on_wait) > 1:
                    waits = list(si.on_wait)
                    for i, w in enumerate(waits[:-1]):
                        out.append(mybir.InstEventSemaphore(
                            name=f"{inst.name}-w{i}",
                            engine=inst.engine,
                            sync_info=mybir.SyncInfo(on_wait=[w], on_update=[]),
                        ))
                        n += 1
                    inst.sync_info = mybir.SyncInfo(
                        on_wait=[waits[-1]], on_update=list(si.on_update))
                out.append(inst)
            blk.instructions = out
    return n


def _build():
    nc = bass.Bass()
    nc.x_d = nc.dram_tensor("x", [C, NL], F32, kind="ExternalInput")
    nc.xb_d = nc.dram_tensor("xb", [C, N], BF16, kind="ExternalInput")
    nc.ctx_d = nc.dram_tensor("ctx", [CTXC, MCTX], BF16, kind="ExternalInput")
    nc.w_d = {}
    for name, shape in [
        ("wq1t", [C, INNER]), ("wk1t", [C, INNER]), ("wv1t", [C, INNER]),
        ("wo1t", [INNER, C]),
        ("wq2t", [C, INNER]), ("wk2t", [CTXC, INNER]), ("wv2t", [CTXC, INNER]),
        ("wo2t", [INNER, C]),
        ("wff1t", [C, 2 * FFI]), ("wff2t", [FFI, C]),
    ]:
        nc.w_d[name] = nc.dram_tensor(name, shape, BF16, kind="ExternalInput")
    nc.b_d = {}
    for name, n in [("bo1", C), ("bo2", C), ("bff1", 2 * FFI), ("bff2", C)]:
        nc.b_d[name] = nc.dram_tensor(name, [n], F32, kind="ExternalInput")
    nc.out_d = nc.dram_tensor("out", [C, NL], F32, kind="ExternalOutput")
    nc.dbg = {}
    if DEBUG:
        for name, shape, dt in [
            ("d_h1", [C, N], BF16), ("d_q1", [C, NL], BF16),
            ("d_k1", [C, N], BF16), ("d_attnO1", [C, NL], BF16),
            ("d_x1", [C, NL], F32), ("d_x2", [C, NL], F32),
            ("d_h3", [C, NL], BF16),
        ]:
            nc.dbg[name] = nc.dram_tensor(name, shape, dt, kind="ExternalOutput")
    with tile.TileContext(nc) as tc:
        _emit(tc)
    _split_multi_waits(nc)
    return nc


_CACHE = {}


def _get_program():
    if "nc" not in _CACHE:
        _CACHE["nc"] = _build()
    return _CACHE["nc"]


def _prep_shared(inputs):
    f32 = np.float32
    g1 = np.asarray(inputs["g1"], f32)
    g2 = np.asarray(inputs["g2"], f32)
    g3 = np.asarray(inputs["g3"], f32)
    scale = DH ** -0.5
    d = {
        "wq1t": np.ascontiguousarray(
            (np.asarray(inputs["Wq1"], f32) * scale * g1[None, :]).T).astype(BF16NP),
        "wk1t": np.ascontiguousarray(
            (np.asarray(inputs["Wk1"], f32) * g1[None, :]).T).astype(BF16NP),
        "wv1t": np.ascontiguousarray(
            (np.asarray(inputs["Wv1"], f32) * g1[None, :]).T).astype(BF16NP),
        "wo1t": np.ascontiguousarray(np.asarray(inputs["Wo1"], f32).T).astype(BF16NP),
        "wq2t": np.ascontiguousarray(
            (np.asarray(inputs["Wq2"], f32) * scale * g2[None, :]).T).astype(BF16NP),
        "wk2t": np.ascontiguousarray(np.asarray(inputs["Wk2"], f32).T).astype(BF16NP),
        "wv2t": np.ascontiguousarray(np.asarray(inputs["Wv2"], f32).T).astype(BF16NP),
        "wo2t": np.ascontiguousarray(np.asarray(inputs["Wo2"], f32).T).astype(BF16NP),
        "wff1t": np.ascontiguousarray(
            (np.asarray(inputs["Wff1"], f32) * g3[None, :]).T).astype(BF16NP),
        "wff2t": np.ascontiguousarray(np.asarray(inputs["Wff2"], f32).T).astype(BF16NP),
        "bo1": np.ascontiguousarray(np.asarray(inputs["bo1"], f32)),
        "bo2": np.ascontiguousarray(np.asarray(inputs["bo2"], f32)),
        "bff1": np.ascontiguousarray(np.asarray(inputs["bff1"], f32)),
        "bff2": np.ascontiguousarray(np.asarray(inputs["bff2"], f32)),
    }
    return d


def make_in_maps(inputs):
    x = np.asarray(inputs["x"], np.float32)
    ctxf = np.asarray(inputs["context"], np.float32)
    shared = _prep_shared(inputs)
    in_maps = []
    for core in range(8):
        b, s = core // 2, core % 2
        xb = x[b]
        if s:
            xc = np.ascontiguousarray(
                np.concatenate([xb[:, NL:], xb[:, :NL]], axis=1))
        else:
            xc = np.ascontiguousarray(xb)
        m = dict(shared)
        m["x"] = np.ascontiguousarray(xc[:, :NL])
        m["xb"] = xc.astype(BF16NP)
        m["ctx"] = np.ascontiguousarray(ctxf[b]).astype(BF16NP)
        in_maps.append(m)
    return in_maps


def kernel(**inputs):
    nc = _get_program()
    in_maps = make_in_maps(inputs)
    res = run_bass_kernel_spmd(nc, in_maps, core_ids=list(range(8)))
    out = np.empty((B, C, N), np.float32)
    for core in range(8):
        b, s = core // 2, core % 2
        out[b][:, s * NL:(s + 1) * NL] = res.results[core]["out"]
    return out



# revision 34
# speedup vs baseline: 1.1079x; 1.1079x over previous
"""Trainium2 Bass kernel for a BasicTransformerBlock (self-attn + cross-attn + GEGLU FF).

Sharding: 8 cores = (batch b in 0..3) x (sequence half s in 0..1). No collectives.
Each core receives the full x[b] [512, 2048] (rotated so its local half is always
columns 0..1023), builds self-attention K/V over all 2048 positions, and computes
LN/Q/attention/FF only for its local 1024 positions. Output [512, 1024] per core.

Numerics: bf16 matmuls with fp32 PSUM accumulation; LayerNorm gains folded into the
following weight matrices on the host; attention softmax computed without
max-subtraction (scores are bounded ~+-1.5 here); softmax denominator obtained by
augmenting V^T with a ones-column (row 64 of the AV output = sum_j exp).
"""

import os
import sys

import numpy as np

for _p in ("/opt/trn_rl_repo", "/root/.axon_site/_ro/trn_rl_repo"):
    if os.path.isdir(_p) and _p not in sys.path:
        sys.path.insert(0, _p)

import ml_dtypes

import concourse.bass as bass
import concourse.tile as tile
from concourse import mybir
from concourse.bass_utils import run_bass_kernel_spmd

BF16NP = ml_dtypes.bfloat16
E4NP = ml_dtypes.float8_e4m3
AFT = mybir.ActivationFunctionType
DR = mybir.MatmulPerfMode.DoubleRow
F32 = mybir.dt.float32
BF16 = mybir.dt.bfloat16
FP8 = mybir.dt.float8e4

# Problem dims (hardcoded per spec)
P = 128
B = 4
C = 512      # model dim
N = 2048     # full seq len
NL = 1024    # local seq len per core
CTXC = 768   # context channels
MCTX = 256   # context seq len
H = 8
DH = 64
INNER = 512
FFI = 2048
EPS = 1e-5

CT = C // P        # 4 channel tiles
IT = INNER // P    # 4 inner tiles
XT = CTXC // P     # 6 ctx channel tiles
FT = FFI // P      # 16 ff tiles
CPAIR = CT // 2    # 2 channel-tile pairs
FPAIR = FT // 2    # 8 ff-tile pairs
SW = 64.0          # fp8 weight pre-scale for the FF block
NCH = 512          # free-dim chunk size
ICN = NL // NCH    # 2 local i-chunks
JT1 = N // P       # 16 self-attn j tiles
JT2 = MCTX // P    # 2 cross-attn j tiles
DEBUG = False


def _emit(tc):
    nc = tc.nc
    from contextlib import ExitStack

    with ExitStack() as ctx:
        ctx.enter_context(nc.allow_low_precision(
            reason="bf16 rows/broadcasts validated end-to-end vs fp32 reference"))
        main = ctx.enter_context(tc.tile_pool(name="main", bufs=1))
        tp = ctx.enter_context(tc.tile_pool(name="tp", bufs=4))

        x_d = nc.x_d
        ctx_d = nc.ctx_d
        w_d = nc.w_d
        b_d = nc.b_d
        out_d = nc.out_d

        # ---- constants ----
        ones_col = main.tile([P, 1], F32, tag="ones_col", name="ones_col")
        nc.vector.memset(ones_col, 1.0)
        ones_col_bf = main.tile([P, 1], BF16, tag="ones_col_bf", name="ones_col_bf")
        nc.vector.memset(ones_col_bf, 1.0)
        ones_row = main.tile([1, P], BF16, tag="ones_row", name="ones_row")
        nc.vector.memset(ones_row, 1.0)
        eps_t = main.tile([P, 1], F32, tag="eps", name="eps")
        nc.vector.memset(eps_t, EPS)

        # ---- load weights (attention ones up-front; FF weights later) ----
        def load_split(pool, tag, dram, nkt, cols, dtype):
            """One wide DMA for a [nkt*128, cols] DRAM tensor into a single
            [128, nkt*cols] SBUF tile; returns per-kt [128, cols] views."""
            t = pool.tile([P, nkt * cols], dtype, tag=tag, name=tag)
            nc.sync.dma_start(
                out=t.rearrange("p (kt c) -> p kt c", kt=nkt),
                in_=dram.rearrange("(kt p) c -> p kt c", p=P))
            return [t[:, kt * cols:(kt + 1) * cols] for kt in range(nkt)]

        def load_w(pool, name, nkt, cols):
            return load_split(pool, name, w_d[name], nkt, cols, BF16)


        def load_bias(name, n):
            f = n // P
            t = main.tile([P, f], F32, tag=f"b_{name}", name=f"b_{name}")
            nc.sync.dma_start(out=t, in_=b_d[name].rearrange("(f p) -> p f", p=P))
            return t

        ca_cm = tc.tile_pool(name="ca", bufs=1)
        ca = ca_cm.__enter__()
        sa_cm = tc.tile_pool(name="sa", bufs=1)
        sa = sa_cm.__enter__()
        # ---- load activations (before weights: LN1 needs x first) ----
        xfp_cm = tc.tile_pool(name="xfull", bufs=1)
        xfp = xfp_cm.__enter__()
        # xfull: one [128, CT*N] tile, DMA'd in 4 column-chunks so LN1's
        # first chunk starts as soon as its slice lands
        xft = xfp.tile([P, CT * N], BF16, tag="xf", name="xf")
        _xf_nc = N // NCH
        for cc in range(_xf_nc):
            nc.sync.dma_start(
                out=xft.rearrange("p (kt nc c) -> p nc kt c", kt=CT,
                                  nc=_xf_nc)[:, cc],
                in_=nc.xb_d.rearrange("(kt p) (nc c) -> p nc kt c", p=P,
                                      nc=_xf_nc)[:, cc])
        xfull = [xft[:, kt * N:(kt + 1) * N] for kt in range(CT)]
        xres = load_split(main, "xres", x_d, CT, NL, F32)
        ctx_sb = load_split(main, "ctx", ctx_d, XT, MCTX, BF16)

        # biases + weights after activations so LN1's x tiles arrive first
        bo1_t = load_bias("bo1", C)
        bo2_t = load_bias("bo2", C)
        bff1_t = load_bias("bff1", 2 * FFI)
        bff2_t = load_bias("bff2", C)
        wq1 = load_w(main, "wq1t", CT, INNER)
        wk1 = load_w(main, "wk1t", CT, INNER)
        wv1 = load_w(main, "wv1t", CT, INNER)
        wo1 = load_w(main, "wo1t", IT, C)
        wq2 = load_w(main, "wq2t", CT, INNER)
        wk2 = load_w(main, "wk2t", XT, INNER)
        wv2 = load_w(main, "wv2t", XT, INNER)
        wo2 = load_w(main, "wo2t", IT, C)

        attnO = [main.tile([P, NL], BF16, tag=f"attnO{t}", name=f"attnO{t}")
                 for t in range(IT)]
        # bf16 shadow of xres, refreshed during Wo phases so LN2/LN3 stats
        # read it without serial casts at the head of their chains
        xresb = [main.tile([P, NL], BF16, tag=f"xresb{t}", name=f"xresb{t}")
                 for t in range(CT)]

        # ---------- LayerNorm ----------
        def layernorm(hpool, src_tiles, ncols, lnid, xb_src=None, dst=None):
            if dst is not None:
                h_out = [dst[:, kt * ncols:(kt + 1) * ncols]
                         for kt in range(CT)]
            else:
                h_out = []
                for kt in range(CT):
                    h_out.append(hpool.tile([P, ncols], BF16, tag=f"h{kt}",
                                            name=f"h{lnid}_{kt}"))
            with tc.tile_pool(name=f"psLN{lnid}", bufs=2, space="PSUM") as psLN, \
                 tc.tile_pool(name=f"psB{lnid}", bufs=2, space="PSUM") as psB, \
                 tc.tile_pool(name=f"st{lnid}", bufs=1) as st:
                rows = make_ln_rows(st, ncols)
                for cc in range(ncols // NCH):
                    ln_chunk(src_tiles, rows, cc * NCH, lnid, psLN, psB, h_out,
                             cc * NCH, xb_src=xb_src)
            return h_out

        def make_ln_rows(st, ncols):
            mean_row = st.tile([1, ncols], BF16, tag="mrow", name="mrow")
            msq_row = st.tile([1, ncols], F32, tag="qrow", name="qrow")
            var_row = st.tile([1, ncols], BF16, tag="vrow", name="vrow")
            a_row = st.tile([1, ncols], BF16, tag="arow", name="arow")
            return (mean_row, msq_row, var_row, a_row)

        def ln_chunk(src_tiles, rows, col0, lnid, psLN, psB, h_out, hcol0,
                     xb_src=None):
            """LN stats+normalize for one 512-column chunk.

            src cols [col0, col0+NCH) -> h_out cols [hcol0.., ..+NCH)."""
            mean_row, msq_row, var_row, a_row = rows
            src_f32 = src_tiles[0].dtype == F32
            cs = slice(col0, col0 + NCH)
            rs = slice(hcol0, hcol0 + NCH)
            with tc.tile_pool(name=f"x2{lnid}c{col0}", bufs=3) as x2p:
                if xb_src is not None:
                    xb = [s[:, cs] for s in xb_src]
                elif src_f32:
                    xb = []
                    for kt in range(CT):
                        xbt = x2p.tile([P, NCH], BF16, tag="xb", name="xb")
                        nc.vector.tensor_copy(out=xbt, in_=src_tiles[kt][:, cs])
                        xb.append(xbt)
                else:
                    xb = [s[:, cs] for s in src_tiles]
                m_ps = psLN.tile([1, NCH], F32, tag="pp", name="m_ps")
                q_ps = psLN.tile([1, NCH], F32, tag="pp", name="q_ps")
                for kt in range(CT):
                    nc.tensor.matmul(m_ps, lhsT=ones_col_bf, rhs=xb[kt],
                                     start=(kt == 0), stop=(kt == CT - 1))
                for kt in range(CT):
                    x2 = x2p.tile([P, NCH], BF16, tag="x2", name="x2")
                    # gpsimd: both operands SBUF bf16; frees DVE for the
                    # normalize chain (gpsimd is otherwise idle)
                    nc.gpsimd.tensor_mul(out=x2, in0=xb[kt], in1=xb[kt])
                    nc.tensor.matmul(q_ps, lhsT=ones_col_bf, rhs=x2,
                                     start=(kt == 0), stop=(kt == CT - 1))
                nc.vector.tensor_scalar_mul(out=mean_row[0:1, rs], in0=m_ps,
                                            scalar1=1.0 / C)
                nc.vector.tensor_scalar_mul(out=msq_row[0:1, rs], in0=q_ps,
                                            scalar1=1.0 / C)
                nc.vector.tensor_mul(out=var_row[0:1, rs], in0=mean_row[0:1, rs],
                                     in1=mean_row[0:1, rs])
                nc.vector.tensor_sub(out=var_row[0:1, rs], in0=msq_row[0:1, rs],
                                     in1=var_row[0:1, rs])
                nc.scalar.activation(out=var_row[0:1, rs], in_=var_row[0:1, rs],
                                     func=AFT.Sqrt, bias=eps_t[0:1, 0:1])
                nc.vector.reciprocal(out=a_row[0:1, rs], in_=var_row[0:1, rs])
                mb = psB.tile([P, NCH], F32, tag="pp", name="mb")
                ab = psB.tile([P, NCH], F32, tag="pp", name="ab")
                nc.tensor.matmul(mb, lhsT=ones_row, rhs=mean_row[0:1, rs],
                                 start=True, stop=True)
                nc.tensor.matmul(ab, lhsT=ones_row, rhs=a_row[0:1, rs],
                                 start=True, stop=True)
                for kt in range(CT):
                    t1 = tp.tile([P, NCH], F32, tag="t1", name="t1")
                    nc.vector.tensor_sub(out=t1, in0=src_tiles[kt][:, cs], in1=mb)
                    nc.vector.tensor_mul(out=h_out[kt][:, rs], in0=t1, in1=ab)

        # ---------- projection helper ----------
        def proj(psP, w_tiles, rhs_tiles, nkt, out_mt, ncols, cb):
            cw = min(NCH, ncols)
            for mt in range(out_mt):
                for cc in range(ncols // cw):
                    ps = psP.tile([P, cw], F32, tag="pp", name="pp")
                    for kt in range(nkt):
                        nc.tensor.matmul(
                            ps,
                            lhsT=w_tiles[kt][:, mt * P:(mt + 1) * P],
                            rhs=rhs_tiles[kt][:, cc * cw:(cc + 1) * cw],
                            start=(kt == 0), stop=(kt == nkt - 1))
                    cb(mt, cc, cw, ps)

        def make_vt(psP, pool, lhs_tiles, nkt, w_tiles, jt, name):
            ps = psP.tile([P, INNER], F32, tag="pp", name="pp")
            for kt in range(nkt):
                nc.tensor.matmul(
                    ps,
                    lhsT=lhs_tiles[kt][:, jt * P:(jt + 1) * P],
                    rhs=w_tiles[kt],
                    start=(kt == 0), stop=(kt == nkt - 1))
            vt = pool.tile([P, H, DH + 1], BF16, tag=f"vt{jt}", name=name)
            nc.vector.tensor_copy(
                out=vt[:, :, 0:DH],
                in_=ps.rearrange("p (h d) -> p h d", h=H))
            nc.vector.memset(vt[:, :, DH:DH + 1], 1.0)
            return vt

        # ---------- attention ----------
        def attn_ic(k_sb, vt_sb, q_sb, njt, dst, ic, psS, psO, ep, rp):
            for hp in range(H // 2):
                t = hp
                po = [psO.tile([P, NCH], F32, tag=f"po{i}", name=f"po{i}")
                      for i in range(2)]
                for jt in range(njt):
                    ps = psS.tile([P, 2 * NCH], F32, tag="ps", name="ps")
                    for hh in range(2):
                        nc.tensor.matmul(
                            ps[:, hh * NCH:(hh + 1) * NCH],
                            lhsT=k_sb[t][hh * DH:(hh + 1) * DH, jt * P:(jt + 1) * P],
                            rhs=q_sb[t][hh * DH:(hh + 1) * DH, ic * NCH:(ic + 1) * NCH],
                            start=True, stop=True)
                    e = ep.tile([P, 2 * NCH], BF16, tag="e", name="e")
                    nc.scalar.activation(out=e, in_=ps, func=AFT.Exp)
                    for hh in range(2):
                        h = 2 * hp + hh
                        nc.tensor.matmul(
                            po[hh][0:DH + 1, :],
                            lhsT=vt_sb[jt][:, h, :],
                            rhs=e[:, hh * NCH:(hh + 1) * NCH],
                            start=(jt == 0), stop=(jt == njt - 1))
                for hh in range(2):
                    rrow = rp.tile([1, NCH], BF16, tag="rrow", name="rrow")
                    nc.vector.reciprocal(out=rrow, in_=po[hh][DH:DH + 1, :])
                    # broadcast 1/denom into po's unused partitions 64..127
                    nc.tensor.matmul(po[hh][DH:2 * DH, :],
                                     lhsT=ones_row[0:1, 0:DH], rhs=rrow,
                                     start=True, stop=True)
                    un = rp.tile([DH, NCH], BF16, tag="un", name="un")
                    nc.vector.tensor_copy(out=un, in_=po[hh][0:DH, :])
                    nc.vector.tensor_mul(
                        out=dst[t][hh * DH:(hh + 1) * DH, ic * NCH:(ic + 1) * NCH],
                        in0=un, in1=po[hh][DH:2 * DH, :])

        # ---------- output-proj + residual (one ic chunk) ----------
        def wo_resid_ic(psP, wo_tiles, src, bias_t, nkt, ic):
            cs = slice(ic * NCH, (ic + 1) * NCH)
            for mt in range(CT):
                ps = psP.tile([P, NCH], F32, tag="pp", name="pp")
                for kt in range(nkt):
                    nc.tensor.matmul(ps, lhsT=wo_tiles[kt][:, mt * P:(mt + 1) * P],
                                     rhs=src[kt][:, cs],
                                     start=(kt == 0), stop=(kt == nkt - 1))
                t1 = tp.tile([P, NCH], F32, tag="t1", name="t1")
                nc.scalar.activation(out=t1, in_=ps, func=AFT.Identity,
                                     bias=bias_t[:, mt:mt + 1])
                nc.vector.tensor_add(out=xres[mt][:, cs], in0=t1,
                                     in1=xres[mt][:, cs])
                nc.vector.tensor_copy(out=xresb[mt][:, cs], in_=xres[mt][:, cs])

        # ================= phase 1: LN1 over the full sequence =================
        h1p_cm = tc.tile_pool(name="h1p", bufs=1)
        h1p = h1p_cm.__enter__()
        h1 = layernorm(h1p, xfull, N, "1")

        # ============= phase 2: Q/K/V projections (self) =============
        q1_sb = [sa.tile([P, NL], BF16, tag=f"q{t}", name=f"q1_{t}") for t in range(IT)]
        k1_sb = [sa.tile([P, N], BF16, tag=f"k{t}", name=f"k1_{t}") for t in range(IT)]
        with tc.tile_pool(name="psP1", bufs=4, space="PSUM") as psP:
            proj(psP, wq1, [ht[:, 0:NL] for ht in h1], CT, IT, NL,
                 lambda mt, cc, cw, ps: nc.vector.tensor_copy(
                     out=q1_sb[mt][:, cc * cw:(cc + 1) * cw], in_=ps))
            proj(psP, wk1, h1, CT, IT, N,
                 lambda mt, cc, cw, ps: nc.vector.tensor_copy(
                     out=k1_sb[mt][:, cc * cw:(cc + 1) * cw], in_=ps))
            vt1_sb = [make_vt(psP, sa, h1, CT, wv1, jt, f"vt1_{jt}")
                      for jt in range(JT1)]
            # cross-attn K2/V2T depend only on ctx: emit early so the PE work
            # fills self-attention's ACT-bound phase
            k2_sb = [ca.tile([P, MCTX], BF16, tag=f"k{t}", name=f"k2_{t}")
                     for t in range(IT)]
            proj(psP, wk2, ctx_sb, XT, IT, MCTX,
                 lambda mt, cc, cw, ps: nc.vector.tensor_copy(
                     out=k2_sb[mt][:, cc * cw:(cc + 1) * cw], in_=ps))
            vt2_sb = [make_vt(psP, ca, ctx_sb, XT, wv2, jt, f"vt2_{jt}")
                      for jt in range(JT2)]
        if DEBUG:
            for kt in range(CT):
                nc.sync.dma_start(out=nc.dbg["d_h1"][kt * P:(kt + 1) * P, :], in_=h1[kt])
                nc.sync.dma_start(out=nc.dbg["d_q1"][kt * P:(kt + 1) * P, :], in_=q1_sb[kt])
                nc.sync.dma_start(out=nc.dbg["d_k1"][kt * P:(kt + 1) * P, :], in_=k1_sb[kt])
        h1p_cm.__exit__(None, None, None)
        xfp_cm.__exit__(None, None, None)

        # ===== phase 3: self-attention =====
        with tc.tile_pool(name="psS", bufs=2, space="PSUM") as psS, \
             tc.tile_pool(name="psO", bufs=2, space="PSUM") as psO, \
             tc.tile_pool(name="ep", bufs=6) as ep, \
             tc.tile_pool(name="rp", bufs=4) as rp:
            for ic in range(ICN):
                attn_ic(k1_sb, vt1_sb, q1_sb, JT1, attnO, ic, psS, psO,
                        ep, rp)
        sa_cm.__exit__(None, None, None)
        wffp_cm = tc.tile_pool(name="wffp", bufs=1, side="right")
        wffp = wffp_cm.__enter__()
        wff1d = wffp.tile([P, CPAIR, 2, 2 * FFI], FP8, tag="wff1t",
                          name="wff1t")
        nc.sync.dma_start(out=wff1d.rearrange("p a b c -> p (a b c)"),
                          in_=w_d["wff1t"][:])
        wff2d = wffp.tile([P, FPAIR, 2, C], FP8, tag="wff2t", name="wff2t")
        nc.sync.dma_start(out=wff2d.rearrange("p a b c -> p (a b c)"),
                          in_=w_d["wff2t"][:])
        ffh_t = wffp.tile([P, 2, NCH], FP8, tag="ffh_t", name="ffh_t")

        # ===== phase 4: Wo1 + residual =====
        with tc.tile_pool(name="psP2", bufs=4, space="PSUM") as psP:
            for ic in range(ICN):
                wo_resid_ic(psP, wo1, attnO, bo1_t, IT, ic)

        # ===== phase 5: LN2 + Q2 =====
        h2 = layernorm(ca, xres, NL, "2", xb_src=xresb)
        q2_sb = [ca.tile([P, NL], BF16, tag=f"q{t}", name=f"q2_{t}")
                 for t in range(IT)]
        with tc.tile_pool(name="psP3", bufs=4, space="PSUM") as psP:
            proj(psP, wq2, h2, CT, IT, NL,
                 lambda mt, cc, cw, ps: nc.vector.tensor_copy(
                     out=q2_sb[mt][:, cc * cw:(cc + 1) * cw], in_=ps))

        # ===== phase 6: cross-attention =====
        with tc.tile_pool(name="psS2", bufs=2, space="PSUM") as psS, \
             tc.tile_pool(name="psO2", bufs=2, space="PSUM") as psO, \
             tc.tile_pool(name="ep2", bufs=6) as ep, \
             tc.tile_pool(name="rp2", bufs=4) as rp:
            for ic in range(ICN):
                attn_ic(k2_sb, vt2_sb, q2_sb, JT2, attnO, ic, psS, psO,
                        ep, rp)

        # ===== phase 7: Wo2 + residual, then LN3 =====
        with tc.tile_pool(name="psP4", bufs=4, space="PSUM") as psP:
            for ic in range(ICN):
                wo_resid_ic(psP, wo2, attnO, bo2_t, IT, ic)
        h3t = ca.tile([P, CT * NL], FP8, tag="h3t", name="h3t")
        h3 = layernorm(ca, xres, NL, "3", xb_src=xresb, dst=h3t)
        h3p = h3t.rearrange("p (kt n) -> p kt n", kt=CT)

        # ============= phase 8: GEGLU FF =============
        if DEBUG:
            for kt in range(CT):
                nc.sync.dma_start(out=nc.dbg["d_h3"][kt * P:(kt + 1) * P, :], in_=h3[kt])
        with tc.tile_pool(name="psY", bufs=1, space="PSUM") as psY, \
             tc.tile_pool(name="psF", bufs=2, space="PSUM") as psF, \
             tc.tile_pool(name="gp", bufs=3) as gp, \
             tc.tile_pool(name="op", bufs=3) as op:
            for ic in range(ICN):
                cs3 = slice(ic * NCH, (ic + 1) * NCH)
                pys = [psY.tile([P, NCH], F32, tag=f"y{m}", name=f"y{m}")
                       for m in range(CT)]
                for pi in range(FT):
                    ph = psF.tile([P, NCH], F32, tag="ph", name="ph")
                    pg = psF.tile([P, NCH], F32, tag="pg", name="pg")
                    for g in range(CPAIR):
                        nc.tensor.matmul(
                            ph,
                            lhsT=wff1d[:, g, :, pi * P:(pi + 1) * P],
                            rhs=h3p[:, 2 * g:2 * g + 2, cs3],
                            start=(g == 0), stop=(g == CPAIR - 1),
                            perf_mode=DR)
                    for g in range(CPAIR):
                        nc.tensor.matmul(
                            pg,
                            lhsT=wff1d[:, g, :, FFI + pi * P:FFI + (pi + 1) * P],
                            rhs=h3p[:, 2 * g:2 * g + 2, cs3],
                            start=(g == 0), stop=(g == CPAIR - 1),
                            perf_mode=DR)
                    gel = gp.tile([P, NCH], BF16, tag="gel", name="gel")
                    nc.scalar.activation(out=gel, in_=pg, func=AFT.Gelu,
                                         scale=1.0 / SW,
                                         bias=bff1_t[:, FT + pi:FT + pi + 1])
                    hb = tp.tile([P, NCH], F32, tag="hb", name="hb")
                    nc.scalar.activation(out=hb, in_=ph, func=AFT.Identity,
                                         scale=1.0 / SW,
                                         bias=bff1_t[:, pi:pi + 1])
                    nc.vector.tensor_mul(out=ffh_t[:, pi % 2], in0=hb,
                                         in1=gel)
                    if pi % 2 == 1:
                        g2 = pi // 2
                        for mt in range(CT):
                            nc.tensor.matmul(
                                pys[mt],
                                lhsT=wff2d[:, g2, :, mt * P:(mt + 1) * P],
                                rhs=ffh_t,
                                start=(g2 == 0), stop=(g2 == FPAIR - 1),
                                perf_mode=DR)
                for mt in range(CT):
                    t1 = tp.tile([P, NCH], F32, tag="t1", name="t1")
                    nc.scalar.activation(out=t1, in_=pys[mt], func=AFT.Identity,
                                         scale=1.0 / SW,
                                         bias=bff2_t[:, mt:mt + 1])
                    ot = op.tile([P, NCH], F32, tag="ot", name="ot")
                    nc.vector.tensor_add(out=ot, in0=t1,
                                         in1=xres[mt][:, ic * NCH:(ic + 1) * NCH])
                    nc.sync.dma_start(
                        out=out_d[mt * P:(mt + 1) * P, ic * NCH:(ic + 1) * NCH],
                        in_=ot)
        ca_cm.__exit__(None, None, None)
        wffp_cm.__exit__(None, None, None)


def _split_multi_waits(nc):
    """This walrus build accepts at most one sem-wait per instruction; Tile
    emits several. Split extras into standalone InstEventSemaphore pre-waits
    on the same engine (engines execute their stream in order, so semantics
    are preserved)."""
    n = 0
    for fn in nc.m.functions:
        for blk in fn.blocks:
            out = []
            for inst in blk.instructions:
                si = inst.sync_info
                if si is not None and si.on_wait and len(si.on_wait) > 1:
                    waits = list(si.on_wait)
                    for i, w in enumerate(waits[:-1]):
                        out.append(mybir.InstEventSemaphore(
                            name=f"{inst.name}-w{i}",
                            engine=inst.engine,
                            sync_info=mybir.SyncInfo(on_wait=[w], on_update=[]),
                        ))
                        n += 1
                    inst.sync_info = mybir.SyncInfo(
                        on_wait=[waits[-1]], on_update=list(si.on_update))
                out.append(inst)
            blk.instructions = out
    return n


def _build():
    nc = bass.Bass()
    nc.x_d = nc.dram_tensor("x", [C, NL], F32, kind="ExternalInput")
    nc.xb_d = nc.dram_tensor("xb", [C, N], BF16, kind="ExternalInput")
    nc.ctx_d = nc.dram_tensor("ctx", [CTXC, MCTX], BF16, kind="ExternalInput")
    nc.w_d = {}
    for name, shape in [
        ("wq1t", [C, INNER]), ("wk1t", [C, INNER]), ("wv1t", [C, INNER]),
        ("wo1t", [INNER, C]),
        ("wq2t", [C, INNER]), ("wk2t", [CTXC, INNER]), ("wv2t", [CTXC, INNER]),
        ("wo2t", [INNER, C]),
    ]:
        nc.w_d[name] = nc.dram_tensor(name, shape, BF16, kind="ExternalInput")
    nc.w_d["wff1t"] = nc.dram_tensor("wff1t", [P, CPAIR * 2 * 2 * FFI], FP8,
                                     kind="ExternalInput")
    nc.w_d["wff2t"] = nc.dram_tensor("wff2t", [P, FPAIR * 2 * C], FP8,
                                     kind="ExternalInput")
    nc.b_d = {}
    for name, n in [("bo1", C), ("bo2", C), ("bff1", 2 * FFI), ("bff2", C)]:
        nc.b_d[name] = nc.dram_tensor(name, [n], F32, kind="ExternalInput")
    nc.out_d = nc.dram_tensor("out", [C, NL], F32, kind="ExternalOutput")
    nc.dbg = {}
    if DEBUG:
        for name, shape, dt in [
            ("d_h1", [C, N], BF16), ("d_q1", [C, NL], BF16),
            ("d_k1", [C, N], BF16), ("d_attnO1", [C, NL], BF16),
            ("d_x1", [C, NL], F32), ("d_x2", [C, NL], F32),
            ("d_h3", [C, NL], BF16),
        ]:
            nc.dbg[name] = nc.dram_tensor(name, shape, dt, kind="ExternalOutput")
    with tile.TileContext(nc) as tc:
        _emit(tc)
    _split_multi_waits(nc)
    return nc


_CACHE = {}


def _get_program():
    if "nc" not in _CACHE:
        _CACHE["nc"] = _build()
    return _CACHE["nc"]


def _dr_weight(A, npair):
    """A: [K, M] f32 (already scaled). Returns [128, npair*2*M] fp8 in
    DoubleRow layout: out[p, g, i, m] = A[(2g+i)*128+p, m]."""
    K, M = A.shape
    assert K == npair * 2 * P
    t = A.reshape(npair, 2, P, M).transpose(2, 0, 1, 3)
    return np.ascontiguousarray(t.reshape(P, npair * 2 * M)).astype(E4NP)


def _prep_shared(inputs):
    f32 = np.float32
    g1 = np.asarray(inputs["g1"], f32)
    g2 = np.asarray(inputs["g2"], f32)
    g3 = np.asarray(inputs["g3"], f32)
    scale = DH ** -0.5
    d = {
        "wq1t": np.ascontiguousarray(
            (np.asarray(inputs["Wq1"], f32) * scale * g1[None, :]).T).astype(BF16NP),
        "wk1t": np.ascontiguousarray(
            (np.asarray(inputs["Wk1"], f32) * g1[None, :]).T).astype(BF16NP),
        "wv1t": np.ascontiguousarray(
            (np.asarray(inputs["Wv1"], f32) * g1[None, :]).T).astype(BF16NP),
        "wo1t": np.ascontiguousarray(np.asarray(inputs["Wo1"], f32).T).astype(BF16NP),
        "wq2t": np.ascontiguousarray(
            (np.asarray(inputs["Wq2"], f32) * scale * g2[None, :]).T).astype(BF16NP),
        "wk2t": np.ascontiguousarray(np.asarray(inputs["Wk2"], f32).T).astype(BF16NP),
        "wv2t": np.ascontiguousarray(np.asarray(inputs["Wv2"], f32).T).astype(BF16NP),
        "wo2t": np.ascontiguousarray(np.asarray(inputs["Wo2"], f32).T).astype(BF16NP),
        "wff1t": _dr_weight(
            (np.asarray(inputs["Wff1"], f32) * g3[None, :]).T * SW, CPAIR),
        "wff2t": _dr_weight(np.asarray(inputs["Wff2"], f32).T * SW, FPAIR),
        "bo1": np.ascontiguousarray(np.asarray(inputs["bo1"], f32)),
        "bo2": np.ascontiguousarray(np.asarray(inputs["bo2"], f32)),
        "bff1": np.ascontiguousarray(np.asarray(inputs["bff1"], f32)),
        "bff2": np.ascontiguousarray(np.asarray(inputs["bff2"], f32)),
    }
    return d


def make_in_maps(inputs):
    x = np.asarray(inputs["x"], np.float32)
    ctxf = np.asarray(inputs["context"], np.float32)
    shared = _prep_shared(inputs)
    in_maps = []
    for core in range(8):
        b, s = core // 2, core % 2
        xb = x[b]
        if s:
            xc = np.ascontiguousarray(
                np.concatenate([xb[:, NL:], xb[:, :NL]], axis=1))
        else:
            xc = np.ascontiguousarray(xb)
        m = dict(shared)
        m["x"] = np.ascontiguousarray(xc[:, :NL])
        m["xb"] = xc.astype(BF16NP)
        m["ctx"] = np.ascontiguousarray(ctxf[b]).astype(BF16NP)
        in_maps.append(m)
    return in_maps


def kernel(**inputs):
    nc = _get_program()
    in_maps = make_in_maps(inputs)
    res = run_bass_kernel_spmd(nc, in_maps, core_ids=list(range(8)))
    out = np.empty((B, C, N), np.float32)
    for core in range(8):
        b, s = core // 2, core % 2
        out[b][:, s * NL:(s + 1) * NL] = res.results[core]["out"]
    return out



# revision 36
# speedup vs baseline: 1.1643x; 1.0509x over previous
"""Trainium2 Bass kernel for a BasicTransformerBlock (self-attn + cross-attn + GEGLU FF).

Sharding: 8 cores = (batch b in 0..3) x (sequence half s in 0..1). No collectives.
Each core receives the full x[b] [512, 2048] (rotated so its local half is always
columns 0..1023), builds self-attention K/V over all 2048 positions, and computes
LN/Q/attention/FF only for its local 1024 positions. Output [512, 1024] per core.

Numerics: bf16 matmuls with fp32 PSUM accumulation; LayerNorm gains folded into the
following weight matrices on the host; attention softmax computed without
max-subtraction (scores are bounded ~+-1.5 here); softmax denominator obtained by
augmenting V^T with a ones-column (row 64 of the AV output = sum_j exp).
"""

import os
import sys

import numpy as np

for _p in ("/opt/trn_rl_repo", "/root/.axon_site/_ro/trn_rl_repo"):
    if os.path.isdir(_p) and _p not in sys.path:
        sys.path.insert(0, _p)

import ml_dtypes

import concourse.bass as bass
import concourse.tile as tile
from concourse import mybir
from concourse.bass_utils import run_bass_kernel_spmd

BF16NP = ml_dtypes.bfloat16
E4NP = ml_dtypes.float8_e4m3
AFT = mybir.ActivationFunctionType
DR = mybir.MatmulPerfMode.DoubleRow
F32 = mybir.dt.float32
BF16 = mybir.dt.bfloat16
FP8 = mybir.dt.float8e4

# Problem dims (hardcoded per spec)
P = 128
B = 4
C = 512      # model dim
N = 2048     # full seq len
NL = 1024    # local seq len per core
CTXC = 768   # context channels
MCTX = 256   # context seq len
H = 8
DH = 64
INNER = 512
FFI = 2048
EPS = 1e-5

CT = C // P        # 4 channel tiles
IT = INNER // P    # 4 inner tiles
XT = CTXC // P     # 6 ctx channel tiles
FT = FFI // P      # 16 ff tiles
CPAIR = CT // 2    # 2 channel-tile pairs
FPAIR = FT // 2    # 8 ff-tile pairs
SW = 64.0          # fp8 weight pre-scale (FF block + self-attn QKV)
SEXP = 1.0 / (SW * SW)  # self-attn scores psum = 64q * 64k = 4096 * true
NCH = 512          # free-dim chunk size
ICN = NL // NCH    # 2 local i-chunks
JT1 = N // P       # 16 self-attn j tiles
JT2 = MCTX // P    # 2 cross-attn j tiles
DEBUG = False


def _emit(tc):
    nc = tc.nc
    from contextlib import ExitStack

    with ExitStack() as ctx:
        ctx.enter_context(nc.allow_low_precision(
            reason="bf16 rows/broadcasts validated end-to-end vs fp32 reference"))
        main = ctx.enter_context(tc.tile_pool(name="main", bufs=1))
        tp = ctx.enter_context(tc.tile_pool(name="tp", bufs=4))

        x_d = nc.x_d
        ctx_d = nc.ctx_d
        w_d = nc.w_d
        b_d = nc.b_d
        out_d = nc.out_d

        # ---- constants ----
        ones_col = main.tile([P, 1], F32, tag="ones_col", name="ones_col")
        nc.vector.memset(ones_col, 1.0)
        ones_col_bf = main.tile([P, 1], BF16, tag="ones_col_bf", name="ones_col_bf")
        nc.vector.memset(ones_col_bf, 1.0)
        ones_row = main.tile([1, P], BF16, tag="ones_row", name="ones_row")
        nc.vector.memset(ones_row, 1.0)
        eps_t = main.tile([P, 1], F32, tag="eps", name="eps")
        nc.vector.memset(eps_t, EPS)

        # ---- load weights (attention ones up-front; FF weights later) ----
        def load_split(pool, tag, dram, nkt, cols, dtype):
            """One wide DMA for a [nkt*128, cols] DRAM tensor into a single
            [128, nkt*cols] SBUF tile; returns per-kt [128, cols] views."""
            t = pool.tile([P, nkt * cols], dtype, tag=tag, name=tag)
            nc.sync.dma_start(
                out=t.rearrange("p (kt c) -> p kt c", kt=nkt),
                in_=dram.rearrange("(kt p) c -> p kt c", p=P))
            return [t[:, kt * cols:(kt + 1) * cols] for kt in range(nkt)]

        def load_w(pool, name, nkt, cols):
            return load_split(pool, name, w_d[name], nkt, cols, BF16)


        def load_bias(name, n):
            f = n // P
            t = main.tile([P, f], F32, tag=f"b_{name}", name=f"b_{name}")
            nc.sync.dma_start(out=t, in_=b_d[name].rearrange("(f p) -> p f", p=P))
            return t

        ca_cm = tc.tile_pool(name="ca", bufs=1)
        ca = ca_cm.__enter__()
        sa_cm = tc.tile_pool(name="sa", bufs=1)
        sa = sa_cm.__enter__()
        # ---- load activations (before weights: LN1 needs x first) ----
        xfp_cm = tc.tile_pool(name="xfull", bufs=1)
        xfp = xfp_cm.__enter__()
        # xfull: one [128, CT*N] tile, DMA'd in 4 column-chunks so LN1's
        # first chunk starts as soon as its slice lands
        xft = xfp.tile([P, CT * N], BF16, tag="xf", name="xf")
        _xf_nc = N // NCH
        for cc in range(_xf_nc):
            nc.sync.dma_start(
                out=xft.rearrange("p (kt nc c) -> p nc kt c", kt=CT,
                                  nc=_xf_nc)[:, cc],
                in_=nc.xb_d.rearrange("(kt p) (nc c) -> p nc kt c", p=P,
                                      nc=_xf_nc)[:, cc])
        xfull = [xft[:, kt * N:(kt + 1) * N] for kt in range(CT)]
        xres = load_split(main, "xres", x_d, CT, NL, F32)
        ctx_sb = load_split(main, "ctx", ctx_d, XT, MCTX, BF16)

        # biases + weights after activations so LN1's x tiles arrive first
        bo1_t = load_bias("bo1", C)
        bo2_t = load_bias("bo2", C)
        bff1_t = load_bias("bff1", 2 * FFI)
        bff2_t = load_bias("bff2", C)
        def load_w8(name, npair, cols):
            t = main.tile([P, npair, 2, cols], FP8, tag=name, name=name)
            nc.sync.dma_start(out=t.rearrange("p a b c -> p (a b c)"),
                              in_=w_d[name][:])
            return t

        wq1d = load_w8("wq1t", CPAIR, INNER)
        wk1d = load_w8("wk1t", CPAIR, INNER)
        wv1d = load_w8("wv1t", CPAIR, INNER)
        wo1 = load_w(main, "wo1t", IT, C)
        wq2 = load_w(main, "wq2t", CT, INNER)
        wk2 = load_w(main, "wk2t", XT, INNER)
        wv2 = load_w(main, "wv2t", XT, INNER)
        wo2 = load_w(main, "wo2t", IT, C)

        attnO = [main.tile([P, NL], BF16, tag=f"attnO{t}", name=f"attnO{t}")
                 for t in range(IT)]
        # bf16 shadow of xres, refreshed during Wo phases so LN2/LN3 stats
        # read it without serial casts at the head of their chains
        xresb = [main.tile([P, NL], BF16, tag=f"xresb{t}", name=f"xresb{t}")
                 for t in range(CT)]

        # ---------- LayerNorm ----------
        def layernorm(hpool, src_tiles, ncols, lnid, xb_src=None, dst=None):
            if dst is not None:
                h_out = [dst[:, kt * ncols:(kt + 1) * ncols]
                         for kt in range(CT)]
            else:
                h_out = []
                for kt in range(CT):
                    h_out.append(hpool.tile([P, ncols], BF16, tag=f"h{kt}",
                                            name=f"h{lnid}_{kt}"))
            with tc.tile_pool(name=f"psLN{lnid}", bufs=2, space="PSUM") as psLN, \
                 tc.tile_pool(name=f"psB{lnid}", bufs=2, space="PSUM") as psB, \
                 tc.tile_pool(name=f"st{lnid}", bufs=1) as st:
                rows = make_ln_rows(st, ncols)
                for cc in range(ncols // NCH):
                    ln_chunk(src_tiles, rows, cc * NCH, lnid, psLN, psB, h_out,
                             cc * NCH, xb_src=xb_src)
            return h_out

        def make_ln_rows(st, ncols):
            mean_row = st.tile([1, ncols], BF16, tag="mrow", name="mrow")
            msq_row = st.tile([1, ncols], F32, tag="qrow", name="qrow")
            var_row = st.tile([1, ncols], BF16, tag="vrow", name="vrow")
            a_row = st.tile([1, ncols], BF16, tag="arow", name="arow")
            return (mean_row, msq_row, var_row, a_row)

        def ln_chunk(src_tiles, rows, col0, lnid, psLN, psB, h_out, hcol0,
                     xb_src=None):
            """LN stats+normalize for one 512-column chunk.

            src cols [col0, col0+NCH) -> h_out cols [hcol0.., ..+NCH)."""
            mean_row, msq_row, var_row, a_row = rows
            src_f32 = src_tiles[0].dtype == F32
            cs = slice(col0, col0 + NCH)
            rs = slice(hcol0, hcol0 + NCH)
            with tc.tile_pool(name=f"x2{lnid}c{col0}", bufs=3) as x2p:
                if xb_src is not None:
                    xb = [s[:, cs] for s in xb_src]
                elif src_f32:
                    xb = []
                    for kt in range(CT):
                        xbt = x2p.tile([P, NCH], BF16, tag="xb", name="xb")
                        nc.vector.tensor_copy(out=xbt, in_=src_tiles[kt][:, cs])
                        xb.append(xbt)
                else:
                    xb = [s[:, cs] for s in src_tiles]
                m_ps = psLN.tile([1, NCH], F32, tag="pp", name="m_ps")
                q_ps = psLN.tile([1, NCH], F32, tag="pp", name="q_ps")
                for kt in range(CT):
                    nc.tensor.matmul(m_ps, lhsT=ones_col_bf, rhs=xb[kt],
                                     start=(kt == 0), stop=(kt == CT - 1))
                for kt in range(CT):
                    x2 = x2p.tile([P, NCH], BF16, tag="x2", name="x2")
                    # gpsimd: both operands SBUF bf16; frees DVE for the
                    # normalize chain (gpsimd is otherwise idle)
                    nc.gpsimd.tensor_mul(out=x2, in0=xb[kt], in1=xb[kt])
                    nc.tensor.matmul(q_ps, lhsT=ones_col_bf, rhs=x2,
                                     start=(kt == 0), stop=(kt == CT - 1))
                nc.vector.tensor_scalar_mul(out=mean_row[0:1, rs], in0=m_ps,
                                            scalar1=1.0 / C)
                nc.vector.tensor_scalar_mul(out=msq_row[0:1, rs], in0=q_ps,
                                            scalar1=1.0 / C)
                nc.vector.tensor_mul(out=var_row[0:1, rs], in0=mean_row[0:1, rs],
                                     in1=mean_row[0:1, rs])
                nc.vector.tensor_sub(out=var_row[0:1, rs], in0=msq_row[0:1, rs],
                                     in1=var_row[0:1, rs])
                nc.scalar.activation(out=var_row[0:1, rs], in_=var_row[0:1, rs],
                                     func=AFT.Sqrt, bias=eps_t[0:1, 0:1])
                nc.vector.reciprocal(out=a_row[0:1, rs], in_=var_row[0:1, rs])
                mb = psB.tile([P, NCH], F32, tag="pp", name="mb")
                ab = psB.tile([P, NCH], F32, tag="pp", name="ab")
                nc.tensor.matmul(mb, lhsT=ones_row, rhs=mean_row[0:1, rs],
                                 start=True, stop=True)
                nc.tensor.matmul(ab, lhsT=ones_row, rhs=a_row[0:1, rs],
                                 start=True, stop=True)
                for kt in range(CT):
                    t1 = tp.tile([P, NCH], F32, tag="t1", name="t1")
                    nc.vector.tensor_sub(out=t1, in0=src_tiles[kt][:, cs], in1=mb)
                    nc.vector.tensor_mul(out=h_out[kt][:, rs], in0=t1, in1=ab)

        # ---------- DR projection helpers (self-attn, fp8 x64) ----------
        def proj_dr8(psP, w_t, h_v, npair, out_mt, ncols, cb):
            cw = min(NCH, ncols)
            for mt in range(out_mt):
                for cc in range(ncols // cw):
                    ps = psP.tile([P, cw], F32, tag="pp", name="pp")
                    for g in range(npair):
                        nc.tensor.matmul(
                            ps, lhsT=w_t[:, g, :, mt * P:(mt + 1) * P],
                            rhs=h_v[:, 2 * g:2 * g + 2, cc * cw:(cc + 1) * cw],
                            start=(g == 0), stop=(g == npair - 1),
                            perf_mode=DR)
                    cb(mt, cc, cw, ps)

        def make_vt8(psP, pool, h_v, npair, w_t, jt, name):
            ps = psP.tile([P, INNER], F32, tag="pp", name="pp")
            for g in range(npair):
                nc.tensor.matmul(
                    ps, lhsT=h_v[:, 2 * g:2 * g + 2, jt * P:(jt + 1) * P],
                    rhs=w_t[:, g], start=(g == 0), stop=(g == npair - 1),
                    perf_mode=DR)
            vt = pool.tile([P, H, DH + 1], BF16, tag=f"vt{jt}", name=name)
            nc.vector.tensor_copy(
                out=vt[:, :, 0:DH],
                in_=ps.rearrange("p (h d) -> p h d", h=H))
            nc.vector.memset(vt[:, :, DH:DH + 1], 1.0)
            return vt

        # ---------- projection helper ----------
        def proj(psP, w_tiles, rhs_tiles, nkt, out_mt, ncols, cb):
            cw = min(NCH, ncols)
            for mt in range(out_mt):
                for cc in range(ncols // cw):
                    ps = psP.tile([P, cw], F32, tag="pp", name="pp")
                    for kt in range(nkt):
                        nc.tensor.matmul(
                            ps,
                            lhsT=w_tiles[kt][:, mt * P:(mt + 1) * P],
                            rhs=rhs_tiles[kt][:, cc * cw:(cc + 1) * cw],
                            start=(kt == 0), stop=(kt == nkt - 1))
                    cb(mt, cc, cw, ps)

        def make_vt(psP, pool, lhs_tiles, nkt, w_tiles, jt, name):
            ps = psP.tile([P, INNER], F32, tag="pp", name="pp")
            for kt in range(nkt):
                nc.tensor.matmul(
                    ps,
                    lhsT=lhs_tiles[kt][:, jt * P:(jt + 1) * P],
                    rhs=w_tiles[kt],
                    start=(kt == 0), stop=(kt == nkt - 1))
            vt = pool.tile([P, H, DH + 1], BF16, tag=f"vt{jt}", name=name)
            nc.vector.tensor_copy(
                out=vt[:, :, 0:DH],
                in_=ps.rearrange("p (h d) -> p h d", h=H))
            nc.vector.memset(vt[:, :, DH:DH + 1], 1.0)
            return vt

        # ---------- attention ----------
        def attn_ic(k_sb, vt_sb, q_sb, njt, dst, ic, psS, psO, ep, rp,
                    escale=1.0):
            for hp in range(H // 2):
                t = hp
                po = [psO.tile([P, NCH], F32, tag=f"po{i}", name=f"po{i}")
                      for i in range(2)]
                for jt in range(njt):
                    ps = psS.tile([P, 2 * NCH], F32, tag="ps", name="ps")
                    for hh in range(2):
                        nc.tensor.matmul(
                            ps[:, hh * NCH:(hh + 1) * NCH],
                            lhsT=k_sb[t][hh * DH:(hh + 1) * DH, jt * P:(jt + 1) * P],
                            rhs=q_sb[t][hh * DH:(hh + 1) * DH, ic * NCH:(ic + 1) * NCH],
                            start=True, stop=True)
                    e = ep.tile([P, 2 * NCH], BF16, tag="e", name="e")
                    nc.scalar.activation(out=e, in_=ps, func=AFT.Exp,
                                         scale=escale)
                    for hh in range(2):
                        h = 2 * hp + hh
                        nc.tensor.matmul(
                            po[hh][0:DH + 1, :],
                            lhsT=vt_sb[jt][:, h, :],
                            rhs=e[:, hh * NCH:(hh + 1) * NCH],
                            start=(jt == 0), stop=(jt == njt - 1))
                for hh in range(2):
                    rrow = rp.tile([1, NCH], BF16, tag="rrow", name="rrow")
                    nc.vector.reciprocal(out=rrow, in_=po[hh][DH:DH + 1, :])
                    # broadcast 1/denom into po's unused partitions 64..127
                    nc.tensor.matmul(po[hh][DH:2 * DH, :],
                                     lhsT=ones_row[0:1, 0:DH], rhs=rrow,
                                     start=True, stop=True)
                    un = rp.tile([DH, NCH], BF16, tag="un", name="un")
                    nc.vector.tensor_copy(out=un, in_=po[hh][0:DH, :])
                    nc.vector.tensor_mul(
                        out=dst[t][hh * DH:(hh + 1) * DH, ic * NCH:(ic + 1) * NCH],
                        in0=un, in1=po[hh][DH:2 * DH, :])

        # ---------- output-proj + residual (one ic chunk) ----------
        def wo_resid_ic(psP, wo_tiles, src, bias_t, nkt, ic, oscale=1.0):
            cs = slice(ic * NCH, (ic + 1) * NCH)
            for mt in range(CT):
                ps = psP.tile([P, NCH], F32, tag="pp", name="pp")
                for kt in range(nkt):
                    nc.tensor.matmul(ps, lhsT=wo_tiles[kt][:, mt * P:(mt + 1) * P],
                                     rhs=src[kt][:, cs],
                                     start=(kt == 0), stop=(kt == nkt - 1))
                t1 = tp.tile([P, NCH], F32, tag="t1", name="t1")
                nc.scalar.activation(out=t1, in_=ps, func=AFT.Identity,
                                     scale=oscale, bias=bias_t[:, mt:mt + 1])
                nc.vector.tensor_add(out=xres[mt][:, cs], in0=t1,
                                     in1=xres[mt][:, cs])
                nc.vector.tensor_copy(out=xresb[mt][:, cs], in_=xres[mt][:, cs])

        # ================= phase 1: LN1 over the full sequence =================
        h1p_cm = tc.tile_pool(name="h1p", bufs=1)
        h1p = h1p_cm.__enter__()
        h1t = h1p.tile([P, CT * N], FP8, tag="h1t", name="h1t")
        h1 = layernorm(h1p, xfull, N, "1", dst=h1t)
        h1v = h1t.rearrange("p (kt n) -> p kt n", kt=CT)

        # ============= phase 2: Q/K/V projections (self) =============
        q1_sb = [sa.tile([P, NL], BF16, tag=f"q{t}", name=f"q1_{t}") for t in range(IT)]
        k1_sb = [sa.tile([P, N], BF16, tag=f"k{t}", name=f"k1_{t}") for t in range(IT)]
        with tc.tile_pool(name="psP1", bufs=4, space="PSUM") as psP:
            proj_dr8(psP, wq1d, h1v[:, :, 0:NL].rearrange("p k n -> p k n"),
                     CPAIR, IT, NL,
                     lambda mt, cc, cw, ps: nc.vector.tensor_copy(
                         out=q1_sb[mt][:, cc * cw:(cc + 1) * cw], in_=ps))
            proj_dr8(psP, wk1d, h1v, CPAIR, IT, N,
                     lambda mt, cc, cw, ps: nc.vector.tensor_copy(
                         out=k1_sb[mt][:, cc * cw:(cc + 1) * cw], in_=ps))
            vt1_sb = [make_vt8(psP, sa, h1v, CPAIR, wv1d, jt, f"vt1_{jt}")
                      for jt in range(JT1)]
            # cross-attn K2/V2T depend only on ctx: emit early so the PE work
            # fills self-attention's ACT-bound phase
            k2_sb = [ca.tile([P, MCTX], BF16, tag=f"k{t}", name=f"k2_{t}")
                     for t in range(IT)]
            proj(psP, wk2, ctx_sb, XT, IT, MCTX,
                 lambda mt, cc, cw, ps: nc.vector.tensor_copy(
                     out=k2_sb[mt][:, cc * cw:(cc + 1) * cw], in_=ps))
            vt2_sb = [make_vt(psP, ca, ctx_sb, XT, wv2, jt, f"vt2_{jt}")
                      for jt in range(JT2)]
        if DEBUG:
            for kt in range(CT):
                nc.sync.dma_start(out=nc.dbg["d_h1"][kt * P:(kt + 1) * P, :], in_=h1[kt])
                nc.sync.dma_start(out=nc.dbg["d_q1"][kt * P:(kt + 1) * P, :], in_=q1_sb[kt])
                nc.sync.dma_start(out=nc.dbg["d_k1"][kt * P:(kt + 1) * P, :], in_=k1_sb[kt])
        h1p_cm.__exit__(None, None, None)
        xfp_cm.__exit__(None, None, None)

        # ===== phase 3: self-attention =====
        with tc.tile_pool(name="psS", bufs=2, space="PSUM") as psS, \
             tc.tile_pool(name="psO", bufs=2, space="PSUM") as psO, \
             tc.tile_pool(name="ep", bufs=6) as ep, \
             tc.tile_pool(name="rp", bufs=4) as rp:
            for ic in range(ICN):
                attn_ic(k1_sb, vt1_sb, q1_sb, JT1, attnO, ic, psS, psO,
                        ep, rp, escale=SEXP)
        sa_cm.__exit__(None, None, None)
        wffp_cm = tc.tile_pool(name="wffp", bufs=1, side="right")
        wffp = wffp_cm.__enter__()
        wff1d = wffp.tile([P, CPAIR, 2, 2 * FFI], FP8, tag="wff1t",
                          name="wff1t")
        nc.sync.dma_start(out=wff1d.rearrange("p a b c -> p (a b c)"),
                          in_=w_d["wff1t"][:])
        wff2d = wffp.tile([P, FPAIR, 2, C], FP8, tag="wff2t", name="wff2t")
        nc.sync.dma_start(out=wff2d.rearrange("p a b c -> p (a b c)"),
                          in_=w_d["wff2t"][:])
        ffh_t = wffp.tile([P, 2, NCH], FP8, tag="ffh_t", name="ffh_t")

        # ===== phase 4: Wo1 + residual =====
        with tc.tile_pool(name="psP2", bufs=4, space="PSUM") as psP:
            for ic in range(ICN):
                wo_resid_ic(psP, wo1, attnO, bo1_t, IT, ic, oscale=1.0 / SW)

        # ===== phase 5: LN2 + Q2 =====
        h2 = layernorm(ca, xres, NL, "2", xb_src=xresb)
        q2_sb = [ca.tile([P, NL], BF16, tag=f"q{t}", name=f"q2_{t}")
                 for t in range(IT)]
        with tc.tile_pool(name="psP3", bufs=4, space="PSUM") as psP:
            proj(psP, wq2, h2, CT, IT, NL,
                 lambda mt, cc, cw, ps: nc.vector.tensor_copy(
                     out=q2_sb[mt][:, cc * cw:(cc + 1) * cw], in_=ps))

        # ===== phase 6: cross-attention =====
        with tc.tile_pool(name="psS2", bufs=2, space="PSUM") as psS, \
             tc.tile_pool(name="psO2", bufs=2, space="PSUM") as psO, \
             tc.tile_pool(name="ep2", bufs=6) as ep, \
             tc.tile_pool(name="rp2", bufs=4) as rp:
            for ic in range(ICN):
                attn_ic(k2_sb, vt2_sb, q2_sb, JT2, attnO, ic, psS, psO,
                        ep, rp)

        # ===== phase 7: Wo2 + residual, then LN3 =====
        with tc.tile_pool(name="psP4", bufs=4, space="PSUM") as psP:
            for ic in range(ICN):
                wo_resid_ic(psP, wo2, attnO, bo2_t, IT, ic)
        h3t = ca.tile([P, CT * NL], FP8, tag="h3t", name="h3t")
        h3 = layernorm(ca, xres, NL, "3", xb_src=xresb, dst=h3t)
        h3p = h3t.rearrange("p (kt n) -> p kt n", kt=CT)

        # ============= phase 8: GEGLU FF =============
        if DEBUG:
            for kt in range(CT):
                nc.sync.dma_start(out=nc.dbg["d_h3"][kt * P:(kt + 1) * P, :], in_=h3[kt])
        with tc.tile_pool(name="psY", bufs=1, space="PSUM") as psY, \
             tc.tile_pool(name="psF", bufs=2, space="PSUM") as psF, \
             tc.tile_pool(name="gp", bufs=3) as gp, \
             tc.tile_pool(name="op", bufs=3) as op:
            for ic in range(ICN):
                cs3 = slice(ic * NCH, (ic + 1) * NCH)
                pys = [psY.tile([P, NCH], F32, tag=f"y{m}", name=f"y{m}")
                       for m in range(CT)]
                for pi in range(FT):
                    ph = psF.tile([P, NCH], F32, tag="ph", name="ph")
                    pg = psF.tile([P, NCH], F32, tag="pg", name="pg")
                    for g in range(CPAIR):
                        nc.tensor.matmul(
                            ph,
                            lhsT=wff1d[:, g, :, pi * P:(pi + 1) * P],
                            rhs=h3p[:, 2 * g:2 * g + 2, cs3],
                            start=(g == 0), stop=(g == CPAIR - 1),
                            perf_mode=DR)
                    for g in range(CPAIR):
                        nc.tensor.matmul(
                            pg,
                            lhsT=wff1d[:, g, :, FFI + pi * P:FFI + (pi + 1) * P],
                            rhs=h3p[:, 2 * g:2 * g + 2, cs3],
                            start=(g == 0), stop=(g == CPAIR - 1),
                            perf_mode=DR)
                    gel = gp.tile([P, NCH], BF16, tag="gel", name="gel")
                    nc.scalar.activation(out=gel, in_=pg, func=AFT.Gelu,
                                         scale=1.0 / SW,
                                         bias=bff1_t[:, FT + pi:FT + pi + 1])
                    hb = tp.tile([P, NCH], F32, tag="hb", name="hb")
                    nc.scalar.activation(out=hb, in_=ph, func=AFT.Identity,
                                         scale=1.0 / SW,
                                         bias=bff1_t[:, pi:pi + 1])
                    nc.vector.tensor_mul(out=ffh_t[:, pi % 2], in0=hb,
                                         in1=gel)
                    if pi % 2 == 1:
                        g2 = pi // 2
                        for mt in range(CT):
                            nc.tensor.matmul(
                                pys[mt],
                                lhsT=wff2d[:, g2, :, mt * P:(mt + 1) * P],
                                rhs=ffh_t,
                                start=(g2 == 0), stop=(g2 == FPAIR - 1),
                                perf_mode=DR)
                for mt in range(CT):
                    t1 = tp.tile([P, NCH], F32, tag="t1", name="t1")
                    nc.scalar.activation(out=t1, in_=pys[mt], func=AFT.Identity,
                                         scale=1.0 / SW,
                                         bias=bff2_t[:, mt:mt + 1])
                    ot = op.tile([P, NCH], F32, tag="ot", name="ot")
                    nc.vector.tensor_add(out=ot, in0=t1,
                                         in1=xres[mt][:, ic * NCH:(ic + 1) * NCH])
                    nc.sync.dma_start(
                        out=out_d[mt * P:(mt + 1) * P, ic * NCH:(ic + 1) * NCH],
                        in_=ot)
        ca_cm.__exit__(None, None, None)
        wffp_cm.__exit__(None, None, None)


def _split_multi_waits(nc):
    """This walrus build accepts at most one sem-wait per instruction; Tile
    emits several. Split extras into standalone InstEventSemaphore pre-waits
    on the same engine (engines execute their stream in order, so semantics
    are preserved)."""
    n = 0
    for fn in nc.m.functions:
        for blk in fn.blocks:
            out = []
            for inst in blk.instructions:
                si = inst.sync_info
                if si is not None and si.on_wait and len(si.on_wait) > 1:
                    waits = list(si.on_wait)
                    for i, w in enumerate(waits[:-1]):
                        out.append(mybir.InstEventSemaphore(
                            name=f"{inst.name}-w{i}",
                            engine=inst.engine,
                            sync_info=mybir.SyncInfo(on_wait=[w], on_update=[]),
                        ))
                        n += 1
                    inst.sync_info = mybir.SyncInfo(
                        on_wait=[waits[-1]], on_update=list(si.on_update))
                out.append(inst)
            blk.instructions = out
    return n


def _build():
    nc = bass.Bass()
    nc.x_d = nc.dram_tensor("x", [C, NL], F32, kind="ExternalInput")
    nc.xb_d = nc.dram_tensor("xb", [C, N], BF16, kind="ExternalInput")
    nc.ctx_d = nc.dram_tensor("ctx", [CTXC, MCTX], BF16, kind="ExternalInput")
    nc.w_d = {}
    for name, shape in [
        ("wo1t", [INNER, C]),
        ("wq2t", [C, INNER]), ("wk2t", [CTXC, INNER]), ("wv2t", [CTXC, INNER]),
        ("wo2t", [INNER, C]),
    ]:
        nc.w_d[name] = nc.dram_tensor(name, shape, BF16, kind="ExternalInput")
    for name in ("wq1t", "wk1t", "wv1t"):
        nc.w_d[name] = nc.dram_tensor(name, [P, CPAIR * 2 * INNER], FP8,
                                      kind="ExternalInput")
    nc.w_d["wff1t"] = nc.dram_tensor("wff1t", [P, CPAIR * 2 * 2 * FFI], FP8,
                                     kind="ExternalInput")
    nc.w_d["wff2t"] = nc.dram_tensor("wff2t", [P, FPAIR * 2 * C], FP8,
                                     kind="ExternalInput")
    nc.b_d = {}
    for name, n in [("bo1", C), ("bo2", C), ("bff1", 2 * FFI), ("bff2", C)]:
        nc.b_d[name] = nc.dram_tensor(name, [n], F32, kind="ExternalInput")
    nc.out_d = nc.dram_tensor("out", [C, NL], F32, kind="ExternalOutput")
    nc.dbg = {}
    if DEBUG:
        for name, shape, dt in [
            ("d_h1", [C, N], BF16), ("d_q1", [C, NL], BF16),
            ("d_k1", [C, N], BF16), ("d_attnO1", [C, NL], BF16),
            ("d_x1", [C, NL], F32), ("d_x2", [C, NL], F32),
            ("d_h3", [C, NL], BF16),
        ]:
            nc.dbg[name] = nc.dram_tensor(name, shape, dt, kind="ExternalOutput")
    with tile.TileContext(nc) as tc:
        _emit(tc)
    _split_multi_waits(nc)
    return nc


_CACHE = {}


def _get_program():
    if "nc" not in _CACHE:
        _CACHE["nc"] = _build()
    return _CACHE["nc"]


def _dr_weight(A, npair):
    """A: [K, M] f32 (already scaled). Returns [128, npair*2*M] fp8 in
    DoubleRow layout: out[p, g, i, m] = A[(2g+i)*128+p, m]."""
    K, M = A.shape
    assert K == npair * 2 * P
    t = A.reshape(npair, 2, P, M).transpose(2, 0, 1, 3)
    return np.ascontiguousarray(t.reshape(P, npair * 2 * M)).astype(E4NP)


def _prep_shared(inputs):
    f32 = np.float32
    g1 = np.asarray(inputs["g1"], f32)
    g2 = np.asarray(inputs["g2"], f32)
    g3 = np.asarray(inputs["g3"], f32)
    scale = DH ** -0.5
    d = {
        "wq1t": _dr_weight(
            (np.asarray(inputs["Wq1"], f32) * scale * g1[None, :]).T * SW,
            CPAIR),
        "wk1t": _dr_weight(
            (np.asarray(inputs["Wk1"], f32) * g1[None, :]).T * SW, CPAIR),
        "wv1t": _dr_weight(
            (np.asarray(inputs["Wv1"], f32) * g1[None, :]).T * SW, CPAIR),
        "wo1t": np.ascontiguousarray(np.asarray(inputs["Wo1"], f32).T).astype(BF16NP),
        "wq2t": np.ascontiguousarray(
            (np.asarray(inputs["Wq2"], f32) * scale * g2[None, :]).T).astype(BF16NP),
        "wk2t": np.ascontiguousarray(np.asarray(inputs["Wk2"], f32).T).astype(BF16NP),
        "wv2t": np.ascontiguousarray(np.asarray(inputs["Wv2"], f32).T).astype(BF16NP),
        "wo2t": np.ascontiguousarray(np.asarray(inputs["Wo2"], f32).T).astype(BF16NP),
        "wff1t": _dr_weight(
            (np.asarray(inputs["Wff1"], f32) * g3[None, :]).T * SW, CPAIR),
        "wff2t": _dr_weight(np.asarray(inputs["Wff2"], f32).T * SW, FPAIR),
        "bo1": np.ascontiguousarray(np.asarray(inputs["bo1"], f32)),
        "bo2": np.ascontiguousarray(np.asarray(inputs["bo2"], f32)),
        "bff1": np.ascontiguousarray(np.asarray(inputs["bff1"], f32)),
        "bff2": np.ascontiguousarray(np.asarray(inputs["bff2"], f32)),
    }
    return d


def make_in_maps(inputs):
    x = np.asarray(inputs["x"], np.float32)
    ctxf = np.asarray(inputs["context"], np.float32)
    shared = _prep_shared(inputs)
    in_maps = []
    for core in range(8):
        b, s = core // 2, core % 2
        xb = x[b]
        if s:
            xc = np.ascontiguousarray(
                np.concatenate([xb[:, NL:], xb[:, :NL]], axis=1))
        else:
            xc = np.ascontiguousarray(xb)
        m = dict(shared)
        m["x"] = np.ascontiguousarray(xc[:, :NL])
        m["xb"] = xc.astype(BF16NP)
        m["ctx"] = np.ascontiguousarray(ctxf[b]).astype(BF16NP)
        in_maps.append(m)
    return in_maps


def kernel(**inputs):
    nc = _get_program()
    in_maps = make_in_maps(inputs)
    res = run_bass_kernel_spmd(nc, in_maps, core_ids=list(range(8)))
    out = np.empty((B, C, N), np.float32)
    for core in range(8):
        b, s = core // 2, core % 2
        out[b][:, s * NL:(s + 1) * NL] = res.results[core]["out"]
    return out



# revision 37
# speedup vs baseline: 1.1870x; 1.0195x over previous
"""Trainium2 Bass kernel for a BasicTransformerBlock (self-attn + cross-attn + GEGLU FF).

Sharding: 8 cores = (batch b in 0..3) x (sequence half s in 0..1). No collectives.
Each core receives the full x[b] [512, 2048] (rotated so its local half is always
columns 0..1023), builds self-attention K/V over all 2048 positions, and computes
LN/Q/attention/FF only for its local 1024 positions. Output [512, 1024] per core.

Numerics: bf16 matmuls with fp32 PSUM accumulation; LayerNorm gains folded into the
following weight matrices on the host; attention softmax computed without
max-subtraction (scores are bounded ~+-1.5 here); softmax denominator obtained by
augmenting V^T with a ones-column (row 64 of the AV output = sum_j exp).
"""

import os
import sys

import numpy as np

for _p in ("/opt/trn_rl_repo", "/root/.axon_site/_ro/trn_rl_repo"):
    if os.path.isdir(_p) and _p not in sys.path:
        sys.path.insert(0, _p)

import ml_dtypes

import concourse.bass as bass
import concourse.tile as tile
from concourse import mybir
from concourse.bass_utils import run_bass_kernel_spmd

BF16NP = ml_dtypes.bfloat16
E4NP = ml_dtypes.float8_e4m3
AFT = mybir.ActivationFunctionType
DR = mybir.MatmulPerfMode.DoubleRow
F32 = mybir.dt.float32
BF16 = mybir.dt.bfloat16
FP8 = mybir.dt.float8e4

# Problem dims (hardcoded per spec)
P = 128
B = 4
C = 512      # model dim
N = 2048     # full seq len
NL = 1024    # local seq len per core
CTXC = 768   # context channels
MCTX = 256   # context seq len
H = 8
DH = 64
INNER = 512
FFI = 2048
EPS = 1e-5

CT = C // P        # 4 channel tiles
IT = INNER // P    # 4 inner tiles
XT = CTXC // P     # 6 ctx channel tiles
XPAIR = XT // 2    # 3 ctx channel-tile pairs
FT = FFI // P      # 16 ff tiles
CPAIR = CT // 2    # 2 channel-tile pairs
FPAIR = FT // 2    # 8 ff-tile pairs
SW = 64.0          # fp8 weight pre-scale (FF block + self-attn QKV)
SEXP = 1.0 / (SW * SW)  # self-attn scores psum = 64q * 64k = 4096 * true
NCH = 512          # free-dim chunk size
ICN = NL // NCH    # 2 local i-chunks
JT1 = N // P       # 16 self-attn j tiles
JT2 = MCTX // P    # 2 cross-attn j tiles
DEBUG = False


def _emit(tc):
    nc = tc.nc
    from contextlib import ExitStack

    with ExitStack() as ctx:
        ctx.enter_context(nc.allow_low_precision(
            reason="bf16 rows/broadcasts validated end-to-end vs fp32 reference"))
        main = ctx.enter_context(tc.tile_pool(name="main", bufs=1))
        tp = ctx.enter_context(tc.tile_pool(name="tp", bufs=4))

        x_d = nc.x_d
        ctx_d = nc.ctx_d
        w_d = nc.w_d
        b_d = nc.b_d
        out_d = nc.out_d

        # ---- constants ----
        ones_col = main.tile([P, 1], F32, tag="ones_col", name="ones_col")
        nc.vector.memset(ones_col, 1.0)
        ones_col_bf = main.tile([P, 1], BF16, tag="ones_col_bf", name="ones_col_bf")
        nc.vector.memset(ones_col_bf, 1.0)
        ones_row = main.tile([1, P], BF16, tag="ones_row", name="ones_row")
        nc.vector.memset(ones_row, 1.0)
        eps_t = main.tile([P, 1], F32, tag="eps", name="eps")
        nc.vector.memset(eps_t, EPS)

        # ---- load weights (attention ones up-front; FF weights later) ----
        def load_split(pool, tag, dram, nkt, cols, dtype):
            """One wide DMA for a [nkt*128, cols] DRAM tensor into a single
            [128, nkt*cols] SBUF tile; returns per-kt [128, cols] views."""
            t = pool.tile([P, nkt * cols], dtype, tag=tag, name=tag)
            nc.sync.dma_start(
                out=t.rearrange("p (kt c) -> p kt c", kt=nkt),
                in_=dram.rearrange("(kt p) c -> p kt c", p=P))
            return [t[:, kt * cols:(kt + 1) * cols] for kt in range(nkt)]

        def load_w(pool, name, nkt, cols):
            return load_split(pool, name, w_d[name], nkt, cols, BF16)


        def load_bias(name, n):
            f = n // P
            t = main.tile([P, f], F32, tag=f"b_{name}", name=f"b_{name}")
            nc.sync.dma_start(out=t, in_=b_d[name].rearrange("(f p) -> p f", p=P))
            return t

        ca_cm = tc.tile_pool(name="ca", bufs=1)
        ca = ca_cm.__enter__()
        sa_cm = tc.tile_pool(name="sa", bufs=1)
        sa = sa_cm.__enter__()
        # ---- load activations (before weights: LN1 needs x first) ----
        xfp_cm = tc.tile_pool(name="xfull", bufs=1)
        xfp = xfp_cm.__enter__()
        # xfull: one [128, CT*N] tile, DMA'd in 4 column-chunks so LN1's
        # first chunk starts as soon as its slice lands
        xft = xfp.tile([P, CT * N], BF16, tag="xf", name="xf")
        _xf_nc = N // NCH
        for cc in range(_xf_nc):
            nc.sync.dma_start(
                out=xft.rearrange("p (kt nc c) -> p nc kt c", kt=CT,
                                  nc=_xf_nc)[:, cc],
                in_=nc.xb_d.rearrange("(kt p) (nc c) -> p nc kt c", p=P,
                                      nc=_xf_nc)[:, cc])
        xfull = [xft[:, kt * N:(kt + 1) * N] for kt in range(CT)]
        xres = load_split(main, "xres", x_d, CT, NL, F32)
        ctx_t = main.tile([P, XT * MCTX], FP8, tag="ctx", name="ctx")
        nc.sync.dma_start(
            out=ctx_t.rearrange("p (kt c) -> p kt c", kt=XT),
            in_=ctx_d.rearrange("(kt p) c -> p kt c", p=P))
        ctxv = ctx_t.rearrange("p (kt m) -> p kt m", kt=XT)

        # biases + weights after activations so LN1's x tiles arrive first
        bo1_t = load_bias("bo1", C)
        bo2_t = load_bias("bo2", C)
        bff1_t = load_bias("bff1", 2 * FFI)
        bff2_t = load_bias("bff2", C)
        def load_w8(name, npair, cols):
            t = main.tile([P, npair, 2, cols], FP8, tag=name, name=name)
            nc.sync.dma_start(out=t.rearrange("p a b c -> p (a b c)"),
                              in_=w_d[name][:])
            return t

        wq1d = load_w8("wq1t", CPAIR, INNER)
        wk1d = load_w8("wk1t", CPAIR, INNER)
        wv1d = load_w8("wv1t", CPAIR, INNER)
        wo1d = load_w8("wo1t", CPAIR, C)
        wq2d = load_w8("wq2t", CPAIR, INNER)
        wk2d = load_w8("wk2t", XPAIR, INNER)
        wv2d = load_w8("wv2t", XPAIR, INNER)
        wo2d = load_w8("wo2t", CPAIR, C)

        attnOt = main.tile([P, IT * NL], FP8, tag="attnOt", name="attnOt")
        attnO = [attnOt[:, t * NL:(t + 1) * NL] for t in range(IT)]
        attnOv = attnOt.rearrange("p (kt n) -> p kt n", kt=IT)
        # bf16 shadow of xres, refreshed during Wo phases so LN2/LN3 stats
        # read it without serial casts at the head of their chains
        xresb = [main.tile([P, NL], BF16, tag=f"xresb{t}", name=f"xresb{t}")
                 for t in range(CT)]

        # ---------- LayerNorm ----------
        def layernorm(hpool, src_tiles, ncols, lnid, xb_src=None, dst=None):
            if dst is not None:
                h_out = [dst[:, kt * ncols:(kt + 1) * ncols]
                         for kt in range(CT)]
            else:
                h_out = []
                for kt in range(CT):
                    h_out.append(hpool.tile([P, ncols], BF16, tag=f"h{kt}",
                                            name=f"h{lnid}_{kt}"))
            with tc.tile_pool(name=f"psLN{lnid}", bufs=2, space="PSUM") as psLN, \
                 tc.tile_pool(name=f"psB{lnid}", bufs=2, space="PSUM") as psB, \
                 tc.tile_pool(name=f"st{lnid}", bufs=1) as st:
                rows = make_ln_rows(st, ncols)
                for cc in range(ncols // NCH):
                    ln_chunk(src_tiles, rows, cc * NCH, lnid, psLN, psB, h_out,
                             cc * NCH, xb_src=xb_src)
            return h_out

        def make_ln_rows(st, ncols):
            mean_row = st.tile([1, ncols], BF16, tag="mrow", name="mrow")
            msq_row = st.tile([1, ncols], F32, tag="qrow", name="qrow")
            var_row = st.tile([1, ncols], BF16, tag="vrow", name="vrow")
            a_row = st.tile([1, ncols], BF16, tag="arow", name="arow")
            return (mean_row, msq_row, var_row, a_row)

        def ln_chunk(src_tiles, rows, col0, lnid, psLN, psB, h_out, hcol0,
                     xb_src=None):
            """LN stats+normalize for one 512-column chunk.

            src cols [col0, col0+NCH) -> h_out cols [hcol0.., ..+NCH)."""
            mean_row, msq_row, var_row, a_row = rows
            src_f32 = src_tiles[0].dtype == F32
            cs = slice(col0, col0 + NCH)
            rs = slice(hcol0, hcol0 + NCH)
            with tc.tile_pool(name=f"x2{lnid}c{col0}", bufs=3) as x2p:
                if xb_src is not None:
                    xb = [s[:, cs] for s in xb_src]
                elif src_f32:
                    xb = []
                    for kt in range(CT):
                        xbt = x2p.tile([P, NCH], BF16, tag="xb", name="xb")
                        nc.vector.tensor_copy(out=xbt, in_=src_tiles[kt][:, cs])
                        xb.append(xbt)
                else:
                    xb = [s[:, cs] for s in src_tiles]
                m_ps = psLN.tile([1, NCH], F32, tag="pp", name="m_ps")
                q_ps = psLN.tile([1, NCH], F32, tag="pp", name="q_ps")
                for kt in range(CT):
                    nc.tensor.matmul(m_ps, lhsT=ones_col_bf, rhs=xb[kt],
                                     start=(kt == 0), stop=(kt == CT - 1))
                for kt in range(CT):
                    x2 = x2p.tile([P, NCH], BF16, tag="x2", name="x2")
                    # gpsimd: both operands SBUF bf16; frees DVE for the
                    # normalize chain (gpsimd is otherwise idle)
                    nc.gpsimd.tensor_mul(out=x2, in0=xb[kt], in1=xb[kt])
                    nc.tensor.matmul(q_ps, lhsT=ones_col_bf, rhs=x2,
                                     start=(kt == 0), stop=(kt == CT - 1))
                nc.vector.tensor_scalar_mul(out=mean_row[0:1, rs], in0=m_ps,
                                            scalar1=1.0 / C)
                nc.vector.tensor_scalar_mul(out=msq_row[0:1, rs], in0=q_ps,
                                            scalar1=1.0 / C)
                nc.vector.tensor_mul(out=var_row[0:1, rs], in0=mean_row[0:1, rs],
                                     in1=mean_row[0:1, rs])
                nc.vector.tensor_sub(out=var_row[0:1, rs], in0=msq_row[0:1, rs],
                                     in1=var_row[0:1, rs])
                nc.scalar.activation(out=var_row[0:1, rs], in_=var_row[0:1, rs],
                                     func=AFT.Sqrt, bias=eps_t[0:1, 0:1])
                nc.vector.reciprocal(out=a_row[0:1, rs], in_=var_row[0:1, rs])
                mb = psB.tile([P, NCH], F32, tag="pp", name="mb")
                ab = psB.tile([P, NCH], F32, tag="pp", name="ab")
                nc.tensor.matmul(mb, lhsT=ones_row, rhs=mean_row[0:1, rs],
                                 start=True, stop=True)
                nc.tensor.matmul(ab, lhsT=ones_row, rhs=a_row[0:1, rs],
                                 start=True, stop=True)
                for kt in range(CT):
                    t1 = tp.tile([P, NCH], F32, tag="t1", name="t1")
                    nc.vector.tensor_sub(out=t1, in0=src_tiles[kt][:, cs], in1=mb)
                    nc.vector.tensor_mul(out=h_out[kt][:, rs], in0=t1, in1=ab)

        # ---------- DR projection helpers (self-attn, fp8 x64) ----------
        def proj_dr8(psP, w_t, h_v, npair, out_mt, ncols, cb):
            cw = min(NCH, ncols)
            for mt in range(out_mt):
                for cc in range(ncols // cw):
                    ps = psP.tile([P, cw], F32, tag="pp", name="pp")
                    for g in range(npair):
                        nc.tensor.matmul(
                            ps, lhsT=w_t[:, g, :, mt * P:(mt + 1) * P],
                            rhs=h_v[:, 2 * g:2 * g + 2, cc * cw:(cc + 1) * cw],
                            start=(g == 0), stop=(g == npair - 1),
                            perf_mode=DR)
                    cb(mt, cc, cw, ps)

        def make_vt8(psP, pool, h_v, npair, w_t, jt, name):
            ps = psP.tile([P, INNER], F32, tag="pp", name="pp")
            for g in range(npair):
                nc.tensor.matmul(
                    ps, lhsT=h_v[:, 2 * g:2 * g + 2, jt * P:(jt + 1) * P],
                    rhs=w_t[:, g], start=(g == 0), stop=(g == npair - 1),
                    perf_mode=DR)
            vt = pool.tile([P, H, DH + 1], BF16, tag=f"vt{jt}", name=name)
            nc.vector.tensor_copy(
                out=vt[:, :, 0:DH],
                in_=ps.rearrange("p (h d) -> p h d", h=H))
            nc.vector.memset(vt[:, :, DH:DH + 1], 1.0)
            return vt

        # ---------- projection helper ----------
        def proj(psP, w_tiles, rhs_tiles, nkt, out_mt, ncols, cb):
            cw = min(NCH, ncols)
            for mt in range(out_mt):
                for cc in range(ncols // cw):
                    ps = psP.tile([P, cw], F32, tag="pp", name="pp")
                    for kt in range(nkt):
                        nc.tensor.matmul(
                            ps,
                            lhsT=w_tiles[kt][:, mt * P:(mt + 1) * P],
                            rhs=rhs_tiles[kt][:, cc * cw:(cc + 1) * cw],
                            start=(kt == 0), stop=(kt == nkt - 1))
                    cb(mt, cc, cw, ps)

        def make_vt(psP, pool, lhs_tiles, nkt, w_tiles, jt, name):
            ps = psP.tile([P, INNER], F32, tag="pp", name="pp")
            for kt in range(nkt):
                nc.tensor.matmul(
                    ps,
                    lhsT=lhs_tiles[kt][:, jt * P:(jt + 1) * P],
                    rhs=w_tiles[kt],
                    start=(kt == 0), stop=(kt == nkt - 1))
            vt = pool.tile([P, H, DH + 1], BF16, tag=f"vt{jt}", name=name)
            nc.vector.tensor_copy(
                out=vt[:, :, 0:DH],
                in_=ps.rearrange("p (h d) -> p h d", h=H))
            nc.vector.memset(vt[:, :, DH:DH + 1], 1.0)
            return vt

        # ---------- attention ----------
        def attn_ic(k_sb, vt_sb, q_sb, njt, dst, ic, psS, psO, ep, rp,
                    escale=1.0):
            for hp in range(H // 2):
                t = hp
                po = [psO.tile([P, NCH], F32, tag=f"po{i}", name=f"po{i}")
                      for i in range(2)]
                for jt in range(njt):
                    ps = psS.tile([P, 2 * NCH], F32, tag="ps", name="ps")
                    for hh in range(2):
                        nc.tensor.matmul(
                            ps[:, hh * NCH:(hh + 1) * NCH],
                            lhsT=k_sb[t][hh * DH:(hh + 1) * DH, jt * P:(jt + 1) * P],
                            rhs=q_sb[t][hh * DH:(hh + 1) * DH, ic * NCH:(ic + 1) * NCH],
                            start=True, stop=True)
                    e = ep.tile([P, 2 * NCH], BF16, tag="e", name="e")
                    nc.scalar.activation(out=e, in_=ps, func=AFT.Exp,
                                         scale=escale)
                    for hh in range(2):
                        h = 2 * hp + hh
                        nc.tensor.matmul(
                            po[hh][0:DH + 1, :],
                            lhsT=vt_sb[jt][:, h, :],
                            rhs=e[:, hh * NCH:(hh + 1) * NCH],
                            start=(jt == 0), stop=(jt == njt - 1))
                for hh in range(2):
                    rrow = rp.tile([1, NCH], BF16, tag="rrow", name="rrow")
                    nc.vector.reciprocal(out=rrow, in_=po[hh][DH:DH + 1, :])
                    # broadcast 1/denom into po's unused partitions 64..127
                    nc.tensor.matmul(po[hh][DH:2 * DH, :],
                                     lhsT=ones_row[0:1, 0:DH], rhs=rrow,
                                     start=True, stop=True)
                    un = rp.tile([DH, NCH], BF16, tag="un", name="un")
                    nc.vector.tensor_copy(out=un, in_=po[hh][0:DH, :])
                    nc.vector.tensor_mul(
                        out=dst[t][hh * DH:(hh + 1) * DH, ic * NCH:(ic + 1) * NCH],
                        in0=un, in1=po[hh][DH:2 * DH, :])

        # ---------- output-proj + residual (one ic chunk) ----------
        def wo_resid_ic(psP, wo_t, bias_t, ic, oscale=1.0):
            cs = slice(ic * NCH, (ic + 1) * NCH)
            for mt in range(CT):
                ps = psP.tile([P, NCH], F32, tag="pp", name="pp")
                for g in range(CPAIR):
                    nc.tensor.matmul(ps, lhsT=wo_t[:, g, :, mt * P:(mt + 1) * P],
                                     rhs=attnOv[:, 2 * g:2 * g + 2, cs],
                                     start=(g == 0), stop=(g == CPAIR - 1),
                                     perf_mode=DR)
                t1 = tp.tile([P, NCH], F32, tag="t1", name="t1")
                nc.scalar.activation(out=t1, in_=ps, func=AFT.Identity,
                                     scale=oscale, bias=bias_t[:, mt:mt + 1])
                nc.vector.tensor_add(out=xres[mt][:, cs], in0=t1,
                                     in1=xres[mt][:, cs])
                nc.vector.tensor_copy(out=xresb[mt][:, cs], in_=xres[mt][:, cs])

        # ================= phase 1: LN1 over the full sequence =================
        h1p_cm = tc.tile_pool(name="h1p", bufs=1)
        h1p = h1p_cm.__enter__()
        h1t = h1p.tile([P, CT * N], FP8, tag="h1t", name="h1t")
        h1 = layernorm(h1p, xfull, N, "1", dst=h1t)
        h1v = h1t.rearrange("p (kt n) -> p kt n", kt=CT)

        # ============= phase 2: Q/K/V projections (self) =============
        q1_sb = [sa.tile([P, NL], BF16, tag=f"q{t}", name=f"q1_{t}") for t in range(IT)]
        k1_sb = [sa.tile([P, N], BF16, tag=f"k{t}", name=f"k1_{t}") for t in range(IT)]
        with tc.tile_pool(name="psP1", bufs=4, space="PSUM") as psP:
            proj_dr8(psP, wq1d, h1v[:, :, 0:NL].rearrange("p k n -> p k n"),
                     CPAIR, IT, NL,
                     lambda mt, cc, cw, ps: nc.vector.tensor_copy(
                         out=q1_sb[mt][:, cc * cw:(cc + 1) * cw], in_=ps))
            proj_dr8(psP, wk1d, h1v, CPAIR, IT, N,
                     lambda mt, cc, cw, ps: nc.vector.tensor_copy(
                         out=k1_sb[mt][:, cc * cw:(cc + 1) * cw], in_=ps))
            vt1_sb = [make_vt8(psP, sa, h1v, CPAIR, wv1d, jt, f"vt1_{jt}")
                      for jt in range(JT1)]
            # cross-attn K2/V2T depend only on ctx: emit early so the PE work
            # fills self-attention's ACT-bound phase
            k2_sb = [ca.tile([P, MCTX], BF16, tag=f"k{t}", name=f"k2_{t}")
                     for t in range(IT)]
            proj_dr8(psP, wk2d, ctxv, XPAIR, IT, MCTX,
                     lambda mt, cc, cw, ps: nc.vector.tensor_copy(
                         out=k2_sb[mt][:, cc * cw:(cc + 1) * cw], in_=ps))
            vt2_sb = [make_vt8(psP, ca, ctxv, XPAIR, wv2d, jt, f"vt2_{jt}")
                      for jt in range(JT2)]
        if DEBUG:
            for kt in range(CT):
                nc.sync.dma_start(out=nc.dbg["d_h1"][kt * P:(kt + 1) * P, :], in_=h1[kt])
                nc.sync.dma_start(out=nc.dbg["d_q1"][kt * P:(kt + 1) * P, :], in_=q1_sb[kt])
                nc.sync.dma_start(out=nc.dbg["d_k1"][kt * P:(kt + 1) * P, :], in_=k1_sb[kt])
        h1p_cm.__exit__(None, None, None)
        xfp_cm.__exit__(None, None, None)

        # ===== phase 3: self-attention =====
        with tc.tile_pool(name="psS", bufs=2, space="PSUM") as psS, \
             tc.tile_pool(name="psO", bufs=2, space="PSUM") as psO, \
             tc.tile_pool(name="ep", bufs=6) as ep, \
             tc.tile_pool(name="rp", bufs=4) as rp:
            for ic in range(ICN):
                attn_ic(k1_sb, vt1_sb, q1_sb, JT1, attnO, ic, psS, psO,
                        ep, rp, escale=SEXP)
        sa_cm.__exit__(None, None, None)
        wffp_cm = tc.tile_pool(name="wffp", bufs=1, side="right")
        wffp = wffp_cm.__enter__()
        wff1d = wffp.tile([P, CPAIR, 2, 2 * FFI], FP8, tag="wff1t",
                          name="wff1t")
        nc.sync.dma_start(out=wff1d.rearrange("p a b c -> p (a b c)"),
                          in_=w_d["wff1t"][:])
        wff2d = wffp.tile([P, FPAIR, 2, C], FP8, tag="wff2t", name="wff2t")
        nc.sync.dma_start(out=wff2d.rearrange("p a b c -> p (a b c)"),
                          in_=w_d["wff2t"][:])
        ffh_t = wffp.tile([P, 2, NCH], FP8, tag="ffh_t", name="ffh_t")

        # ===== phase 4: Wo1 + residual =====
        with tc.tile_pool(name="psP2", bufs=4, space="PSUM") as psP:
            for ic in range(ICN):
                wo_resid_ic(psP, wo1d, bo1_t, ic, oscale=1.0 / (SW * SW))

        # ===== phase 5: LN2 + Q2 =====
        h2t = ca.tile([P, CT * NL], FP8, tag="h2t", name="h2t")
        h2 = layernorm(ca, xres, NL, "2", xb_src=xresb, dst=h2t)
        h2v = h2t.rearrange("p (kt n) -> p kt n", kt=CT)
        q2_sb = [ca.tile([P, NL], BF16, tag=f"q{t}", name=f"q2_{t}")
                 for t in range(IT)]
        with tc.tile_pool(name="psP3", bufs=4, space="PSUM") as psP:
            proj_dr8(psP, wq2d, h2v, CPAIR, IT, NL,
                     lambda mt, cc, cw, ps: nc.vector.tensor_copy(
                         out=q2_sb[mt][:, cc * cw:(cc + 1) * cw], in_=ps))

        # ===== phase 6: cross-attention =====
        with tc.tile_pool(name="psS2", bufs=2, space="PSUM") as psS, \
             tc.tile_pool(name="psO2", bufs=2, space="PSUM") as psO, \
             tc.tile_pool(name="ep2", bufs=6) as ep, \
             tc.tile_pool(name="rp2", bufs=4) as rp:
            for ic in range(ICN):
                attn_ic(k2_sb, vt2_sb, q2_sb, JT2, attnO, ic, psS, psO,
                        ep, rp, escale=SEXP)

        # ===== phase 7: Wo2 + residual, then LN3 =====
        with tc.tile_pool(name="psP4", bufs=4, space="PSUM") as psP:
            for ic in range(ICN):
                wo_resid_ic(psP, wo2d, bo2_t, ic, oscale=1.0 / (SW * SW))
        h3t = ca.tile([P, CT * NL], FP8, tag="h3t", name="h3t")
        h3 = layernorm(ca, xres, NL, "3", xb_src=xresb, dst=h3t)
        h3p = h3t.rearrange("p (kt n) -> p kt n", kt=CT)

        # ============= phase 8: GEGLU FF =============
        if DEBUG:
            for kt in range(CT):
                nc.sync.dma_start(out=nc.dbg["d_h3"][kt * P:(kt + 1) * P, :], in_=h3[kt])
        with tc.tile_pool(name="psY", bufs=1, space="PSUM") as psY, \
             tc.tile_pool(name="psF", bufs=2, space="PSUM") as psF, \
             tc.tile_pool(name="gp", bufs=3) as gp, \
             tc.tile_pool(name="op", bufs=3) as op:
            for ic in range(ICN):
                cs3 = slice(ic * NCH, (ic + 1) * NCH)
                pys = [psY.tile([P, NCH], F32, tag=f"y{m}", name=f"y{m}")
                       for m in range(CT)]
                for pi in range(FT):
                    ph = psF.tile([P, NCH], F32, tag="ph", name="ph")
                    pg = psF.tile([P, NCH], F32, tag="pg", name="pg")
                    for g in range(CPAIR):
                        nc.tensor.matmul(
                            ph,
                            lhsT=wff1d[:, g, :, pi * P:(pi + 1) * P],
                            rhs=h3p[:, 2 * g:2 * g + 2, cs3],
                            start=(g == 0), stop=(g == CPAIR - 1),
                            perf_mode=DR)
                    for g in range(CPAIR):
                        nc.tensor.matmul(
                            pg,
                            lhsT=wff1d[:, g, :, FFI + pi * P:FFI + (pi + 1) * P],
                            rhs=h3p[:, 2 * g:2 * g + 2, cs3],
                            start=(g == 0), stop=(g == CPAIR - 1),
                            perf_mode=DR)
                    gel = gp.tile([P, NCH], BF16, tag="gel", name="gel")
                    nc.scalar.activation(out=gel, in_=pg, func=AFT.Gelu,
                                         scale=1.0 / SW,
                                         bias=bff1_t[:, FT + pi:FT + pi + 1])
                    hb = tp.tile([P, NCH], F32, tag="hb", name="hb")
                    nc.scalar.activation(out=hb, in_=ph, func=AFT.Identity,
                                         scale=1.0 / SW,
                                         bias=bff1_t[:, pi:pi + 1])
                    nc.vector.tensor_mul(out=ffh_t[:, pi % 2], in0=hb,
                                         in1=gel)
                    if pi % 2 == 1:
                        g2 = pi // 2
                        for mt in range(CT):
                            nc.tensor.matmul(
                                pys[mt],
                                lhsT=wff2d[:, g2, :, mt * P:(mt + 1) * P],
                                rhs=ffh_t,
                                start=(g2 == 0), stop=(g2 == FPAIR - 1),
                                perf_mode=DR)
                for mt in range(CT):
                    t1 = tp.tile([P, NCH], F32, tag="t1", name="t1")
                    nc.scalar.activation(out=t1, in_=pys[mt], func=AFT.Identity,
                                         scale=1.0 / SW,
                                         bias=bff2_t[:, mt:mt + 1])
                    ot = op.tile([P, NCH], F32, tag="ot", name="ot")
                    nc.vector.tensor_add(out=ot, in0=t1,
                                         in1=xres[mt][:, ic * NCH:(ic + 1) * NCH])
                    nc.sync.dma_start(
                        out=out_d[mt * P:(mt + 1) * P, ic * NCH:(ic + 1) * NCH],
                        in_=ot)
        ca_cm.__exit__(None, None, None)
        wffp_cm.__exit__(None, None, None)


def _split_multi_waits(nc):
    """This walrus build accepts at most one sem-wait per instruction; Tile
    emits several. Split extras into standalone InstEventSemaphore pre-waits
    on the same engine (engines execute their stream in order, so semantics
    are preserved)."""
    n = 0
    for fn in nc.m.functions:
        for blk in fn.blocks:
            out = []
            for inst in blk.instructions:
                si = inst.sync_info
                if si is not None and si.on_wait and len(si.on_wait) > 1:
                    waits = list(si.on_wait)
                    for i, w in enumerate(waits[:-1]):
                        out.append(mybir.InstEventSemaphore(
                            name=f"{inst.name}-w{i}",
                            engine=inst.engine,
                            sync_info=mybir.SyncInfo(on_wait=[w], on_update=[]),
                        ))
                        n += 1
                    inst.sync_info = mybir.SyncInfo(
                        on_wait=[waits[-1]], on_update=list(si.on_update))
                out.append(inst)
            blk.instructions = out
    return n


def _build():
    nc = bass.Bass()
    nc.x_d = nc.dram_tensor("x", [C, NL], F32, kind="ExternalInput")
    nc.xb_d = nc.dram_tensor("xb", [C, N], BF16, kind="ExternalInput")
    nc.ctx_d = nc.dram_tensor("ctx", [CTXC, MCTX], FP8, kind="ExternalInput")
    nc.w_d = {}
    for name in ("wq1t", "wk1t", "wv1t", "wq2t"):
        nc.w_d[name] = nc.dram_tensor(name, [P, CPAIR * 2 * INNER], FP8,
                                      kind="ExternalInput")
    for name in ("wo1t", "wo2t"):
        nc.w_d[name] = nc.dram_tensor(name, [P, CPAIR * 2 * C], FP8,
                                      kind="ExternalInput")
    for name in ("wk2t", "wv2t"):
        nc.w_d[name] = nc.dram_tensor(name, [P, XPAIR * 2 * INNER], FP8,
                                      kind="ExternalInput")
    nc.w_d["wff1t"] = nc.dram_tensor("wff1t", [P, CPAIR * 2 * 2 * FFI], FP8,
                                     kind="ExternalInput")
    nc.w_d["wff2t"] = nc.dram_tensor("wff2t", [P, FPAIR * 2 * C], FP8,
                                     kind="ExternalInput")
    nc.b_d = {}
    for name, n in [("bo1", C), ("bo2", C), ("bff1", 2 * FFI), ("bff2", C)]:
        nc.b_d[name] = nc.dram_tensor(name, [n], F32, kind="ExternalInput")
    nc.out_d = nc.dram_tensor("out", [C, NL], F32, kind="ExternalOutput")
    nc.dbg = {}
    if DEBUG:
        for name, shape, dt in [
            ("d_h1", [C, N], BF16), ("d_q1", [C, NL], BF16),
            ("d_k1", [C, N], BF16), ("d_attnO1", [C, NL], BF16),
            ("d_x1", [C, NL], F32), ("d_x2", [C, NL], F32),
            ("d_h3", [C, NL], BF16),
        ]:
            nc.dbg[name] = nc.dram_tensor(name, shape, dt, kind="ExternalOutput")
    with tile.TileContext(nc) as tc:
        _emit(tc)
    _split_multi_waits(nc)
    return nc


_CACHE = {}


def _get_program():
    if "nc" not in _CACHE:
        _CACHE["nc"] = _build()
    return _CACHE["nc"]


def _dr_weight(A, npair):
    """A: [K, M] f32 (already scaled). Returns [128, npair*2*M] fp8 in
    DoubleRow layout: out[p, g, i, m] = A[(2g+i)*128+p, m]."""
    K, M = A.shape
    assert K == npair * 2 * P
    t = A.reshape(npair, 2, P, M).transpose(2, 0, 1, 3)
    return np.ascontiguousarray(t.reshape(P, npair * 2 * M)).astype(E4NP)


def _prep_shared(inputs):
    f32 = np.float32
    g1 = np.asarray(inputs["g1"], f32)
    g2 = np.asarray(inputs["g2"], f32)
    g3 = np.asarray(inputs["g3"], f32)
    scale = DH ** -0.5
    d = {
        "wq1t": _dr_weight(
            (np.asarray(inputs["Wq1"], f32) * scale * g1[None, :]).T * SW,
            CPAIR),
        "wk1t": _dr_weight(
            (np.asarray(inputs["Wk1"], f32) * g1[None, :]).T * SW, CPAIR),
        "wv1t": _dr_weight(
            (np.asarray(inputs["Wv1"], f32) * g1[None, :]).T * SW, CPAIR),
        "wo1t": _dr_weight(np.asarray(inputs["Wo1"], f32).T * SW, CPAIR),
        "wq2t": _dr_weight(
            (np.asarray(inputs["Wq2"], f32) * scale * g2[None, :]).T * SW,
            CPAIR),
        "wk2t": _dr_weight(np.asarray(inputs["Wk2"], f32).T * SW, XPAIR),
        "wv2t": _dr_weight(np.asarray(inputs["Wv2"], f32).T * SW, XPAIR),
        "wo2t": _dr_weight(np.asarray(inputs["Wo2"], f32).T * SW, CPAIR),
        "wff1t": _dr_weight(
            (np.asarray(inputs["Wff1"], f32) * g3[None, :]).T * SW, CPAIR),
        "wff2t": _dr_weight(np.asarray(inputs["Wff2"], f32).T * SW, FPAIR),
        "bo1": np.ascontiguousarray(np.asarray(inputs["bo1"], f32)),
        "bo2": np.ascontiguousarray(np.asarray(inputs["bo2"], f32)),
        "bff1": np.ascontiguousarray(np.asarray(inputs["bff1"], f32)),
        "bff2": np.ascontiguousarray(np.asarray(inputs["bff2"], f32)),
    }
    return d


def make_in_maps(inputs):
    x = np.asarray(inputs["x"], np.float32)
    ctxf = np.asarray(inputs["context"], np.float32)
    shared = _prep_shared(inputs)
    in_maps = []
    for core in range(8):
        b, s = core // 2, core % 2
        xb = x[b]
        if s:
            xc = np.ascontiguousarray(
                np.concatenate([xb[:, NL:], xb[:, :NL]], axis=1))
        else:
            xc = np.ascontiguousarray(xb)
        m = dict(shared)
        m["x"] = np.ascontiguousarray(xc[:, :NL])
        m["xb"] = xc.astype(BF16NP)
        m["ctx"] = np.ascontiguousarray(ctxf[b]).astype(E4NP)
        in_maps.append(m)
    return in_maps


def kernel(**inputs):
    nc = _get_program()
    in_maps = make_in_maps(inputs)
    res = run_bass_kernel_spmd(nc, in_maps, core_ids=list(range(8)))
    out = np.empty((B, C, N), np.float32)
    for core in range(8):
        b, s = core // 2, core % 2
        out[b][:, s * NL:(s + 1) * NL] = res.results[core]["out"]
    return out



# revision 38
# speedup vs baseline: 1.2536x; 1.0561x over previous
"""Trainium2 Bass kernel for a BasicTransformerBlock (self-attn + cross-attn + GEGLU FF).

Sharding: 8 cores = (batch b in 0..3) x (sequence half s in 0..1). No collectives.
Each core receives the full x[b] [512, 2048] (rotated so its local half is always
columns 0..1023), builds self-attention K/V over all 2048 positions, and computes
LN/Q/attention/FF only for its local 1024 positions. Output [512, 1024] per core.

Numerics: bf16 matmuls with fp32 PSUM accumulation; LayerNorm gains folded into the
following weight matrices on the host; attention softmax computed without
max-subtraction (scores are bounded ~+-1.5 here); softmax denominator obtained by
augmenting V^T with a ones-column (row 64 of the AV output = sum_j exp).
"""

import os
import sys

import numpy as np

for _p in ("/opt/trn_rl_repo", "/root/.axon_site/_ro/trn_rl_repo"):
    if os.path.isdir(_p) and _p not in sys.path:
        sys.path.insert(0, _p)

import ml_dtypes

import concourse.bass as bass
import concourse.tile as tile
from concourse import mybir
from concourse.bass_utils import run_bass_kernel_spmd

BF16NP = ml_dtypes.bfloat16
E4NP = ml_dtypes.float8_e4m3
AFT = mybir.ActivationFunctionType
DR = mybir.MatmulPerfMode.DoubleRow
F32 = mybir.dt.float32
BF16 = mybir.dt.bfloat16
FP8 = mybir.dt.float8e4

# Problem dims (hardcoded per spec)
P = 128
B = 4
C = 512      # model dim
N = 2048     # full seq len
NL = 1024    # local seq len per core
CTXC = 768   # context channels
MCTX = 256   # context seq len
H = 8
DH = 64
INNER = 512
FFI = 2048
EPS = 1e-5

CT = C // P        # 4 channel tiles
IT = INNER // P    # 4 inner tiles
XT = CTXC // P     # 6 ctx channel tiles
XPAIR = XT // 2    # 3 ctx channel-tile pairs
FT = FFI // P      # 16 ff tiles
CPAIR = CT // 2    # 2 channel-tile pairs
FPAIR = FT // 2    # 8 ff-tile pairs
SW = 64.0          # fp8 weight pre-scale (FF block + self-attn QKV)
SEXP = 1.0 / (SW * SW)  # self-attn scores psum = 64q * 64k = 4096 * true
NCH = 512          # free-dim chunk size
ICN = NL // NCH    # 2 local i-chunks
JT1 = N // P       # 16 self-attn j tiles
JT2 = MCTX // P    # 2 cross-attn j tiles
DEBUG = False


def _emit(tc):
    nc = tc.nc
    from contextlib import ExitStack

    with ExitStack() as ctx:
        ctx.enter_context(nc.allow_low_precision(
            reason="bf16 rows/broadcasts validated end-to-end vs fp32 reference"))
        main = ctx.enter_context(tc.tile_pool(name="main", bufs=1))
        tp = ctx.enter_context(tc.tile_pool(name="tp", bufs=4))

        x_d = nc.x_d
        ctx_d = nc.ctx_d
        w_d = nc.w_d
        b_d = nc.b_d
        out_d = nc.out_d

        # ---- constants ----
        ones_col = main.tile([P, 1], F32, tag="ones_col", name="ones_col")
        nc.vector.memset(ones_col, 1.0)
        ones_col_bf = main.tile([P, 1], BF16, tag="ones_col_bf", name="ones_col_bf")
        nc.vector.memset(ones_col_bf, 1.0)
        ones_row = main.tile([1, P], BF16, tag="ones_row", name="ones_row")
        nc.vector.memset(ones_row, 1.0)
        eps_t = main.tile([P, 1], F32, tag="eps", name="eps")
        nc.vector.memset(eps_t, EPS)

        # ---- load weights (attention ones up-front; FF weights later) ----
        def load_split(pool, tag, dram, nkt, cols, dtype):
            """One wide DMA for a [nkt*128, cols] DRAM tensor into a single
            [128, nkt*cols] SBUF tile; returns per-kt [128, cols] views."""
            t = pool.tile([P, nkt * cols], dtype, tag=tag, name=tag)
            nc.sync.dma_start(
                out=t.rearrange("p (kt c) -> p kt c", kt=nkt),
                in_=dram.rearrange("(kt p) c -> p kt c", p=P))
            return [t[:, kt * cols:(kt + 1) * cols] for kt in range(nkt)]

        def load_w(pool, name, nkt, cols):
            return load_split(pool, name, w_d[name], nkt, cols, BF16)


        def load_bias(name, n):
            f = n // P
            t = main.tile([P, f], F32, tag=f"b_{name}", name=f"b_{name}")
            nc.sync.dma_start(out=t, in_=b_d[name].rearrange("(f p) -> p f", p=P))
            return t

        ca_cm = tc.tile_pool(name="ca", bufs=1)
        ca = ca_cm.__enter__()
        sa_cm = tc.tile_pool(name="sa", bufs=1)
        sa = sa_cm.__enter__()
        # ---- load activations (before weights: LN1 needs x first) ----
        xfp_cm = tc.tile_pool(name="xfull", bufs=1)
        xfp = xfp_cm.__enter__()
        # xfull: one [128, CT*N] tile, DMA'd in 4 column-chunks so LN1's
        # first chunk starts as soon as its slice lands
        xft = xfp.tile([P, CT * N], BF16, tag="xf", name="xf")
        _xf_nc = N // NCH
        for cc in range(_xf_nc):
            nc.sync.dma_start(
                out=xft.rearrange("p (kt nc c) -> p nc kt c", kt=CT,
                                  nc=_xf_nc)[:, cc],
                in_=nc.xb_d.rearrange("(kt p) (nc c) -> p nc kt c", p=P,
                                      nc=_xf_nc)[:, cc])
        xfull = [xft[:, kt * N:(kt + 1) * N] for kt in range(CT)]
        xres = load_split(main, "xres", x_d, CT, NL, BF16)
        ctx_t = main.tile([P, XT * MCTX], FP8, tag="ctx", name="ctx")
        nc.sync.dma_start(
            out=ctx_t.rearrange("p (kt c) -> p kt c", kt=XT),
            in_=ctx_d.rearrange("(kt p) c -> p kt c", p=P))
        ctxv = ctx_t.rearrange("p (kt m) -> p kt m", kt=XT)

        # biases + weights after activations so LN1's x tiles arrive first
        bo1_t = load_bias("bo1", C)
        bo2_t = load_bias("bo2", C)
        bff1_t = load_bias("bff1", 2 * FFI)
        bff2_t = load_bias("bff2", C)
        def load_w8(name, npair, cols):
            t = main.tile([P, npair, 2, cols], FP8, tag=name, name=name)
            nc.sync.dma_start(out=t.rearrange("p a b c -> p (a b c)"),
                              in_=w_d[name][:])
            return t

        wq1d = load_w8("wq1t", CPAIR, INNER)
        wk1d = load_w8("wk1t", CPAIR, INNER)
        wv1d = load_w8("wv1t", CPAIR, INNER)
        wo1d = load_w8("wo1t", CPAIR, C)
        wq2d = load_w8("wq2t", CPAIR, INNER)
        wk2d = load_w8("wk2t", XPAIR, INNER)
        wv2d = load_w8("wv2t", XPAIR, INNER)
        wo2d = load_w8("wo2t", CPAIR, C)

        attnOt = main.tile([P, IT * NL], FP8, tag="attnOt", name="attnOt")
        attnO = [attnOt[:, t * NL:(t + 1) * NL] for t in range(IT)]
        attnOv = attnOt.rearrange("p (kt n) -> p kt n", kt=IT)


        # ---------- LayerNorm ----------
        def layernorm(hpool, src_tiles, ncols, lnid, xb_src=None, dst=None):
            if dst is not None:
                h_out = [dst[:, kt * ncols:(kt + 1) * ncols]
                         for kt in range(CT)]
            else:
                h_out = []
                for kt in range(CT):
                    h_out.append(hpool.tile([P, ncols], BF16, tag=f"h{kt}",
                                            name=f"h{lnid}_{kt}"))
            with tc.tile_pool(name=f"psLN{lnid}", bufs=2, space="PSUM") as psLN, \
                 tc.tile_pool(name=f"psB{lnid}", bufs=2, space="PSUM") as psB, \
                 tc.tile_pool(name=f"st{lnid}", bufs=1) as st:
                rows = make_ln_rows(st, ncols)
                for cc in range(ncols // NCH):
                    ln_chunk(src_tiles, rows, cc * NCH, lnid, psLN, psB, h_out,
                             cc * NCH, xb_src=xb_src)
            return h_out

        def make_ln_rows(st, ncols):
            mean_row = st.tile([1, ncols], BF16, tag="mrow", name="mrow")
            msq_row = st.tile([1, ncols], F32, tag="qrow", name="qrow")
            var_row = st.tile([1, ncols], BF16, tag="vrow", name="vrow")
            a_row = st.tile([1, ncols], BF16, tag="arow", name="arow")
            return (mean_row, msq_row, var_row, a_row)

        def ln_chunk(src_tiles, rows, col0, lnid, psLN, psB, h_out, hcol0,
                     xb_src=None):
            """LN stats+normalize for one 512-column chunk.

            src cols [col0, col0+NCH) -> h_out cols [hcol0.., ..+NCH)."""
            mean_row, msq_row, var_row, a_row = rows
            src_f32 = src_tiles[0].dtype == F32
            cs = slice(col0, col0 + NCH)
            rs = slice(hcol0, hcol0 + NCH)
            with tc.tile_pool(name=f"x2{lnid}c{col0}", bufs=3) as x2p:
                if xb_src is not None:
                    xb = [s[:, cs] for s in xb_src]
                elif src_f32:
                    xb = []
                    for kt in range(CT):
                        xbt = x2p.tile([P, NCH], BF16, tag="xb", name="xb")
                        nc.vector.tensor_copy(out=xbt, in_=src_tiles[kt][:, cs])
                        xb.append(xbt)
                else:
                    xb = [s[:, cs] for s in src_tiles]
                m_ps = psLN.tile([1, NCH], F32, tag="pp", name="m_ps")
                q_ps = psLN.tile([1, NCH], F32, tag="pp", name="q_ps")
                for kt in range(CT):
                    nc.tensor.matmul(m_ps, lhsT=ones_col_bf, rhs=xb[kt],
                                     start=(kt == 0), stop=(kt == CT - 1))
                for kt in range(CT):
                    x2 = x2p.tile([P, NCH], BF16, tag="x2", name="x2")
                    # gpsimd: both operands SBUF bf16; frees DVE for the
                    # normalize chain (gpsimd is otherwise idle)
                    nc.gpsimd.tensor_mul(out=x2, in0=xb[kt], in1=xb[kt])
                    nc.tensor.matmul(q_ps, lhsT=ones_col_bf, rhs=x2,
                                     start=(kt == 0), stop=(kt == CT - 1))
                nc.vector.tensor_scalar_mul(out=mean_row[0:1, rs], in0=m_ps,
                                            scalar1=1.0 / C)
                nc.vector.tensor_scalar_mul(out=msq_row[0:1, rs], in0=q_ps,
                                            scalar1=1.0 / C)
                nc.vector.tensor_mul(out=var_row[0:1, rs], in0=mean_row[0:1, rs],
                                     in1=mean_row[0:1, rs])
                nc.vector.tensor_sub(out=var_row[0:1, rs], in0=msq_row[0:1, rs],
                                     in1=var_row[0:1, rs])
                nc.scalar.activation(out=var_row[0:1, rs], in_=var_row[0:1, rs],
                                     func=AFT.Sqrt, bias=eps_t[0:1, 0:1])
                nc.vector.reciprocal(out=a_row[0:1, rs], in_=var_row[0:1, rs])
                mb = psB.tile([P, NCH], F32, tag="pp", name="mb")
                ab = psB.tile([P, NCH], F32, tag="pp", name="ab")
                nc.tensor.matmul(mb, lhsT=ones_row, rhs=mean_row[0:1, rs],
                                 start=True, stop=True)
                nc.tensor.matmul(ab, lhsT=ones_row, rhs=a_row[0:1, rs],
                                 start=True, stop=True)
                for kt in range(CT):
                    t1 = tp.tile([P, NCH], F32, tag="t1", name="t1")
                    nc.vector.tensor_sub(out=t1, in0=src_tiles[kt][:, cs], in1=mb)
                    nc.vector.tensor_mul(out=h_out[kt][:, rs], in0=t1, in1=ab)

        # ---------- DR projection helpers (self-attn, fp8 x64) ----------
        def proj_dr8(psP, w_t, h_v, npair, out_mt, ncols, cb):
            cw = min(NCH, ncols)
            for mt in range(out_mt):
                for cc in range(ncols // cw):
                    ps = psP.tile([P, cw], F32, tag="pp", name="pp")
                    for g in range(npair):
                        nc.tensor.matmul(
                            ps, lhsT=w_t[:, g, :, mt * P:(mt + 1) * P],
                            rhs=h_v[:, 2 * g:2 * g + 2, cc * cw:(cc + 1) * cw],
                            start=(g == 0), stop=(g == npair - 1),
                            perf_mode=DR)
                    cb(mt, cc, cw, ps)

        def make_vt8(psP, pool, h_v, npair, w_t, jt, name):
            ps = psP.tile([P, INNER], F32, tag="pp", name="pp")
            for g in range(npair):
                nc.tensor.matmul(
                    ps, lhsT=h_v[:, 2 * g:2 * g + 2, jt * P:(jt + 1) * P],
                    rhs=w_t[:, g], start=(g == 0), stop=(g == npair - 1),
                    perf_mode=DR)
            vt = pool.tile([P, H, DH + 1], BF16, tag=f"vt{jt}", name=name)
            if jt % 2 == 0:
                nc.vector.tensor_copy(
                    out=vt[:, :, 0:DH],
                    in_=ps.rearrange("p (h d) -> p h d", h=H))
            else:
                nc.scalar.copy(
                    out=vt[:, :, 0:DH],
                    in_=ps.rearrange("p (h d) -> p h d", h=H))
            nc.vector.memset(vt[:, :, DH:DH + 1], 1.0)
            return vt

        # ---------- projection helper ----------
        def proj(psP, w_tiles, rhs_tiles, nkt, out_mt, ncols, cb):
            cw = min(NCH, ncols)
            for mt in range(out_mt):
                for cc in range(ncols // cw):
                    ps = psP.tile([P, cw], F32, tag="pp", name="pp")
                    for kt in range(nkt):
                        nc.tensor.matmul(
                            ps,
                            lhsT=w_tiles[kt][:, mt * P:(mt + 1) * P],
                            rhs=rhs_tiles[kt][:, cc * cw:(cc + 1) * cw],
                            start=(kt == 0), stop=(kt == nkt - 1))
                    cb(mt, cc, cw, ps)

        def make_vt(psP, pool, lhs_tiles, nkt, w_tiles, jt, name):
            ps = psP.tile([P, INNER], F32, tag="pp", name="pp")
            for kt in range(nkt):
                nc.tensor.matmul(
                    ps,
                    lhsT=lhs_tiles[kt][:, jt * P:(jt + 1) * P],
                    rhs=w_tiles[kt],
                    start=(kt == 0), stop=(kt == nkt - 1))
            vt = pool.tile([P, H, DH + 1], BF16, tag=f"vt{jt}", name=name)
            nc.vector.tensor_copy(
                out=vt[:, :, 0:DH],
                in_=ps.rearrange("p (h d) -> p h d", h=H))
            nc.vector.memset(vt[:, :, DH:DH + 1], 1.0)
            return vt

        # ---------- attention ----------
        def attn_ic(k_sb, vt_sb, q_sb, njt, dst, ic, psS, psO, ep, rp,
                    escale=1.0):
            for hp in range(H // 2):
                t = hp
                po = [psO.tile([P, NCH], F32, tag=f"po{i}", name=f"po{i}")
                      for i in range(2)]
                for jt in range(njt):
                    ps = psS.tile([P, 2 * NCH], F32, tag="ps", name="ps")
                    for hh in range(2):
                        nc.tensor.matmul(
                            ps[:, hh * NCH:(hh + 1) * NCH],
                            lhsT=k_sb[t][hh * DH:(hh + 1) * DH, jt * P:(jt + 1) * P],
                            rhs=q_sb[t][hh * DH:(hh + 1) * DH, ic * NCH:(ic + 1) * NCH],
                            start=True, stop=True)
                    e = ep.tile([P, 2 * NCH], BF16, tag="e", name="e")
                    nc.scalar.activation(out=e, in_=ps, func=AFT.Exp,
                                         scale=escale)
                    for hh in range(2):
                        h = 2 * hp + hh
                        nc.tensor.matmul(
                            po[hh][0:DH + 1, :],
                            lhsT=vt_sb[jt][:, h, :],
                            rhs=e[:, hh * NCH:(hh + 1) * NCH],
                            start=(jt == 0), stop=(jt == njt - 1))
                for hh in range(2):
                    rrow = rp.tile([1, NCH], BF16, tag="rrow", name="rrow")
                    nc.vector.reciprocal(out=rrow, in_=po[hh][DH:DH + 1, :])
                    # broadcast 1/denom into po's unused partitions 64..127
                    nc.tensor.matmul(po[hh][DH:2 * DH, :],
                                     lhsT=ones_row[0:1, 0:DH], rhs=rrow,
                                     start=True, stop=True)
                    un = rp.tile([DH, NCH], BF16, tag="un", name="un")
                    nc.vector.tensor_copy(out=un, in_=po[hh][0:DH, :])
                    nc.vector.tensor_mul(
                        out=dst[t][hh * DH:(hh + 1) * DH, ic * NCH:(ic + 1) * NCH],
                        in0=un, in1=po[hh][DH:2 * DH, :])

        # ---------- output-proj + residual (one ic chunk) ----------
        def wo_resid_ic(psP, wo_t, bias_t, ic, oscale=1.0):
            cs = slice(ic * NCH, (ic + 1) * NCH)
            for mt in range(CT):
                ps = psP.tile([P, NCH], F32, tag="pp", name="pp")
                for g in range(CPAIR):
                    nc.tensor.matmul(ps, lhsT=wo_t[:, g, :, mt * P:(mt + 1) * P],
                                     rhs=attnOv[:, 2 * g:2 * g + 2, cs],
                                     start=(g == 0), stop=(g == CPAIR - 1),
                                     perf_mode=DR)
                t1 = tp.tile([P, NCH], F32, tag="t1", name="t1")
                nc.scalar.activation(out=t1, in_=ps, func=AFT.Identity,
                                     scale=oscale, bias=bias_t[:, mt:mt + 1])
                nc.vector.tensor_add(out=xres[mt][:, cs], in0=t1,
                                     in1=xres[mt][:, cs])

        # ================= phase 1: LN1 over the full sequence =================
        h1p_cm = tc.tile_pool(name="h1p", bufs=1)
        h1p = h1p_cm.__enter__()
        h1t = h1p.tile([P, CT * N], FP8, tag="h1t", name="h1t")
        h1 = layernorm(h1p, xfull, N, "1", dst=h1t)
        h1v = h1t.rearrange("p (kt n) -> p kt n", kt=CT)

        # ============= phase 2: Q/K/V projections (self) =============
        q1_sb = [sa.tile([P, NL], BF16, tag=f"q{t}", name=f"q1_{t}") for t in range(IT)]
        k1_sb = [sa.tile([P, N], BF16, tag=f"k{t}", name=f"k1_{t}") for t in range(IT)]
        with tc.tile_pool(name="psP1", bufs=4, space="PSUM") as psP:
            proj_dr8(psP, wq1d, h1v[:, :, 0:NL].rearrange("p k n -> p k n"),
                     CPAIR, IT, NL,
                     lambda mt, cc, cw, ps: nc.vector.tensor_copy(
                         out=q1_sb[mt][:, cc * cw:(cc + 1) * cw], in_=ps))
            def _k1cb(mt, cc, cw, ps):
                if cc % 2 == 0:
                    nc.vector.tensor_copy(
                        out=k1_sb[mt][:, cc * cw:(cc + 1) * cw], in_=ps)
                else:
                    nc.scalar.copy(
                        out=k1_sb[mt][:, cc * cw:(cc + 1) * cw], in_=ps)
            proj_dr8(psP, wk1d, h1v, CPAIR, IT, N, _k1cb)
            vt1_sb = [make_vt8(psP, sa, h1v, CPAIR, wv1d, jt, f"vt1_{jt}")
                      for jt in range(JT1)]
            # cross-attn K2/V2T depend only on ctx: emit early so the PE work
            # fills self-attention's ACT-bound phase
            k2_sb = [ca.tile([P, MCTX], BF16, tag=f"k{t}", name=f"k2_{t}")
                     for t in range(IT)]
            proj_dr8(psP, wk2d, ctxv, XPAIR, IT, MCTX,
                     lambda mt, cc, cw, ps: nc.vector.tensor_copy(
                         out=k2_sb[mt][:, cc * cw:(cc + 1) * cw], in_=ps))
            vt2_sb = [make_vt8(psP, ca, ctxv, XPAIR, wv2d, jt, f"vt2_{jt}")
                      for jt in range(JT2)]
        if DEBUG:
            for kt in range(CT):
                nc.sync.dma_start(out=nc.dbg["d_h1"][kt * P:(kt + 1) * P, :], in_=h1[kt])
                nc.sync.dma_start(out=nc.dbg["d_q1"][kt * P:(kt + 1) * P, :], in_=q1_sb[kt])
                nc.sync.dma_start(out=nc.dbg["d_k1"][kt * P:(kt + 1) * P, :], in_=k1_sb[kt])
        h1p_cm.__exit__(None, None, None)
        xfp_cm.__exit__(None, None, None)

        # ===== phase 3: self-attention =====
        with tc.tile_pool(name="psS", bufs=2, space="PSUM") as psS, \
             tc.tile_pool(name="psO", bufs=2, space="PSUM") as psO, \
             tc.tile_pool(name="ep", bufs=6) as ep, \
             tc.tile_pool(name="rp", bufs=4) as rp:
            for ic in range(ICN):
                attn_ic(k1_sb, vt1_sb, q1_sb, JT1, attnO, ic, psS, psO,
                        ep, rp, escale=SEXP)
        sa_cm.__exit__(None, None, None)
        wffp_cm = tc.tile_pool(name="wffp", bufs=1, side="right")
        wffp = wffp_cm.__enter__()
        wff1d = wffp.tile([P, CPAIR, 2, 2 * FFI], FP8, tag="wff1t",
                          name="wff1t")
        nc.sync.dma_start(out=wff1d.rearrange("p a b c -> p (a b c)"),
                          in_=w_d["wff1t"][:])
        wff2d = wffp.tile([P, FPAIR, 2, C], FP8, tag="wff2t", name="wff2t")
        nc.sync.dma_start(out=wff2d.rearrange("p a b c -> p (a b c)"),
                          in_=w_d["wff2t"][:])
        ffh_t = wffp.tile([P, 2, NCH], FP8, tag="ffh_t", name="ffh_t")

        # ===== phase 4: Wo1 + residual =====
        with tc.tile_pool(name="psP2", bufs=4, space="PSUM") as psP:
            for ic in range(ICN):
                wo_resid_ic(psP, wo1d, bo1_t, ic, oscale=1.0 / (SW * SW))

        # ===== phase 5: LN2 + Q2 =====
        h2t = ca.tile([P, CT * NL], FP8, tag="h2t", name="h2t")
        h2 = layernorm(ca, xres, NL, "2", xb_src=xres, dst=h2t)
        h2v = h2t.rearrange("p (kt n) -> p kt n", kt=CT)
        q2_sb = [ca.tile([P, NL], BF16, tag=f"q{t}", name=f"q2_{t}")
                 for t in range(IT)]
        with tc.tile_pool(name="psP3", bufs=4, space="PSUM") as psP:
            proj_dr8(psP, wq2d, h2v, CPAIR, IT, NL,
                     lambda mt, cc, cw, ps: nc.vector.tensor_copy(
                         out=q2_sb[mt][:, cc * cw:(cc + 1) * cw], in_=ps))

        # ===== phase 6: cross-attention =====
        with tc.tile_pool(name="psS2", bufs=2, space="PSUM") as psS, \
             tc.tile_pool(name="psO2", bufs=2, space="PSUM") as psO, \
             tc.tile_pool(name="ep2", bufs=6) as ep, \
             tc.tile_pool(name="rp2", bufs=4) as rp:
            for ic in range(ICN):
                attn_ic(k2_sb, vt2_sb, q2_sb, JT2, attnO, ic, psS, psO,
                        ep, rp, escale=SEXP)

        # ===== phase 7: Wo2 + residual, then LN3 =====
        with tc.tile_pool(name="psP4", bufs=4, space="PSUM") as psP:
            for ic in range(ICN):
                wo_resid_ic(psP, wo2d, bo2_t, ic, oscale=1.0 / (SW * SW))
        h3t = ca.tile([P, CT * NL], FP8, tag="h3t", name="h3t")
        h3 = layernorm(ca, xres, NL, "3", xb_src=xres, dst=h3t)
        h3p = h3t.rearrange("p (kt n) -> p kt n", kt=CT)

        # ============= phase 8: GEGLU FF =============
        if DEBUG:
            for kt in range(CT):
                nc.sync.dma_start(out=nc.dbg["d_h3"][kt * P:(kt + 1) * P, :], in_=h3[kt])
        with tc.tile_pool(name="psY", bufs=1, space="PSUM") as psY, \
             tc.tile_pool(name="psF", bufs=2, space="PSUM") as psF, \
             tc.tile_pool(name="gp", bufs=3) as gp, \
             tc.tile_pool(name="op", bufs=3) as op:
            for ic in range(ICN):
                cs3 = slice(ic * NCH, (ic + 1) * NCH)
                pys = [psY.tile([P, NCH], F32, tag=f"y{m}", name=f"y{m}")
                       for m in range(CT)]
                for pi in range(FT):
                    ph = psF.tile([P, NCH], F32, tag="ph", name="ph")
                    pg = psF.tile([P, NCH], F32, tag="pg", name="pg")
                    for g in range(CPAIR):
                        nc.tensor.matmul(
                            ph,
                            lhsT=wff1d[:, g, :, pi * P:(pi + 1) * P],
                            rhs=h3p[:, 2 * g:2 * g + 2, cs3],
                            start=(g == 0), stop=(g == CPAIR - 1),
                            perf_mode=DR)
                    for g in range(CPAIR):
                        nc.tensor.matmul(
                            pg,
                            lhsT=wff1d[:, g, :, FFI + pi * P:FFI + (pi + 1) * P],
                            rhs=h3p[:, 2 * g:2 * g + 2, cs3],
                            start=(g == 0), stop=(g == CPAIR - 1),
                            perf_mode=DR)
                    gel = gp.tile([P, NCH], BF16, tag="gel", name="gel")
                    nc.scalar.activation(out=gel, in_=pg, func=AFT.Gelu,
                                         scale=1.0 / SW,
                                         bias=bff1_t[:, FT + pi:FT + pi + 1])
                    hb = tp.tile([P, NCH], F32, tag="hb", name="hb")
                    nc.scalar.activation(out=hb, in_=ph, func=AFT.Identity,
                                         scale=1.0 / SW,
                                         bias=bff1_t[:, pi:pi + 1])
                    nc.vector.tensor_mul(out=ffh_t[:, pi % 2], in0=hb,
                                         in1=gel)
                    if pi % 2 == 1:
                        g2 = pi // 2
                        for mt in range(CT):
                            nc.tensor.matmul(
                                pys[mt],
                                lhsT=wff2d[:, g2, :, mt * P:(mt + 1) * P],
                                rhs=ffh_t,
                                start=(g2 == 0), stop=(g2 == FPAIR - 1),
                                perf_mode=DR)
                for mt in range(CT):
                    t1 = tp.tile([P, NCH], F32, tag="t1", name="t1")
                    nc.scalar.activation(out=t1, in_=pys[mt], func=AFT.Identity,
                                         scale=1.0 / SW,
                                         bias=bff2_t[:, mt:mt + 1])
                    ot = op.tile([P, NCH], F32, tag="ot", name="ot")
                    nc.vector.tensor_add(out=ot, in0=t1,
                                         in1=xres[mt][:, ic * NCH:(ic + 1) * NCH])
                    nc.sync.dma_start(
                        out=out_d[mt * P:(mt + 1) * P, ic * NCH:(ic + 1) * NCH],
                        in_=ot)
        ca_cm.__exit__(None, None, None)
        wffp_cm.__exit__(None, None, None)


def _split_multi_waits(nc):
    """This walrus build accepts at most one sem-wait per instruction; Tile
    emits several. Split extras into standalone InstEventSemaphore pre-waits
    on the same engine (engines execute their stream in order, so semantics
    are preserved)."""
    n = 0
    for fn in nc.m.functions:
        for blk in fn.blocks:
            out = []
            for inst in blk.instructions:
                si = inst.sync_info
                if si is not None and si.on_wait and len(si.on_wait) > 1:
                    waits = list(si.on_wait)
                    for i, w in enumerate(waits[:-1]):
                        out.append(mybir.InstEventSemaphore(
                            name=f"{inst.name}-w{i}",
                            engine=inst.engine,
                            sync_info=mybir.SyncInfo(on_wait=[w], on_update=[]),
                        ))
                        n += 1
                    inst.sync_info = mybir.SyncInfo(
                        on_wait=[waits[-1]], on_update=list(si.on_update))
                out.append(inst)
            blk.instructions = out
    return n


def _build():
    nc = bass.Bass()
    nc.x_d = nc.dram_tensor("x", [C, NL], BF16, kind="ExternalInput")
    nc.xb_d = nc.dram_tensor("xb", [C, N], BF16, kind="ExternalInput")
    nc.ctx_d = nc.dram_tensor("ctx", [CTXC, MCTX], FP8, kind="ExternalInput")
    nc.w_d = {}
    for name in ("wq1t", "wk1t", "wv1t", "wq2t"):
        nc.w_d[name] = nc.dram_tensor(name, [P, CPAIR * 2 * INNER], FP8,
                                      kind="ExternalInput")
    for name in ("wo1t", "wo2t"):
        nc.w_d[name] = nc.dram_tensor(name, [P, CPAIR * 2 * C], FP8,
                                      kind="ExternalInput")
    for name in ("wk2t", "wv2t"):
        nc.w_d[name] = nc.dram_tensor(name, [P, XPAIR * 2 * INNER], FP8,
                                      kind="ExternalInput")
    nc.w_d["wff1t"] = nc.dram_tensor("wff1t", [P, CPAIR * 2 * 2 * FFI], FP8,
                                     kind="ExternalInput")
    nc.w_d["wff2t"] = nc.dram_tensor("wff2t", [P, FPAIR * 2 * C], FP8,
                                     kind="ExternalInput")
    nc.b_d = {}
    for name, n in [("bo1", C), ("bo2", C), ("bff1", 2 * FFI), ("bff2", C)]:
        nc.b_d[name] = nc.dram_tensor(name, [n], F32, kind="ExternalInput")
    nc.out_d = nc.dram_tensor("out", [C, NL], F32, kind="ExternalOutput")
    nc.dbg = {}
    if DEBUG:
        for name, shape, dt in [
            ("d_h1", [C, N], BF16), ("d_q1", [C, NL], BF16),
            ("d_k1", [C, N], BF16), ("d_attnO1", [C, NL], BF16),
            ("d_x1", [C, NL], F32), ("d_x2", [C, NL], F32),
            ("d_h3", [C, NL], BF16),
        ]:
            nc.dbg[name] = nc.dram_tensor(name, shape, dt, kind="ExternalOutput")
    with tile.TileContext(nc) as tc:
        _emit(tc)
    _split_multi_waits(nc)
    return nc


_CACHE = {}


def _get_program():
    if "nc" not in _CACHE:
        _CACHE["nc"] = _build()
    return _CACHE["nc"]


def _dr_weight(A, npair):
    """A: [K, M] f32 (already scaled). Returns [128, npair*2*M] fp8 in
    DoubleRow layout: out[p, g, i, m] = A[(2g+i)*128+p, m]."""
    K, M = A.shape
    assert K == npair * 2 * P
    t = A.reshape(npair, 2, P, M).transpose(2, 0, 1, 3)
    return np.ascontiguousarray(t.reshape(P, npair * 2 * M)).astype(E4NP)


def _prep_shared(inputs):
    f32 = np.float32
    g1 = np.asarray(inputs["g1"], f32)
    g2 = np.asarray(inputs["g2"], f32)
    g3 = np.asarray(inputs["g3"], f32)
    scale = DH ** -0.5
    d = {
        "wq1t": _dr_weight(
            (np.asarray(inputs["Wq1"], f32) * scale * g1[None, :]).T * SW,
            CPAIR),
        "wk1t": _dr_weight(
            (np.asarray(inputs["Wk1"], f32) * g1[None, :]).T * SW, CPAIR),
        "wv1t": _dr_weight(
            (np.asarray(inputs["Wv1"], f32) * g1[None, :]).T * SW, CPAIR),
        "wo1t": _dr_weight(np.asarray(inputs["Wo1"], f32).T * SW, CPAIR),
        "wq2t": _dr_weight(
            (np.asarray(inputs["Wq2"], f32) * scale * g2[None, :]).T * SW,
            CPAIR),
        "wk2t": _dr_weight(np.asarray(inputs["Wk2"], f32).T * SW, XPAIR),
        "wv2t": _dr_weight(np.asarray(inputs["Wv2"], f32).T * SW, XPAIR),
        "wo2t": _dr_weight(np.asarray(inputs["Wo2"], f32).T * SW, CPAIR),
        "wff1t": _dr_weight(
            (np.asarray(inputs["Wff1"], f32) * g3[None, :]).T * SW, CPAIR),
        "wff2t": _dr_weight(np.asarray(inputs["Wff2"], f32).T * SW, FPAIR),
        "bo1": np.ascontiguousarray(np.asarray(inputs["bo1"], f32)),
        "bo2": np.ascontiguousarray(np.asarray(inputs["bo2"], f32)),
        "bff1": np.ascontiguousarray(np.asarray(inputs["bff1"], f32)),
        "bff2": np.ascontiguousarray(np.asarray(inputs["bff2"], f32)),
    }
    return d


def make_in_maps(inputs):
    x = np.asarray(inputs["x"], np.float32)
    ctxf = np.asarray(inputs["context"], np.float32)
    shared = _prep_shared(inputs)
    in_maps = []
    for core in range(8):
        b, s = core // 2, core % 2
        xb = x[b]
        if s:
            xc = np.ascontiguousarray(
                np.concatenate([xb[:, NL:], xb[:, :NL]], axis=1))
        else:
            xc = np.ascontiguousarray(xb)
        m = dict(shared)
        m["x"] = np.ascontiguousarray(xc[:, :NL]).astype(BF16NP)
        m["xb"] = xc.astype(BF16NP)
        m["ctx"] = np.ascontiguousarray(ctxf[b]).astype(E4NP)
        in_maps.append(m)
    return in_maps


def kernel(**inputs):
    nc = _get_program()
    in_maps = make_in_maps(inputs)
    res = run_bass_kernel_spmd(nc, in_maps, core_ids=list(range(8)))
    out = np.empty((B, C, N), np.float32)
    for core in range(8):
        b, s = core // 2, core % 2
        out[b][:, s * NL:(s + 1) * NL] = res.results[core]["out"]
    return out



# revision 40
# speedup vs baseline: 1.2721x; 1.0148x over previous
"""Trainium2 Bass kernel for a BasicTransformerBlock (self-attn + cross-attn + GEGLU FF).

Sharding: 8 cores = (batch b in 0..3) x (sequence half s in 0..1). No collectives.
Each core receives the full x[b] [512, 2048] (rotated so its local half is always
columns 0..1023), builds self-attention K/V over all 2048 positions, and computes
LN/Q/attention/FF only for its local 1024 positions. Output [512, 1024] per core.

Numerics: bf16 matmuls with fp32 PSUM accumulation; LayerNorm gains folded into the
following weight matrices on the host; attention softmax computed without
max-subtraction (scores are bounded ~+-1.5 here); softmax denominator obtained by
augmenting V^T with a ones-column (row 64 of the AV output = sum_j exp).
"""

import os
import sys

import numpy as np

for _p in ("/opt/trn_rl_repo", "/root/.axon_site/_ro/trn_rl_repo"):
    if os.path.isdir(_p) and _p not in sys.path:
        sys.path.insert(0, _p)

import ml_dtypes

import concourse.bass as bass
import concourse.tile as tile
from concourse import mybir
from concourse.bass_utils import run_bass_kernel_spmd

BF16NP = ml_dtypes.bfloat16
E4NP = ml_dtypes.float8_e4m3
AFT = mybir.ActivationFunctionType
DR = mybir.MatmulPerfMode.DoubleRow
F32 = mybir.dt.float32
BF16 = mybir.dt.bfloat16
FP8 = mybir.dt.float8e4

# Problem dims (hardcoded per spec)
P = 128
B = 4
C = 512      # model dim
N = 2048     # full seq len
NL = 1024    # local seq len per core
CTXC = 768   # context channels
MCTX = 256   # context seq len
H = 8
DH = 64
INNER = 512
FFI = 2048
EPS = 1e-5

CT = C // P        # 4 channel tiles
IT = INNER // P    # 4 inner tiles
XT = CTXC // P     # 6 ctx channel tiles
XPAIR = XT // 2    # 3 ctx channel-tile pairs
FT = FFI // P      # 16 ff tiles
CPAIR = CT // 2    # 2 channel-tile pairs
FPAIR = FT // 2    # 8 ff-tile pairs
SW = 64.0          # fp8 weight pre-scale (FF block + self-attn QKV)
SEXP = 1.0 / (SW * SW)  # self-attn scores psum = 64q * 64k = 4096 * true
NCH = 512          # free-dim chunk size
ICN = NL // NCH    # 2 local i-chunks
JT1 = N // P       # 16 self-attn j tiles
JT2 = MCTX // P    # 2 cross-attn j tiles
DEBUG = False


def _emit(tc):
    nc = tc.nc
    from contextlib import ExitStack

    with ExitStack() as ctx:
        ctx.enter_context(nc.allow_low_precision(
            reason="bf16 rows/broadcasts validated end-to-end vs fp32 reference"))
        main = ctx.enter_context(tc.tile_pool(name="main", bufs=1))
        tp = ctx.enter_context(tc.tile_pool(name="tp", bufs=4))

        x_d = nc.x_d
        ctx_d = nc.ctx_d
        w_d = nc.w_d
        b_d = nc.b_d
        out_d = nc.out_d

        # ---- constants ----
        ones_col = main.tile([P, 1], F32, tag="ones_col", name="ones_col")
        nc.vector.memset(ones_col, 1.0)
        ones_col_bf = main.tile([P, 1], BF16, tag="ones_col_bf", name="ones_col_bf")
        nc.vector.memset(ones_col_bf, 1.0)
        ones_row = main.tile([1, P], BF16, tag="ones_row", name="ones_row")
        nc.vector.memset(ones_row, 1.0)
        eps_t = main.tile([P, 1], F32, tag="eps", name="eps")
        nc.vector.memset(eps_t, EPS)

        # ---- load weights (attention ones up-front; FF weights later) ----
        def load_split(pool, tag, dram, nkt, cols, dtype):
            """One wide DMA for a [nkt*128, cols] DRAM tensor into a single
            [128, nkt*cols] SBUF tile; returns per-kt [128, cols] views."""
            t = pool.tile([P, nkt * cols], dtype, tag=tag, name=tag)
            nc.sync.dma_start(
                out=t.rearrange("p (kt c) -> p kt c", kt=nkt),
                in_=dram.rearrange("(kt p) c -> p kt c", p=P))
            return [t[:, kt * cols:(kt + 1) * cols] for kt in range(nkt)]

        def load_w(pool, name, nkt, cols):
            return load_split(pool, name, w_d[name], nkt, cols, BF16)


        def load_bias(name, n):
            f = n // P
            t = main.tile([P, f], F32, tag=f"b_{name}", name=f"b_{name}")
            nc.sync.dma_start(out=t, in_=b_d[name].rearrange("(f p) -> p f", p=P))
            return t

        ca_cm = tc.tile_pool(name="ca", bufs=1)
        ca = ca_cm.__enter__()
        sa_cm = tc.tile_pool(name="sa", bufs=1)
        sa = sa_cm.__enter__()
        # ---- load activations (before weights: LN1 needs x first) ----
        xfp_cm = tc.tile_pool(name="xfull", bufs=1)
        xfp = xfp_cm.__enter__()
        # xfull: one [128, CT*N] tile, DMA'd in 4 column-chunks so LN1's
        # first chunk starts as soon as its slice lands
        xft = xfp.tile([P, CT * N], BF16, tag="xf", name="xf")
        _xf_nc = N // NCH
        for cc in range(_xf_nc):
            nc.sync.dma_start(
                out=xft.rearrange("p (kt nc c) -> p nc kt c", kt=CT,
                                  nc=_xf_nc)[:, cc],
                in_=nc.xb_d.rearrange("(kt p) (nc c) -> p nc kt c", p=P,
                                      nc=_xf_nc)[:, cc])
        xfull = [xft[:, kt * N:(kt + 1) * N] for kt in range(CT)]
        xres = load_split(main, "xres", x_d, CT, NL, BF16)
        ctx_t = main.tile([P, XT * MCTX], FP8, tag="ctx", name="ctx")
        nc.sync.dma_start(
            out=ctx_t.rearrange("p (kt c) -> p kt c", kt=XT),
            in_=ctx_d.rearrange("(kt p) c -> p kt c", p=P))
        ctxv = ctx_t.rearrange("p (kt m) -> p kt m", kt=XT)

        # biases + weights after activations so LN1's x tiles arrive first
        bo1_t = load_bias("bo1", C)
        bo2_t = load_bias("bo2", C)
        bff1_t = load_bias("bff1", 2 * FFI)
        bff2_t = load_bias("bff2", C)
        def load_w8(name, npair, cols):
            t = main.tile([P, npair, 2, cols], FP8, tag=name, name=name)
            nc.sync.dma_start(out=t.rearrange("p a b c -> p (a b c)"),
                              in_=w_d[name][:])
            return t

        wq1d = load_w8("wq1t", CPAIR, INNER)
        wk1d = load_w8("wk1t", CPAIR, INNER)
        wv1d = load_w8("wv1t", CPAIR, INNER)
        wo1d = load_w8("wo1t", CPAIR, C)
        wq2d = load_w8("wq2t", CPAIR, INNER)
        wk2d = load_w8("wk2t", XPAIR, INNER)
        wv2d = load_w8("wv2t", XPAIR, INNER)
        wo2d = load_w8("wo2t", CPAIR, C)

        attnOt = main.tile([P, IT * NL], FP8, tag="attnOt", name="attnOt")
        attnO = [attnOt[:, t * NL:(t + 1) * NL] for t in range(IT)]
        attnOv = attnOt.rearrange("p (kt n) -> p kt n", kt=IT)


        # ---------- LayerNorm ----------
        def layernorm(hpool, src_tiles, ncols, lnid, xb_src=None, dst=None):
            if dst is not None:
                h_out = [dst[:, kt * ncols:(kt + 1) * ncols]
                         for kt in range(CT)]
            else:
                h_out = []
                for kt in range(CT):
                    h_out.append(hpool.tile([P, ncols], BF16, tag=f"h{kt}",
                                            name=f"h{lnid}_{kt}"))
            with tc.tile_pool(name=f"psLN{lnid}", bufs=2, space="PSUM") as psLN, \
                 tc.tile_pool(name=f"psB{lnid}", bufs=2, space="PSUM") as psB, \
                 tc.tile_pool(name=f"st{lnid}", bufs=1) as st:
                rows = make_ln_rows(st, ncols)
                for cc in range(ncols // NCH):
                    ln_chunk(src_tiles, rows, cc * NCH, lnid, psLN, psB, h_out,
                             cc * NCH, xb_src=xb_src)
            return h_out

        def make_ln_rows(st, ncols):
            mean_row = st.tile([1, ncols], BF16, tag="mrow", name="mrow")
            msq_row = st.tile([1, ncols], F32, tag="qrow", name="qrow")
            var_row = st.tile([1, ncols], BF16, tag="vrow", name="vrow")
            a_row = st.tile([1, ncols], BF16, tag="arow", name="arow")
            return (mean_row, msq_row, var_row, a_row)

        def ln_chunk(src_tiles, rows, col0, lnid, psLN, psB, h_out, hcol0,
                     xb_src=None):
            """LN stats+normalize for one 512-column chunk.

            src cols [col0, col0+NCH) -> h_out cols [hcol0.., ..+NCH)."""
            mean_row, msq_row, var_row, a_row = rows
            src_f32 = src_tiles[0].dtype == F32
            cs = slice(col0, col0 + NCH)
            rs = slice(hcol0, hcol0 + NCH)
            with tc.tile_pool(name=f"x2{lnid}c{col0}", bufs=3) as x2p:
                if xb_src is not None:
                    xb = [s[:, cs] for s in xb_src]
                elif src_f32:
                    xb = []
                    for kt in range(CT):
                        xbt = x2p.tile([P, NCH], BF16, tag="xb", name="xb")
                        nc.vector.tensor_copy(out=xbt, in_=src_tiles[kt][:, cs])
                        xb.append(xbt)
                else:
                    xb = [s[:, cs] for s in src_tiles]
                m_ps = psLN.tile([1, NCH], F32, tag="pp", name="m_ps")
                q_ps = psLN.tile([1, NCH], F32, tag="pp", name="q_ps")
                for kt in range(CT):
                    nc.tensor.matmul(m_ps, lhsT=ones_col_bf, rhs=xb[kt],
                                     start=(kt == 0), stop=(kt == CT - 1))
                for kt in range(CT):
                    x2 = x2p.tile([P, NCH], BF16, tag="x2", name="x2")
                    # gpsimd: both operands SBUF bf16; frees DVE for the
                    # normalize chain (gpsimd is otherwise idle)
                    nc.gpsimd.tensor_mul(out=x2, in0=xb[kt], in1=xb[kt])
                    nc.tensor.matmul(q_ps, lhsT=ones_col_bf, rhs=x2,
                                     start=(kt == 0), stop=(kt == CT - 1))
                nc.vector.tensor_scalar_mul(out=mean_row[0:1, rs], in0=m_ps,
                                            scalar1=1.0 / C)
                nc.vector.tensor_scalar_mul(out=msq_row[0:1, rs], in0=q_ps,
                                            scalar1=1.0 / C)
                nc.vector.tensor_mul(out=var_row[0:1, rs], in0=mean_row[0:1, rs],
                                     in1=mean_row[0:1, rs])
                nc.vector.tensor_sub(out=var_row[0:1, rs], in0=msq_row[0:1, rs],
                                     in1=var_row[0:1, rs])
                nc.scalar.activation(out=var_row[0:1, rs], in_=var_row[0:1, rs],
                                     func=AFT.Sqrt, bias=eps_t[0:1, 0:1])
                nc.vector.reciprocal(out=a_row[0:1, rs], in_=var_row[0:1, rs])
                mb = psB.tile([P, NCH], F32, tag="pp", name="mb")
                ab = psB.tile([P, NCH], F32, tag="pp", name="ab")
                nc.tensor.matmul(mb, lhsT=ones_row, rhs=mean_row[0:1, rs],
                                 start=True, stop=True)
                nc.tensor.matmul(ab, lhsT=ones_row, rhs=a_row[0:1, rs],
                                 start=True, stop=True)
                for kt in range(CT):
                    t1 = tp.tile([P, NCH], F32, tag="t1", name="t1")
                    nc.vector.tensor_sub(out=t1, in0=src_tiles[kt][:, cs], in1=mb)
                    nc.vector.tensor_mul(out=h_out[kt][:, rs], in0=t1, in1=ab)

        # ---------- DR projection helpers (self-attn, fp8 x64) ----------
        def proj_dr8(psP, w_t, h_v, npair, out_mt, ncols, cb):
            cw = min(NCH, ncols)
            for mt in range(out_mt):
                for cc in range(ncols // cw):
                    ps = psP.tile([P, cw], F32, tag="pp", name="pp")
                    for g in range(npair):
                        nc.tensor.matmul(
                            ps, lhsT=w_t[:, g, :, mt * P:(mt + 1) * P],
                            rhs=h_v[:, 2 * g:2 * g + 2, cc * cw:(cc + 1) * cw],
                            start=(g == 0), stop=(g == npair - 1),
                            perf_mode=DR)
                    cb(mt, cc, cw, ps)

        def make_vt8(psP, pool, h_v, npair, w_t, jt, name):
            ps = psP.tile([P, INNER], F32, tag="pp", name="pp")
            for g in range(npair):
                nc.tensor.matmul(
                    ps, lhsT=h_v[:, 2 * g:2 * g + 2, jt * P:(jt + 1) * P],
                    rhs=w_t[:, g], start=(g == 0), stop=(g == npair - 1),
                    perf_mode=DR)
            vt = pool.tile([P, H, DH + 1], BF16, tag=f"vt{jt}", name=name)
            if jt % 2 == 0:
                nc.vector.tensor_copy(
                    out=vt[:, :, 0:DH],
                    in_=ps.rearrange("p (h d) -> p h d", h=H))
            else:
                nc.scalar.copy(
                    out=vt[:, :, 0:DH],
                    in_=ps.rearrange("p (h d) -> p h d", h=H))
            nc.vector.memset(vt[:, :, DH:DH + 1], 1.0)
            return vt

        # ---------- projection helper ----------
        def proj(psP, w_tiles, rhs_tiles, nkt, out_mt, ncols, cb):
            cw = min(NCH, ncols)
            for mt in range(out_mt):
                for cc in range(ncols // cw):
                    ps = psP.tile([P, cw], F32, tag="pp", name="pp")
                    for kt in range(nkt):
                        nc.tensor.matmul(
                            ps,
                            lhsT=w_tiles[kt][:, mt * P:(mt + 1) * P],
                            rhs=rhs_tiles[kt][:, cc * cw:(cc + 1) * cw],
                            start=(kt == 0), stop=(kt == nkt - 1))
                    cb(mt, cc, cw, ps)

        def make_vt(psP, pool, lhs_tiles, nkt, w_tiles, jt, name):
            ps = psP.tile([P, INNER], F32, tag="pp", name="pp")
            for kt in range(nkt):
                nc.tensor.matmul(
                    ps,
                    lhsT=lhs_tiles[kt][:, jt * P:(jt + 1) * P],
                    rhs=w_tiles[kt],
                    start=(kt == 0), stop=(kt == nkt - 1))
            vt = pool.tile([P, H, DH + 1], BF16, tag=f"vt{jt}", name=name)
            nc.vector.tensor_copy(
                out=vt[:, :, 0:DH],
                in_=ps.rearrange("p (h d) -> p h d", h=H))
            nc.vector.memset(vt[:, :, DH:DH + 1], 1.0)
            return vt

        # ---------- attention ----------
        def attn_ic(k_sb, vt_sb, q_sb, njt, dst, ic, psS, psO, ep, rp,
                    escale=1.0):
            for hp in range(H // 2):
                t = hp
                po = [psO.tile([P, NCH], F32, tag=f"po{i}", name=f"po{i}")
                      for i in range(2)]
                for jt in range(njt):
                    ps = psS.tile([P, 2 * NCH], F32, tag="ps", name="ps")
                    for hh in range(2):
                        nc.tensor.matmul(
                            ps[:, hh * NCH:(hh + 1) * NCH],
                            lhsT=k_sb[t][hh * DH:(hh + 1) * DH, jt * P:(jt + 1) * P],
                            rhs=q_sb[t][hh * DH:(hh + 1) * DH, ic * NCH:(ic + 1) * NCH],
                            start=True, stop=True)
                    e = ep.tile([P, 2 * NCH], BF16, tag="e", name="e")
                    nc.scalar.activation(out=e, in_=ps, func=AFT.Exp,
                                         scale=escale)
                    for hh in range(2):
                        h = 2 * hp + hh
                        nc.tensor.matmul(
                            po[hh][0:DH + 1, :],
                            lhsT=vt_sb[jt][:, h, :],
                            rhs=e[:, hh * NCH:(hh + 1) * NCH],
                            start=(jt == 0), stop=(jt == njt - 1))
                for hh in range(2):
                    rrow = rp.tile([1, NCH], BF16, tag="rrow", name="rrow")
                    nc.vector.reciprocal(out=rrow, in_=po[hh][DH:DH + 1, :])
                    # broadcast 1/denom into po's unused partitions 64..127
                    nc.tensor.matmul(po[hh][DH:2 * DH, :],
                                     lhsT=ones_row[0:1, 0:DH], rhs=rrow,
                                     start=True, stop=True)
                    un = rp.tile([DH, NCH], BF16, tag="un", name="un")
                    if njt == JT2:
                        nc.scalar.copy(out=un, in_=po[hh][0:DH, :])
                    else:
                        nc.vector.tensor_copy(out=un, in_=po[hh][0:DH, :])
                    nc.vector.tensor_mul(
                        out=dst[t][hh * DH:(hh + 1) * DH, ic * NCH:(ic + 1) * NCH],
                        in0=un, in1=po[hh][DH:2 * DH, :])

        # ---------- output-proj + residual (one ic chunk) ----------
        def wo_resid_ic(psP, wo_t, bias_t, ic, oscale=1.0):
            cs = slice(ic * NCH, (ic + 1) * NCH)
            for mt in range(CT):
                ps = psP.tile([P, NCH], F32, tag="pp", name="pp")
                for g in range(CPAIR):
                    nc.tensor.matmul(ps, lhsT=wo_t[:, g, :, mt * P:(mt + 1) * P],
                                     rhs=attnOv[:, 2 * g:2 * g + 2, cs],
                                     start=(g == 0), stop=(g == CPAIR - 1),
                                     perf_mode=DR)
                t1 = tp.tile([P, NCH], F32, tag="t1", name="t1")
                nc.scalar.activation(out=t1, in_=ps, func=AFT.Identity,
                                     scale=oscale, bias=bias_t[:, mt:mt + 1])
                nc.vector.tensor_add(out=xres[mt][:, cs], in0=t1,
                                     in1=xres[mt][:, cs])

        # ================= phase 1: LN1 over the full sequence =================
        h1p_cm = tc.tile_pool(name="h1p", bufs=1)
        h1p = h1p_cm.__enter__()
        h1t = h1p.tile([P, CT * N], FP8, tag="h1t", name="h1t")
        h1 = layernorm(h1p, xfull, N, "1", dst=h1t)
        h1v = h1t.rearrange("p (kt n) -> p kt n", kt=CT)

        # ============= phase 2: Q/K/V projections (self) =============
        q1_sb = [sa.tile([P, NL], BF16, tag=f"q{t}", name=f"q1_{t}") for t in range(IT)]
        k1_sb = [sa.tile([P, N], BF16, tag=f"k{t}", name=f"k1_{t}") for t in range(IT)]
        with tc.tile_pool(name="psP1", bufs=4, space="PSUM") as psP:
            proj_dr8(psP, wq1d, h1v[:, :, 0:NL].rearrange("p k n -> p k n"),
                     CPAIR, IT, NL,
                     lambda mt, cc, cw, ps: nc.vector.tensor_copy(
                         out=q1_sb[mt][:, cc * cw:(cc + 1) * cw], in_=ps))
            def _k1cb(mt, cc, cw, ps):
                if cc % 2 == 0:
                    nc.vector.tensor_copy(
                        out=k1_sb[mt][:, cc * cw:(cc + 1) * cw], in_=ps)
                else:
                    nc.scalar.copy(
                        out=k1_sb[mt][:, cc * cw:(cc + 1) * cw], in_=ps)
            proj_dr8(psP, wk1d, h1v, CPAIR, IT, N, _k1cb)
            vt1_sb = [make_vt8(psP, sa, h1v, CPAIR, wv1d, jt, f"vt1_{jt}")
                      for jt in range(JT1)]
            # cross-attn K2/V2T depend only on ctx: emit early so the PE work
            # fills self-attention's ACT-bound phase
            k2_sb = [ca.tile([P, MCTX], BF16, tag=f"k{t}", name=f"k2_{t}")
                     for t in range(IT)]
            proj_dr8(psP, wk2d, ctxv, XPAIR, IT, MCTX,
                     lambda mt, cc, cw, ps: nc.scalar.copy(
                         out=k2_sb[mt][:, cc * cw:(cc + 1) * cw], in_=ps))
            vt2_sb = [make_vt8(psP, ca, ctxv, XPAIR, wv2d, jt, f"vt2_{jt}")
                      for jt in range(JT2)]
        if DEBUG:
            for kt in range(CT):
                nc.sync.dma_start(out=nc.dbg["d_h1"][kt * P:(kt + 1) * P, :], in_=h1[kt])
                nc.sync.dma_start(out=nc.dbg["d_q1"][kt * P:(kt + 1) * P, :], in_=q1_sb[kt])
                nc.sync.dma_start(out=nc.dbg["d_k1"][kt * P:(kt + 1) * P, :], in_=k1_sb[kt])
        h1p_cm.__exit__(None, None, None)
        xfp_cm.__exit__(None, None, None)

        # ===== phase 3: self-attention =====
        with tc.tile_pool(name="psS", bufs=2, space="PSUM") as psS, \
             tc.tile_pool(name="psO", bufs=2, space="PSUM") as psO, \
             tc.tile_pool(name="ep", bufs=6) as ep, \
             tc.tile_pool(name="rp", bufs=4) as rp:
            for ic in range(ICN):
                attn_ic(k1_sb, vt1_sb, q1_sb, JT1, attnO, ic, psS, psO,
                        ep, rp, escale=SEXP)
        sa_cm.__exit__(None, None, None)
        wffp_cm = tc.tile_pool(name="wffp", bufs=1, side="right")
        wffp = wffp_cm.__enter__()
        wff1d = wffp.tile([P, CPAIR, 2, 2 * FFI], FP8, tag="wff1t",
                          name="wff1t")
        nc.sync.dma_start(out=wff1d.rearrange("p a b c -> p (a b c)"),
                          in_=w_d["wff1t"][:])
        wff2d = wffp.tile([P, FPAIR, 2, C], FP8, tag="wff2t", name="wff2t")
        nc.sync.dma_start(out=wff2d.rearrange("p a b c -> p (a b c)"),
                          in_=w_d["wff2t"][:])
        ffh_t = wffp.tile([P, 2, NCH], FP8, tag="ffh_t", name="ffh_t")

        # ===== phase 4: Wo1 + residual =====
        with tc.tile_pool(name="psP2", bufs=4, space="PSUM") as psP:
            for ic in range(ICN):
                wo_resid_ic(psP, wo1d, bo1_t, ic, oscale=1.0 / (SW * SW))

        # ===== phase 5: LN2 + Q2 =====
        h2t = ca.tile([P, CT * NL], FP8, tag="h2t", name="h2t")
        h2 = layernorm(ca, xres, NL, "2", xb_src=xres, dst=h2t)
        h2v = h2t.rearrange("p (kt n) -> p kt n", kt=CT)
        q2_sb = [ca.tile([P, NL], BF16, tag=f"q{t}", name=f"q2_{t}")
                 for t in range(IT)]
        with tc.tile_pool(name="psP3", bufs=4, space="PSUM") as psP:
            def _q2cb(mt, cc, cw, ps):
                if (mt + cc) % 2 == 0:
                    nc.vector.tensor_copy(
                        out=q2_sb[mt][:, cc * cw:(cc + 1) * cw], in_=ps)
                else:
                    nc.scalar.copy(
                        out=q2_sb[mt][:, cc * cw:(cc + 1) * cw], in_=ps)
            proj_dr8(psP, wq2d, h2v, CPAIR, IT, NL, _q2cb)

        # ===== phase 6: cross-attention =====
        with tc.tile_pool(name="psS2", bufs=2, space="PSUM") as psS, \
             tc.tile_pool(name="psO2", bufs=2, space="PSUM") as psO, \
             tc.tile_pool(name="ep2", bufs=6) as ep, \
             tc.tile_pool(name="rp2", bufs=4) as rp:
            for ic in range(ICN):
                attn_ic(k2_sb, vt2_sb, q2_sb, JT2, attnO, ic, psS, psO,
                        ep, rp, escale=SEXP)

        # ===== phase 7: Wo2 + residual, then LN3 =====
        with tc.tile_pool(name="psP4", bufs=4, space="PSUM") as psP:
            for ic in range(ICN):
                wo_resid_ic(psP, wo2d, bo2_t, ic, oscale=1.0 / (SW * SW))
        h3t = ca.tile([P, CT * NL], FP8, tag="h3t", name="h3t")
        h3 = layernorm(ca, xres, NL, "3", xb_src=xres, dst=h3t)
        h3p = h3t.rearrange("p (kt n) -> p kt n", kt=CT)

        # ============= phase 8: GEGLU FF =============
        if DEBUG:
            for kt in range(CT):
                nc.sync.dma_start(out=nc.dbg["d_h3"][kt * P:(kt + 1) * P, :], in_=h3[kt])
        with tc.tile_pool(name="psY", bufs=1, space="PSUM") as psY, \
             tc.tile_pool(name="psF", bufs=2, space="PSUM") as psF, \
             tc.tile_pool(name="gp", bufs=3) as gp, \
             tc.tile_pool(name="op", bufs=3) as op:
            for ic in range(ICN):
                cs3 = slice(ic * NCH, (ic + 1) * NCH)
                pys = [psY.tile([P, NCH], F32, tag=f"y{m}", name=f"y{m}")
                       for m in range(CT)]
                for pi in range(FT):
                    ph = psF.tile([P, NCH], F32, tag="ph", name="ph")
                    pg = psF.tile([P, NCH], F32, tag="pg", name="pg")
                    for g in range(CPAIR):
                        nc.tensor.matmul(
                            ph,
                            lhsT=wff1d[:, g, :, pi * P:(pi + 1) * P],
                            rhs=h3p[:, 2 * g:2 * g + 2, cs3],
                            start=(g == 0), stop=(g == CPAIR - 1),
                            perf_mode=DR)
                    for g in range(CPAIR):
                        nc.tensor.matmul(
                            pg,
                            lhsT=wff1d[:, g, :, FFI + pi * P:FFI + (pi + 1) * P],
                            rhs=h3p[:, 2 * g:2 * g + 2, cs3],
                            start=(g == 0), stop=(g == CPAIR - 1),
                            perf_mode=DR)
                    gel = gp.tile([P, NCH], BF16, tag="gel", name="gel")
                    nc.scalar.activation(out=gel, in_=pg, func=AFT.Gelu,
                                         scale=1.0 / SW,
                                         bias=bff1_t[:, FT + pi:FT + pi + 1])
                    hb = tp.tile([P, NCH], F32, tag="hb", name="hb")
                    nc.scalar.activation(out=hb, in_=ph, func=AFT.Identity,
                                         scale=1.0 / SW,
                                         bias=bff1_t[:, pi:pi + 1])
                    nc.vector.tensor_mul(out=ffh_t[:, pi % 2], in0=hb,
                                         in1=gel)
                    if pi % 2 == 1:
                        g2 = pi // 2
                        for mt in range(CT):
                            nc.tensor.matmul(
                                pys[mt],
                                lhsT=wff2d[:, g2, :, mt * P:(mt + 1) * P],
                                rhs=ffh_t,
                                start=(g2 == 0), stop=(g2 == FPAIR - 1),
                                perf_mode=DR)
                for mt in range(CT):
                    t1 = tp.tile([P, NCH], F32, tag="t1", name="t1")
                    nc.scalar.activation(out=t1, in_=pys[mt], func=AFT.Identity,
                                         scale=1.0 / SW,
                                         bias=bff2_t[:, mt:mt + 1])
                    ot = op.tile([P, NCH], F32, tag="ot", name="ot")
                    nc.vector.tensor_add(out=ot, in0=t1,
                                         in1=xres[mt][:, ic * NCH:(ic + 1) * NCH])
                    nc.sync.dma_start(
                        out=out_d[mt * P:(mt + 1) * P, ic * NCH:(ic + 1) * NCH],
                        in_=ot)
        ca_cm.__exit__(None, None, None)
        wffp_cm.__exit__(None, None, None)


def _split_multi_waits(nc):
    """This walrus build accepts at most one sem-wait per instruction; Tile
    emits several. Split extras into standalone InstEventSemaphore pre-waits
    on the same engine (engines execute their stream in order, so semantics
    are preserved)."""
    n = 0
    for fn in nc.m.functions:
        for blk in fn.blocks:
            out = []
            for inst in blk.instructions:
                si = inst.sync_info
                if si is not None and si.on_wait and len(si.on_wait) > 1:
                    waits = list(si.on_wait)
                    for i, w in enumerate(waits[:-1]):
                        out.append(mybir.InstEventSemaphore(
                            name=f"{inst.name}-w{i}",
                            engine=inst.engine,
                            sync_info=mybir.SyncInfo(on_wait=[w], on_update=[]),
                        ))
                        n += 1
                    inst.sync_info = mybir.SyncInfo(
                        on_wait=[waits[-1]], on_update=list(si.on_update))
                out.append(inst)
            blk.instructions = out
    return n


def _build():
    nc = bass.Bass()
    nc.x_d = nc.dram_tensor("x", [C, NL], BF16, kind="ExternalInput")
    nc.xb_d = nc.dram_tensor("xb", [C, N], BF16, kind="ExternalInput")
    nc.ctx_d = nc.dram_tensor("ctx", [CTXC, MCTX], FP8, kind="ExternalInput")
    nc.w_d = {}
    for name in ("wq1t", "wk1t", "wv1t", "wq2t"):
        nc.w_d[name] = nc.dram_tensor(name, [P, CPAIR * 2 * INNER], FP8,
                                      kind="ExternalInput")
    for name in ("wo1t", "wo2t"):
        nc.w_d[name] = nc.dram_tensor(name, [P, CPAIR * 2 * C], FP8,
                                      kind="ExternalInput")
    for name in ("wk2t", "wv2t"):
        nc.w_d[name] = nc.dram_tensor(name, [P, XPAIR * 2 * INNER], FP8,
                                      kind="ExternalInput")
    nc.w_d["wff1t"] = nc.dram_tensor("wff1t", [P, CPAIR * 2 * 2 * FFI], FP8,
                                     kind="ExternalInput")
    nc.w_d["wff2t"] = nc.dram_tensor("wff2t", [P, FPAIR * 2 * C], FP8,
                                     kind="ExternalInput")
    nc.b_d = {}
    for name, n in [("bo1", C), ("bo2", C), ("bff1", 2 * FFI), ("bff2", C)]:
        nc.b_d[name] = nc.dram_tensor(name, [n], F32, kind="ExternalInput")
    nc.out_d = nc.dram_tensor("out", [C, NL], F32, kind="ExternalOutput")
    nc.dbg = {}
    if DEBUG:
        for name, shape, dt in [
            ("d_h1", [C, N], BF16), ("d_q1", [C, NL], BF16),
            ("d_k1", [C, N], BF16), ("d_attnO1", [C, NL], BF16),
            ("d_x1", [C, NL], F32), ("d_x2", [C, NL], F32),
            ("d_h3", [C, NL], BF16),
        ]:
            nc.dbg[name] = nc.dram_tensor(name, shape, dt, kind="ExternalOutput")
    with tile.TileContext(nc) as tc:
        _emit(tc)
    _split_multi_waits(nc)
    return nc


_CACHE = {}


def _get_program():
    if "nc" not in _CACHE:
        _CACHE["nc"] = _build()
    return _CACHE["nc"]


def _dr_weight(A, npair):
    """A: [K, M] f32 (already scaled). Returns [128, npair*2*M] fp8 in
    DoubleRow layout: out[p, g, i, m] = A[(2g+i)*128+p, m]."""
    K, M = A.shape
    assert K == npair * 2 * P
    t = A.reshape(npair, 2, P, M).transpose(2, 0, 1, 3)
    return np.ascontiguousarray(t.reshape(P, npair * 2 * M)).astype(E4NP)


def _prep_shared(inputs):
    f32 = np.float32
    g1 = np.asarray(inputs["g1"], f32)
    g2 = np.asarray(inputs["g2"], f32)
    g3 = np.asarray(inputs["g3"], f32)
    scale = DH ** -0.5
    d = {
        "wq1t": _dr_weight(
            (np.asarray(inputs["Wq1"], f32) * scale * g1[None, :]).T * SW,
            CPAIR),
        "wk1t": _dr_weight(
            (np.asarray(inputs["Wk1"], f32) * g1[None, :]).T * SW, CPAIR),
        "wv1t": _dr_weight(
            (np.asarray(inputs["Wv1"], f32) * g1[None, :]).T * SW, CPAIR),
        "wo1t": _dr_weight(np.asarray(inputs["Wo1"], f32).T * SW, CPAIR),
        "wq2t": _dr_weight(
            (np.asarray(inputs["Wq2"], f32) * scale * g2[None, :]).T * SW,
            CPAIR),
        "wk2t": _dr_weight(np.asarray(inputs["Wk2"], f32).T * SW, XPAIR),
        "wv2t": _dr_weight(np.asarray(inputs["Wv2"], f32).T * SW, XPAIR),
        "wo2t": _dr_weight(np.asarray(inputs["Wo2"], f32).T * SW, CPAIR),
        "wff1t": _dr_weight(
            (np.asarray(inputs["Wff1"], f32) * g3[None, :]).T * SW, CPAIR),
        "wff2t": _dr_weight(np.asarray(inputs["Wff2"], f32).T * SW, FPAIR),
        "bo1": np.ascontiguousarray(np.asarray(inputs["bo1"], f32)),
        "bo2": np.ascontiguousarray(np.asarray(inputs["bo2"], f32)),
        "bff1": np.ascontiguousarray(np.asarray(inputs["bff1"], f32)),
        "bff2": np.ascontiguousarray(np.asarray(inputs["bff2"], f32)),
    }
    return d


def make_in_maps(inputs):
    x = np.asarray(inputs["x"], np.float32)
    ctxf = np.asarray(inputs["context"], np.float32)
    shared = _prep_shared(inputs)
    in_maps = []
    for core in range(8):
        b, s = core // 2, core % 2
        xb = x[b]
        if s:
            xc = np.ascontiguousarray(
                np.concatenate([xb[:, NL:], xb[:, :NL]], axis=1))
        else:
            xc = np.ascontiguousarray(xb)
        m = dict(shared)
        m["x"] = np.ascontiguousarray(xc[:, :NL]).astype(BF16NP)
        m["xb"] = xc.astype(BF16NP)
        m["ctx"] = np.ascontiguousarray(ctxf[b]).astype(E4NP)
        in_maps.append(m)
    return in_maps


def kernel(**inputs):
    nc = _get_program()
    in_maps = make_in_maps(inputs)
    res = run_bass_kernel_spmd(nc, in_maps, core_ids=list(range(8)))
    out = np.empty((B, C, N), np.float32)
    for core in range(8):
        b, s = core // 2, core % 2
        out[b][:, s * NL:(s + 1) * NL] = res.results[core]["out"]
    return out



# revision 44
# speedup vs baseline: 1.2906x; 1.0145x over previous
"""Trainium2 Bass kernel for a BasicTransformerBlock (self-attn + cross-attn + GEGLU FF).

Sharding: 8 cores = (batch b in 0..3) x (sequence half s in 0..1). No collectives.
Each core receives the full x[b] [512, 2048] (rotated so its local half is always
columns 0..1023), builds self-attention K/V over all 2048 positions, and computes
LN/Q/attention/FF only for its local 1024 positions. Output [512, 1024] per core.

Numerics: bf16 matmuls with fp32 PSUM accumulation; LayerNorm gains folded into the
following weight matrices on the host; attention softmax computed without
max-subtraction (scores are bounded ~+-1.5 here); softmax denominator obtained by
augmenting V^T with a ones-column (row 64 of the AV output = sum_j exp).
"""

import os
import sys

import numpy as np

for _p in ("/opt/trn_rl_repo", "/root/.axon_site/_ro/trn_rl_repo"):
    if os.path.isdir(_p) and _p not in sys.path:
        sys.path.insert(0, _p)

import ml_dtypes

import concourse.bass as bass
import concourse.tile as tile
from concourse import mybir
from concourse.bass_utils import run_bass_kernel_spmd

BF16NP = ml_dtypes.bfloat16
E4NP = ml_dtypes.float8_e4m3
AFT = mybir.ActivationFunctionType
DR = mybir.MatmulPerfMode.DoubleRow
ALU = mybir.AluOpType
F32 = mybir.dt.float32
BF16 = mybir.dt.bfloat16
FP8 = mybir.dt.float8e4

# Problem dims (hardcoded per spec)
P = 128
B = 4
C = 512      # model dim
N = 2048     # full seq len
NL = 1024    # local seq len per core
CTXC = 768   # context channels
MCTX = 256   # context seq len
H = 8
DH = 64
INNER = 512
FFI = 2048
EPS = 1e-5

CT = C // P        # 4 channel tiles
IT = INNER // P    # 4 inner tiles
XT = CTXC // P     # 6 ctx channel tiles
XPAIR = XT // 2    # 3 ctx channel-tile pairs
FT = FFI // P      # 16 ff tiles
CPAIR = CT // 2    # 2 channel-tile pairs
FPAIR = FT // 2    # 8 ff-tile pairs
SW = 64.0          # fp8 weight pre-scale (FF block + self-attn QKV)
SEXP = 1.0 / (SW * SW)  # self-attn scores psum = 64q * 64k = 4096 * true
NCH = 512          # free-dim chunk size
ICN = NL // NCH    # 2 local i-chunks
JT1 = N // P       # 16 self-attn j tiles
JT2 = MCTX // P    # 2 cross-attn j tiles
DEBUG = False


def _emit(tc):
    nc = tc.nc
    from contextlib import ExitStack

    with ExitStack() as ctx:
        ctx.enter_context(nc.allow_low_precision(
            reason="bf16 rows/broadcasts validated end-to-end vs fp32 reference"))
        main = ctx.enter_context(tc.tile_pool(name="main", bufs=1))
        tp = ctx.enter_context(tc.tile_pool(name="tp", bufs=4))

        x_d = nc.x_d
        ctx_d = nc.ctx_d
        w_d = nc.w_d
        b_d = nc.b_d
        out_d = nc.out_d

        # ---- constants ----
        ones_col = main.tile([P, 1], F32, tag="ones_col", name="ones_col")
        nc.vector.memset(ones_col, 1.0)
        ones_col_bf = main.tile([P, 1], BF16, tag="ones_col_bf", name="ones_col_bf")
        nc.vector.memset(ones_col_bf, 1.0)
        ones_row = main.tile([1, P], BF16, tag="ones_row", name="ones_row")
        nc.vector.memset(ones_row, 1.0)
        eps_t = main.tile([P, 1], F32, tag="eps", name="eps")
        nc.vector.memset(eps_t, EPS)

        # ---- load weights (attention ones up-front; FF weights later) ----
        def load_split(pool, tag, dram, nkt, cols, dtype):
            """One wide DMA for a [nkt*128, cols] DRAM tensor into a single
            [128, nkt*cols] SBUF tile; returns per-kt [128, cols] views."""
            t = pool.tile([P, nkt * cols], dtype, tag=tag, name=tag)
            nc.sync.dma_start(
                out=t.rearrange("p (kt c) -> p kt c", kt=nkt),
                in_=dram.rearrange("(kt p) c -> p kt c", p=P))
            return [t[:, kt * cols:(kt + 1) * cols] for kt in range(nkt)]

        def load_w(pool, name, nkt, cols):
            return load_split(pool, name, w_d[name], nkt, cols, BF16)


        def load_bias(name, n):
            f = n // P
            t = main.tile([P, f], F32, tag=f"b_{name}", name=f"b_{name}")
            nc.sync.dma_start(out=t, in_=b_d[name].rearrange("(f p) -> p f", p=P))
            return t

        ca_cm = tc.tile_pool(name="ca", bufs=1)
        ca = ca_cm.__enter__()
        sa_cm = tc.tile_pool(name="sa", bufs=1)
        sa = sa_cm.__enter__()
        # ---- load activations (before weights: LN1 needs x first) ----
        xfp_cm = tc.tile_pool(name="xfull", bufs=1)
        xfp = xfp_cm.__enter__()
        # xfull: one [128, CT*N] tile, DMA'd in 4 column-chunks so LN1's
        # first chunk starts as soon as its slice lands
        xft = xfp.tile([P, CT * N], BF16, tag="xf", name="xf")
        _xf_nc = N // NCH
        for cc in range(_xf_nc):
            nc.sync.dma_start(
                out=xft.rearrange("p (kt nc c) -> p nc kt c", kt=CT,
                                  nc=_xf_nc)[:, cc],
                in_=nc.xb_d.rearrange("(kt p) (nc c) -> p nc kt c", p=P,
                                      nc=_xf_nc)[:, cc])
        xfull = [xft[:, kt * N:(kt + 1) * N] for kt in range(CT)]
        xres = load_split(main, "xres", x_d, CT, NL, BF16)
        ctx_t = main.tile([P, XT * MCTX], FP8, tag="ctx", name="ctx")
        nc.sync.dma_start(
            out=ctx_t.rearrange("p (kt c) -> p kt c", kt=XT),
            in_=ctx_d.rearrange("(kt p) c -> p kt c", p=P))
        ctxv = ctx_t.rearrange("p (kt m) -> p kt m", kt=XT)

        # biases + weights after activations so LN1's x tiles arrive first
        bo1_t = load_bias("bo1", C)
        bo2_t = load_bias("bo2", C)
        bff1_t = load_bias("bff1", 2 * FFI)
        bff2_t = load_bias("bff2", C)
        def load_w8(name, npair, cols):
            t = main.tile([P, npair, 2, cols], FP8, tag=name, name=name)
            nc.sync.dma_start(out=t.rearrange("p a b c -> p (a b c)"),
                              in_=w_d[name][:])
            return t

        wq1d = load_w8("wq1t", CPAIR, INNER)
        wk1d = load_w8("wk1t", CPAIR, INNER)
        wv1d = load_w8("wv1t", CPAIR, INNER)
        wo1d = load_w8("wo1t", CPAIR, C)
        wq2d = load_w8("wq2t", CPAIR, INNER)
        wk2d = load_w8("wk2t", XPAIR, INNER)
        wv2d = load_w8("wv2t", XPAIR, INNER)
        wo2d = load_w8("wo2t", CPAIR, C)

        attnOt = main.tile([P, IT * NL], FP8, tag="attnOt", name="attnOt")
        attnO = [attnOt[:, t * NL:(t + 1) * NL] for t in range(IT)]
        attnOv = attnOt.rearrange("p (kt n) -> p kt n", kt=IT)


        # ---------- LayerNorm ----------
        def layernorm(hpool, src_tiles, ncols, lnid, xb_src=None, dst=None):
            if dst is not None:
                h_out = [dst[:, kt * ncols:(kt + 1) * ncols]
                         for kt in range(CT)]
            else:
                h_out = []
                for kt in range(CT):
                    h_out.append(hpool.tile([P, ncols], BF16, tag=f"h{kt}",
                                            name=f"h{lnid}_{kt}"))
            with tc.tile_pool(name=f"psLN{lnid}", bufs=2, space="PSUM") as psLN, \
                 tc.tile_pool(name=f"psB{lnid}", bufs=2, space="PSUM") as psB, \
                 tc.tile_pool(name=f"st{lnid}", bufs=1) as st:
                rows = make_ln_rows(st, ncols)
                for cc in range(ncols // NCH):
                    ln_chunk(src_tiles, rows, cc * NCH, lnid, psLN, psB, h_out,
                             cc * NCH, xb_src=xb_src)
            return h_out

        def make_ln_rows(st, ncols):
            mean_row = st.tile([1, ncols], BF16, tag="mrow", name="mrow")
            msq_row = st.tile([1, ncols], F32, tag="qrow", name="qrow")
            var_row = st.tile([1, ncols], BF16, tag="vrow", name="vrow")
            a_row = st.tile([1, ncols], BF16, tag="arow", name="arow")
            return (mean_row, msq_row, var_row, a_row)

        def ln_chunk(src_tiles, rows, col0, lnid, psLN, psB, h_out, hcol0,
                     xb_src=None):
            """LN stats+normalize for one 512-column chunk.

            src cols [col0, col0+NCH) -> h_out cols [hcol0.., ..+NCH)."""
            mean_row, msq_row, var_row, a_row = rows
            src_f32 = src_tiles[0].dtype == F32
            cs = slice(col0, col0 + NCH)
            rs = slice(hcol0, hcol0 + NCH)
            with tc.tile_pool(name=f"x2{lnid}c{col0}", bufs=3) as x2p:
                if xb_src is not None:
                    xb = [s[:, cs] for s in xb_src]
                elif src_f32:
                    xb = []
                    for kt in range(CT):
                        xbt = x2p.tile([P, NCH], BF16, tag="xb", name="xb")
                        nc.vector.tensor_copy(out=xbt, in_=src_tiles[kt][:, cs])
                        xb.append(xbt)
                else:
                    xb = [s[:, cs] for s in src_tiles]
                m_ps = psLN.tile([1, NCH], F32, tag="pp", name="m_ps")
                q_ps = psLN.tile([1, NCH], F32, tag="pp", name="q_ps")
                for kt in range(CT):
                    nc.tensor.matmul(m_ps, lhsT=ones_col_bf, rhs=xb[kt],
                                     start=(kt == 0), stop=(kt == CT - 1))
                for kt in range(CT):
                    x2 = x2p.tile([P, NCH], BF16, tag="x2", name="x2")
                    # gpsimd: both operands SBUF bf16; frees DVE for the
                    # normalize chain (gpsimd is otherwise idle)
                    nc.gpsimd.tensor_mul(out=x2, in0=xb[kt], in1=xb[kt])
                    nc.tensor.matmul(q_ps, lhsT=ones_col_bf, rhs=x2,
                                     start=(kt == 0), stop=(kt == CT - 1))
                nc.vector.tensor_scalar_mul(out=mean_row[0:1, rs], in0=m_ps,
                                            scalar1=1.0 / C)
                nc.vector.tensor_scalar_mul(out=msq_row[0:1, rs], in0=q_ps,
                                            scalar1=1.0 / C)
                nc.vector.tensor_mul(out=var_row[0:1, rs], in0=mean_row[0:1, rs],
                                     in1=mean_row[0:1, rs])
                nc.vector.tensor_sub(out=var_row[0:1, rs], in0=msq_row[0:1, rs],
                                     in1=var_row[0:1, rs])
                nc.scalar.activation(out=var_row[0:1, rs], in_=var_row[0:1, rs],
                                     func=AFT.Sqrt, bias=eps_t[0:1, 0:1])
                nc.vector.reciprocal(out=a_row[0:1, rs], in_=var_row[0:1, rs])
                mb = psB.tile([P, NCH], F32, tag="pp", name="mb")
                ab = psB.tile([P, NCH], F32, tag="pp", name="ab")
                nc.tensor.matmul(mb, lhsT=ones_row, rhs=mean_row[0:1, rs],
                                 start=True, stop=True)
                nc.tensor.matmul(ab, lhsT=ones_row, rhs=a_row[0:1, rs],
                                 start=True, stop=True)
                for kt in range(CT):
                    t1 = tp.tile([P, NCH], F32, tag="t1", name="t1")
                    nc.vector.tensor_sub(out=t1, in0=src_tiles[kt][:, cs], in1=mb)
                    nc.vector.tensor_mul(out=h_out[kt][:, rs], in0=t1, in1=ab)

        # ---------- DR projection helpers (self-attn, fp8 x64) ----------
        def proj_dr8(psP, w_t, h_v, npair, out_mt, ncols, cb):
            cw = min(NCH, ncols)
            for mt in range(out_mt):
                for cc in range(ncols // cw):
                    ps = psP.tile([P, cw], F32, tag="pp", name="pp")
                    for g in range(npair):
                        nc.tensor.matmul(
                            ps, lhsT=w_t[:, g, :, mt * P:(mt + 1) * P],
                            rhs=h_v[:, 2 * g:2 * g + 2, cc * cw:(cc + 1) * cw],
                            start=(g == 0), stop=(g == npair - 1),
                            perf_mode=DR)
                    cb(mt, cc, cw, ps)

        def make_vt8(psP, pool, h_v, npair, w_t, jt, name):
            ps = psP.tile([P, INNER], F32, tag="pp", name="pp")
            for g in range(npair):
                nc.tensor.matmul(
                    ps, lhsT=h_v[:, 2 * g:2 * g + 2, jt * P:(jt + 1) * P],
                    rhs=w_t[:, g], start=(g == 0), stop=(g == npair - 1),
                    perf_mode=DR)
            vt = pool.tile([P, H, DH + 1], BF16, tag=f"vt{jt}", name=name)
            if jt % 2 == 0:
                nc.vector.tensor_copy(
                    out=vt[:, :, 0:DH],
                    in_=ps.rearrange("p (h d) -> p h d", h=H))
            else:
                nc.scalar.copy(
                    out=vt[:, :, 0:DH],
                    in_=ps.rearrange("p (h d) -> p h d", h=H))
            nc.vector.memset(vt[:, :, DH:DH + 1], 1.0)
            return vt

        # ---------- projection helper ----------
        def proj(psP, w_tiles, rhs_tiles, nkt, out_mt, ncols, cb):
            cw = min(NCH, ncols)
            for mt in range(out_mt):
                for cc in range(ncols // cw):
                    ps = psP.tile([P, cw], F32, tag="pp", name="pp")
                    for kt in range(nkt):
                        nc.tensor.matmul(
                            ps,
                            lhsT=w_tiles[kt][:, mt * P:(mt + 1) * P],
                            rhs=rhs_tiles[kt][:, cc * cw:(cc + 1) * cw],
                            start=(kt == 0), stop=(kt == nkt - 1))
                    cb(mt, cc, cw, ps)

        def make_vt(psP, pool, lhs_tiles, nkt, w_tiles, jt, name):
            ps = psP.tile([P, INNER], F32, tag="pp", name="pp")
            for kt in range(nkt):
                nc.tensor.matmul(
                    ps,
                    lhsT=lhs_tiles[kt][:, jt * P:(jt + 1) * P],
                    rhs=w_tiles[kt],
                    start=(kt == 0), stop=(kt == nkt - 1))
            vt = pool.tile([P, H, DH + 1], BF16, tag=f"vt{jt}", name=name)
            nc.vector.tensor_copy(
                out=vt[:, :, 0:DH],
                in_=ps.rearrange("p (h d) -> p h d", h=H))
            nc.vector.memset(vt[:, :, DH:DH + 1], 1.0)
            return vt

        # ---------- attention ----------
        def attn_ic(k_sb, vt_sb, q_sb, njt, dst, ic, psS, psO, ep, rp,
                    escale=1.0):
            for hp in range(H // 2):
                t = hp
                po = [psO.tile([P, NCH], F32, tag=f"po{i}", name=f"po{i}")
                      for i in range(2)]
                for jt in range(njt):
                    ps = psS.tile([P, 2 * NCH], F32, tag="ps", name="ps")
                    for hh in range(2):
                        nc.tensor.matmul(
                            ps[:, hh * NCH:(hh + 1) * NCH],
                            lhsT=k_sb[t][hh * DH:(hh + 1) * DH, jt * P:(jt + 1) * P],
                            rhs=q_sb[t][hh * DH:(hh + 1) * DH, ic * NCH:(ic + 1) * NCH],
                            start=True, stop=True)
                    e = ep.tile([P, 2 * NCH], BF16, tag="e", name="e")
                    nc.scalar.activation(out=e, in_=ps, func=AFT.Exp,
                                         scale=escale)
                    for hh in range(2):
                        h = 2 * hp + hh
                        nc.tensor.matmul(
                            po[hh][0:DH + 1, :],
                            lhsT=vt_sb[jt][:, h, :],
                            rhs=e[:, hh * NCH:(hh + 1) * NCH],
                            start=(jt == 0), stop=(jt == njt - 1))
                for hh in range(2):
                    rrow = rp.tile([1, NCH], BF16, tag="rrow", name="rrow")
                    nc.vector.reciprocal(out=rrow, in_=po[hh][DH:DH + 1, :])
                    # broadcast 1/denom into po's unused partitions 64..127
                    nc.tensor.matmul(po[hh][DH:2 * DH, :],
                                     lhsT=ones_row[0:1, 0:DH], rhs=rrow,
                                     start=True, stop=True)
                    un = rp.tile([DH, NCH], BF16, tag="un", name="un")
                    if njt == JT2:
                        nc.scalar.copy(out=un, in_=po[hh][0:DH, :])
                    else:
                        nc.vector.tensor_copy(out=un, in_=po[hh][0:DH, :])
                    nc.vector.tensor_mul(
                        out=dst[t][hh * DH:(hh + 1) * DH, ic * NCH:(ic + 1) * NCH],
                        in0=un, in1=po[hh][DH:2 * DH, :])

        # ---------- output-proj + residual (one ic chunk) ----------
        def wo_resid_ic(psP, wo_t, bias_t, ic, oscale=1.0):
            cs = slice(ic * NCH, (ic + 1) * NCH)
            for mt in range(CT):
                ps = psP.tile([P, NCH], F32, tag="pp", name="pp")
                for g in range(CPAIR):
                    nc.tensor.matmul(ps, lhsT=wo_t[:, g, :, mt * P:(mt + 1) * P],
                                     rhs=attnOv[:, 2 * g:2 * g + 2, cs],
                                     start=(g == 0), stop=(g == CPAIR - 1),
                                     perf_mode=DR)
                t1 = tp.tile([P, NCH], F32, tag="t1", name="t1")
                nc.scalar.activation(out=t1, in_=ps, func=AFT.Identity,
                                     scale=oscale, bias=bias_t[:, mt:mt + 1])
                nc.vector.tensor_add(out=xres[mt][:, cs], in0=t1,
                                     in1=xres[mt][:, cs])

        # ================= phase 1: LN1 over the full sequence =================
        h1p_cm = tc.tile_pool(name="h1p", bufs=1)
        h1p = h1p_cm.__enter__()
        h1t = h1p.tile([P, CT * N], FP8, tag="h1t", name="h1t")
        h1 = layernorm(h1p, xfull, N, "1", dst=h1t)
        h1v = h1t.rearrange("p (kt n) -> p kt n", kt=CT)

        # ============= phase 2: Q/K/V projections (self) =============
        q1_sb = [sa.tile([P, NL], BF16, tag=f"q{t}", name=f"q1_{t}") for t in range(IT)]
        k1_sb = [sa.tile([P, N], BF16, tag=f"k{t}", name=f"k1_{t}") for t in range(IT)]
        with tc.tile_pool(name="psP1", bufs=4, space="PSUM") as psP:
            proj_dr8(psP, wq1d, h1v[:, :, 0:NL].rearrange("p k n -> p k n"),
                     CPAIR, IT, NL,
                     lambda mt, cc, cw, ps: nc.vector.tensor_copy(
                         out=q1_sb[mt][:, cc * cw:(cc + 1) * cw], in_=ps))
            def _k1cb(mt, cc, cw, ps):
                if cc % 2 == 0:
                    nc.vector.tensor_copy(
                        out=k1_sb[mt][:, cc * cw:(cc + 1) * cw], in_=ps)
                else:
                    nc.scalar.copy(
                        out=k1_sb[mt][:, cc * cw:(cc + 1) * cw], in_=ps)
            proj_dr8(psP, wk1d, h1v, CPAIR, IT, N, _k1cb)
            vt1_sb = [make_vt8(psP, sa, h1v, CPAIR, wv1d, jt, f"vt1_{jt}")
                      for jt in range(JT1)]
            # cross-attn K2/V2T depend only on ctx: emit early so the PE work
            # fills self-attention's ACT-bound phase
            k2_sb = [ca.tile([P, MCTX], BF16, tag=f"k{t}", name=f"k2_{t}")
                     for t in range(IT)]
            proj_dr8(psP, wk2d, ctxv, XPAIR, IT, MCTX,
                     lambda mt, cc, cw, ps: nc.scalar.copy(
                         out=k2_sb[mt][:, cc * cw:(cc + 1) * cw], in_=ps))
            vt2_sb = [make_vt8(psP, ca, ctxv, XPAIR, wv2d, jt, f"vt2_{jt}")
                      for jt in range(JT2)]
        if DEBUG:
            for kt in range(CT):
                nc.sync.dma_start(out=nc.dbg["d_h1"][kt * P:(kt + 1) * P, :], in_=h1[kt])
                nc.sync.dma_start(out=nc.dbg["d_q1"][kt * P:(kt + 1) * P, :], in_=q1_sb[kt])
                nc.sync.dma_start(out=nc.dbg["d_k1"][kt * P:(kt + 1) * P, :], in_=k1_sb[kt])
        h1p_cm.__exit__(None, None, None)
        xfp_cm.__exit__(None, None, None)

        # ===== phase 3: self-attention =====
        with tc.tile_pool(name="psS", bufs=2, space="PSUM") as psS, \
             tc.tile_pool(name="psO", bufs=2, space="PSUM") as psO, \
             tc.tile_pool(name="ep", bufs=6) as ep, \
             tc.tile_pool(name="rp", bufs=4) as rp:
            for ic in range(ICN):
                attn_ic(k1_sb, vt1_sb, q1_sb, JT1, attnO, ic, psS, psO,
                        ep, rp, escale=SEXP)
        sa_cm.__exit__(None, None, None)
        wffp_cm = tc.tile_pool(name="wffp", bufs=1, side="right")
        wffp = wffp_cm.__enter__()
        wff1d = wffp.tile([P, CPAIR, 2, 2 * FFI], FP8, tag="wff1t",
                          name="wff1t")
        nc.sync.dma_start(out=wff1d.rearrange("p a b c -> p (a b c)"),
                          in_=w_d["wff1t"][:])
        wff2d = wffp.tile([P, FPAIR, 2, C], FP8, tag="wff2t", name="wff2t")
        nc.sync.dma_start(out=wff2d.rearrange("p a b c -> p (a b c)"),
                          in_=w_d["wff2t"][:])
        ffh_t = wffp.tile([P, 2, NCH], FP8, tag="ffh_t", name="ffh_t")

        # ===== phase 4: Wo1 + residual =====
        with tc.tile_pool(name="psP2", bufs=4, space="PSUM") as psP:
            for ic in range(ICN):
                wo_resid_ic(psP, wo1d, bo1_t, ic, oscale=1.0 / (SW * SW))

        # ===== phase 5: LN2 + Q2 =====
        h2t = ca.tile([P, CT * NL], FP8, tag="h2t", name="h2t")
        h2 = layernorm(ca, xres, NL, "2", xb_src=xres, dst=h2t)
        h2v = h2t.rearrange("p (kt n) -> p kt n", kt=CT)
        q2_sb = [ca.tile([P, NL], BF16, tag=f"q{t}", name=f"q2_{t}")
                 for t in range(IT)]
        with tc.tile_pool(name="psP3", bufs=4, space="PSUM") as psP:
            def _q2cb(mt, cc, cw, ps):
                if (mt + cc) % 2 == 0:
                    nc.vector.tensor_copy(
                        out=q2_sb[mt][:, cc * cw:(cc + 1) * cw], in_=ps)
                else:
                    nc.scalar.copy(
                        out=q2_sb[mt][:, cc * cw:(cc + 1) * cw], in_=ps)
            proj_dr8(psP, wq2d, h2v, CPAIR, IT, NL, _q2cb)

        # ===== phase 6: cross-attention =====
        with tc.tile_pool(name="psS2", bufs=2, space="PSUM") as psS, \
             tc.tile_pool(name="psO2", bufs=2, space="PSUM") as psO, \
             tc.tile_pool(name="ep2", bufs=6) as ep, \
             tc.tile_pool(name="rp2", bufs=4) as rp:
            for ic in range(ICN):
                attn_ic(k2_sb, vt2_sb, q2_sb, JT2, attnO, ic, psS, psO,
                        ep, rp, escale=SEXP)

        # ===== phase 7: Wo2 + residual, then LN3 =====
        with tc.tile_pool(name="psP4", bufs=4, space="PSUM") as psP:
            for ic in range(ICN):
                wo_resid_ic(psP, wo2d, bo2_t, ic, oscale=1.0 / (SW * SW))
        h3t = ca.tile([P, CT * NL], FP8, tag="h3t", name="h3t")
        h3 = layernorm(ca, xres, NL, "3", xb_src=xres, dst=h3t)
        h3p = h3t.rearrange("p (kt n) -> p kt n", kt=CT)

        # ============= phase 8: GEGLU FF =============
        if DEBUG:
            for kt in range(CT):
                nc.sync.dma_start(out=nc.dbg["d_h3"][kt * P:(kt + 1) * P, :], in_=h3[kt])
        with tc.tile_pool(name="psY", bufs=1, space="PSUM") as psY, \
             tc.tile_pool(name="psF", bufs=2, space="PSUM") as psF, \
             tc.tile_pool(name="gp", bufs=3) as gp, \
             tc.tile_pool(name="op", bufs=3) as op:
            for ic in range(ICN):
                cs3 = slice(ic * NCH, (ic + 1) * NCH)
                pys = [psY.tile([P, NCH], F32, tag=f"y{m}", name=f"y{m}")
                       for m in range(CT)]
                for pi in range(FT):
                    ph = psF.tile([P, NCH], F32, tag="ph", name="ph")
                    pg = psF.tile([P, NCH], F32, tag="pg", name="pg")
                    for g in range(CPAIR):
                        nc.tensor.matmul(
                            ph,
                            lhsT=wff1d[:, g, :, pi * P:(pi + 1) * P],
                            rhs=h3p[:, 2 * g:2 * g + 2, cs3],
                            start=(g == 0), stop=(g == CPAIR - 1),
                            perf_mode=DR)
                    for g in range(CPAIR):
                        nc.tensor.matmul(
                            pg,
                            lhsT=wff1d[:, g, :, FFI + pi * P:FFI + (pi + 1) * P],
                            rhs=h3p[:, 2 * g:2 * g + 2, cs3],
                            start=(g == 0), stop=(g == CPAIR - 1),
                            perf_mode=DR)
                    gel = gp.tile([P, NCH], BF16, tag="gel", name="gel")
                    nc.scalar.activation(out=gel, in_=pg, func=AFT.Gelu,
                                         scale=1.0 / SW,
                                         bias=bff1_t[:, FT + pi:FT + pi + 1])
                    hb = tp.tile([P, NCH], F32, tag="hb", name="hb")
                    if pi % 2 == 0:
                        nc.vector.tensor_scalar(out=hb, in0=ph,
                                                scalar1=1.0 / SW,
                                                scalar2=bff1_t[:, pi:pi + 1],
                                                op0=ALU.mult, op1=ALU.add)
                    else:
                        nc.scalar.activation(out=hb, in_=ph, func=AFT.Identity,
                                             scale=1.0 / SW,
                                             bias=bff1_t[:, pi:pi + 1])
                    nc.vector.tensor_mul(out=ffh_t[:, pi % 2], in0=hb,
                                         in1=gel)
                    if pi % 2 == 1:
                        g2 = pi // 2
                        for mt in range(CT):
                            nc.tensor.matmul(
                                pys[mt],
                                lhsT=wff2d[:, g2, :, mt * P:(mt + 1) * P],
                                rhs=ffh_t,
                                start=(g2 == 0), stop=(g2 == FPAIR - 1),
                                perf_mode=DR)
                for mt in range(CT):
                    t1 = tp.tile([P, NCH], F32, tag="t1", name="t1")
                    nc.scalar.activation(out=t1, in_=pys[mt], func=AFT.Identity,
                                         scale=1.0 / SW,
                                         bias=bff2_t[:, mt:mt + 1])
                    ot = op.tile([P, NCH], F32, tag="ot", name="ot")
                    nc.vector.tensor_add(out=ot, in0=t1,
                                         in1=xres[mt][:, ic * NCH:(ic + 1) * NCH])
                    nc.sync.dma_start(
                        out=out_d[mt * P:(mt + 1) * P, ic * NCH:(ic + 1) * NCH],
                        in_=ot)
        ca_cm.__exit__(None, None, None)
        wffp_cm.__exit__(None, None, None)


def _split_multi_waits(nc):
    """This walrus build accepts at most one sem-wait per instruction; Tile
    emits several. Split extras into standalone InstEventSemaphore pre-waits
    on the same engine (engines execute their stream in order, so semantics
    are preserved)."""
    n = 0
    for fn in nc.m.functions:
        for blk in fn.blocks:
            out = []
            for inst in blk.instructions:
                si = inst.sync_info
                if si is not None and si.on_wait and len(si.on_wait) > 1:
                    waits = list(si.on_wait)
                    for i, w in enumerate(waits[:-1]):
                        out.append(mybir.InstEventSemaphore(
                            name=f"{inst.name}-w{i}",
                            engine=inst.engine,
                            sync_info=mybir.SyncInfo(on_wait=[w], on_update=[]),
                        ))
                        n += 1
                    inst.sync_info = mybir.SyncInfo(
                        on_wait=[waits[-1]], on_update=list(si.on_update))
                out.append(inst)
            blk.instructions = out
    return n


def _build():
    nc = bass.Bass()
    nc.x_d = nc.dram_tensor("x", [C, NL], BF16, kind="ExternalInput")
    nc.xb_d = nc.dram_tensor("xb", [C, N], BF16, kind="ExternalInput")
    nc.ctx_d = nc.dram_tensor("ctx", [CTXC, MCTX], FP8, kind="ExternalInput")
    nc.w_d = {}
    for name in ("wq1t", "wk1t", "wv1t", "wq2t"):
        nc.w_d[name] = nc.dram_tensor(name, [P, CPAIR * 2 * INNER], FP8,
                                      kind="ExternalInput")
    for name in ("wo1t", "wo2t"):
        nc.w_d[name] = nc.dram_tensor(name, [P, CPAIR * 2 * C], FP8,
                                      kind="ExternalInput")
    for name in ("wk2t", "wv2t"):
        nc.w_d[name] = nc.dram_tensor(name, [P, XPAIR * 2 * INNER], FP8,
                                      kind="ExternalInput")
    nc.w_d["wff1t"] = nc.dram_tensor("wff1t", [P, CPAIR * 2 * 2 * FFI], FP8,
                                     kind="ExternalInput")
    nc.w_d["wff2t"] = nc.dram_tensor("wff2t", [P, FPAIR * 2 * C], FP8,
                                     kind="ExternalInput")
    nc.b_d = {}
    for name, n in [("bo1", C), ("bo2", C), ("bff1", 2 * FFI), ("bff2", C)]:
        nc.b_d[name] = nc.dram_tensor(name, [n], F32, kind="ExternalInput")
    nc.out_d = nc.dram_tensor("out", [C, NL], F32, kind="ExternalOutput")
    nc.dbg = {}
    if DEBUG:
        for name, shape, dt in [
            ("d_h1", [C, N], BF16), ("d_q1", [C, NL], BF16),
            ("d_k1", [C, N], BF16), ("d_attnO1", [C, NL], BF16),
            ("d_x1", [C, NL], F32), ("d_x2", [C, NL], F32),
            ("d_h3", [C, NL], BF16),
        ]:
            nc.dbg[name] = nc.dram_tensor(name, shape, dt, kind="ExternalOutput")
    with tile.TileContext(nc) as tc:
        _emit(tc)
    _split_multi_waits(nc)
    return nc


_CACHE = {}


def _get_program():
    if "nc" not in _CACHE:
        _CACHE["nc"] = _build()
    return _CACHE["nc"]


def _dr_weight(A, npair):
    """A: [K, M] f32 (already scaled). Returns [128, npair*2*M] fp8 in
    DoubleRow layout: out[p, g, i, m] = A[(2g+i)*128+p, m]."""
    K, M = A.shape
    assert K == npair * 2 * P
    t = A.reshape(npair, 2, P, M).transpose(2, 0, 1, 3)
    return np.ascontiguousarray(t.reshape(P, npair * 2 * M)).astype(E4NP)


def _prep_shared(inputs):
    f32 = np.float32
    g1 = np.asarray(inputs["g1"], f32)
    g2 = np.asarray(inputs["g2"], f32)
    g3 = np.asarray(inputs["g3"], f32)
    scale = DH ** -0.5
    d = {
        "wq1t": _dr_weight(
            (np.asarray(inputs["Wq1"], f32) * scale * g1[None, :]).T * SW,
            CPAIR),
        "wk1t": _dr_weight(
            (np.asarray(inputs["Wk1"], f32) * g1[None, :]).T * SW, CPAIR),
        "wv1t": _dr_weight(
            (np.asarray(inputs["Wv1"], f32) * g1[None, :]).T * SW, CPAIR),
        "wo1t": _dr_weight(np.asarray(inputs["Wo1"], f32).T * SW, CPAIR),
        "wq2t": _dr_weight(
            (np.asarray(inputs["Wq2"], f32) * scale * g2[None, :]).T * SW,
            CPAIR),
        "wk2t": _dr_weight(np.asarray(inputs["Wk2"], f32).T * SW, XPAIR),
        "wv2t": _dr_weight(np.asarray(inputs["Wv2"], f32).T * SW, XPAIR),
        "wo2t": _dr_weight(np.asarray(inputs["Wo2"], f32).T * SW, CPAIR),
        "wff1t": _dr_weight(
            (np.asarray(inputs["Wff1"], f32) * g3[None, :]).T * SW, CPAIR),
        "wff2t": _dr_weight(np.asarray(inputs["Wff2"], f32).T * SW, FPAIR),
        "bo1": np.ascontiguousarray(np.asarray(inputs["bo1"], f32)),
        "bo2": np.ascontiguousarray(np.asarray(inputs["bo2"], f32)),
        "bff1": np.ascontiguousarray(np.asarray(inputs["bff1"], f32)),
        "bff2": np.ascontiguousarray(np.asarray(inputs["bff2"], f32)),
    }
    return d


def make_in_maps(inputs):
    x = np.asarray(inputs["x"], np.float32)
    ctxf = np.asarray(inputs["context"], np.float32)
    shared = _prep_shared(inputs)
    in_maps = []
    for core in range(8):
        b, s = core // 2, core % 2
        xb = x[b]
        if s:
            xc = np.ascontiguousarray(
                np.concatenate([xb[:, NL:], xb[:, :NL]], axis=1))
        else:
            xc = np.ascontiguousarray(xb)
        m = dict(shared)
        m["x"] = np.ascontiguousarray(xc[:, :NL]).astype(BF16NP)
        m["xb"] = xc.astype(BF16NP)
        m["ctx"] = np.ascontiguousarray(ctxf[b]).astype(E4NP)
        in_maps.append(m)
    return in_maps


def kernel(**inputs):
    nc = _get_program()
    in_maps = make_in_maps(inputs)
    res = run_bass_kernel_spmd(nc, in_maps, core_ids=list(range(8)))
    out = np.empty((B, C, N), np.float32)
    for core in range(8):
        b, s = core // 2, core % 2
        out[b][:, s * NL:(s + 1) * NL] = res.results[core]["out"]
    return out

